# revision 40
# baseline (speedup 1.0000x reference)
"""Trainium2 Bass kernel for nn_BasicRecurrentEntityEncoder.

Full-input contract: kernel(**inputs) takes the complete (unsharded) numpy
inputs and returns the full [B, K, D] float32 output. Internally the batch
is sharded over 8 NeuronCores (data parallel, no collectives), the embedding
bag-of-words gather runs through dma_gather against a per-core compacted
bf16 table, and the 64-step entity recurrence runs in a transposed
[D, (b,k)] layout with bf16 matmul operands.

Structure (625.8us baseline -> 457.2us):
  - Gather pipeline fully overlapped with the scan: groups 0-1 up front
    (group 0's dma split in two so the word-sum starts early, behind a PE
    p-state warm-up), groups 2-7 streamed under scan block g-2 with the
    word-sum spread 4 slot-matmuls per step. Dedicated gather PSUM banks
    (psG) keep it off the scan's PSUM tags.
  - e.keys gate logits precomputed per group ([128,512] matmul) with the
    sentence mask and block-diagonal batch mask folded in host-side as
    +-35 logit offsets (gk0m); per step they are matmul-injected into the
    gate bank, so the gate path is inject + 2 E_t^T h matmuls + one
    ScalarE exp. The exp(-logit) rows are broadcast-SUMMED by a ones8
    matmul (masked entries vanish), and the sigmoid reciprocal is fused
    into the custom DVE op MULSIG: u = relu(psh) * 1/(1+sum exp), with
    relu on ScalarE (HW allows only one PSUM operand per DVE op).
  - kvt = keys V is computed on the host; kvt + eW broadcasts are PSUM
    preloads (h-independent), U^T h accumulates on top.
  - upd/sq are 2x-mode bf16 tensor_tensor ops; norm rsqrt = exp(-.5 ln)
    on ScalarE with ln written back to PSUM (cheaper access), inv
    broadcast by a ones1 matmul, h_new = upd * inv on DVE.
  - Two batch groups (b 0-7 | b 8-15) with separate PSUM banks pipeline
    the serial dependency chain across engines.
"""

import sys

if "/opt/trn_rl_repo" not in sys.path:
    sys.path.insert(0, "/opt/trn_rl_repo")

import numpy as np
import ml_dtypes

from concourse import bacc, mybir
import concourse.bass as bass
import concourse.tile as tile
from concourse.bass_utils import run_bass_kernel_spmd
from concourse.masks import make_identity

# Force every ScalarE activation onto the one table set that covers all the
# functions this kernel uses (relu/exp/ln/copy/identity). The default
# chooser greedily picks the first set per function, inserting ~550ns table
# reloads on the critical path. Padding the dict keeps act_func_set_id
# indices aligned with act_info.json while making only the all-covering set
# usable.
_ONE_SET = "natural_log_exp_and_others"


import concourse.hw_specs as _hw_specs
_ORIG_TABLES = _hw_specs.get_activation_tables


def _patched_tables(module_arch):
    real = _ORIG_TABLES(module_arch)
    names = list(real.keys())
    assert _ONE_SET in names, names
    out = {}
    for n in names:
        if n == _ONE_SET:
            out[n] = real[n]
            break
        out[n] = set()
    return out


def _install_table_patch():
    import functools
    cached = functools.cache(_patched_tables)
    bacc.get_activation_tables = cached
    _hw_specs.get_activation_tables = cached


_install_table_patch()

# Custom DVE op #1: out ~= 1/(1 + in0) in ONE VectorE instruction (8 ALU
# stages): u = in0+1; seed y0 = bitcast(~bits(u)); t = u*y0 lands in
# [-4.5, -4] for any positive u; quadratic minimax fixup P(t) ~= 1/t gives
# out = y0*P(t) at ~1e-5 relative error.
import concourse.dve_ops as _dve_ops
from concourse.dve_spec import AluOp as _AluOp, Bin as _Bin, Spec as _Spec
from concourse.dve_spec import C0 as _C0, C1 as _C1, C2 as _C2, One as _One
from concourse.dve_spec import Src0 as _Src0, Src1 as _Src1, relu as _relu
from concourse.dve_spec import lower as _dve_lower
from concourse.dve_spec import _has_src1 as _dve_has_src1
from concourse.dve_uop import DveOpSpec as _DveOpSpec

# MULSIG fuses u = r * sigmoid(logit): in0 = relu(psh) (SBUF bf16, relu on
# ScalarE -- HW allows only ONE PSUM input per DVE op so the gate broadcast
# keeps the PSUM slot), in1 = sum_sent exp(-logit) from the ones8 matmul.
# out = in0 * 1/(1+in1) via the bitwise-NOT seed and a LINEAR fixup
# P(t) = c0 + c1*t on t in [-4.5, -4] (max sigmoid abs err ~1.4e-3).
_RS_C0, _RS_C1 = -0.47250233, -0.05572371


def _mulsig_ref(in0, in1, c0, c1, c2):
    u = (np.asarray(in1, np.float32) + np.float32(1.0)).astype(np.float32)
    y0 = (~u.view(np.int32)).view(np.float32)
    t = (u * y0).astype(np.float32)
    sig = (y0 * (np.float32(c0) + np.float32(c1) * t)).astype(np.float32)
    return np.asarray(in0, np.float32) * sig


def _register_op(name, spec):
    row = 1 + len(_dve_ops.OPS)
    assert row < 0x20
    shas = {}
    for ver in ("v3", "v4"):
        s = _DveOpSpec(name=name, opcode=row, uops=_dve_lower(spec, ver=ver),
                       rd1_en=_dve_has_src1(spec))
        shas[ver] = s.sha(ver)
    op = _dve_ops.DveOp(name, spec, subdim=False, uops_sha=shas)
    _dve_ops.OPS.append(op)
    _dve_ops._SUB_OPCODE_FOR_NAME[name] = row
    _dve_ops.CUSTOM_DVE_SPECS[name] = spec
    return op


def _relusig_ref(in0, in1, c0, c1, c2):
    r = np.maximum(np.nan_to_num(np.asarray(in0, np.float32), nan=0.0), 0.0)
    return _mulsig_ref(r, in1, c0, c1, c2)


def _make_mulsig():
    u = _Bin(_AluOp.ADD, _Src1, _One)
    y0 = _Bin(_AluOp.BITWISE_NOT, u, u)
    t = u * y0
    sig = y0 * (_C0 + _C1 * t)
    return _register_op("MULSIG_ANT",
                        _Spec(body=_Src0 * sig, reference=_mulsig_ref))


def _make_relusig():
    u = _Bin(_AluOp.ADD, _Src1, _One)
    y0 = _Bin(_AluOp.BITWISE_NOT, u, u)
    t = u * y0
    sig = y0 * (_C0 + _C1 * t)
    return _register_op("RELUSIG_ANT",
                        _Spec(body=_relu(_Src0) * sig, reference=_relusig_ref))


_MULSIG = _make_mulsig()
_RELUSIG = _make_relusig()

F32 = mybir.dt.float32
BF16 = mybir.dt.bfloat16
I16 = mybir.dt.int16
AF = mybir.ActivationFunctionType
OP = mybir.AluOpType

B, S, L, K, D = 128, 64, 32, 32, 256
NC = 8
BL = B // NC              # 16 batch rows per core
BK = BL * K               # 512 = free dim of the state
NG = 8                    # gather groups per core (128 sentences each)
TOKG = 128 * L            # 4096 tokens per group
TABLE_ROWS = 32768        # compacted per-core vocab (unique ids <= 32768)
EPS = 1e-12

_CACHED = {}


def _build_program():
    nc = bacc.Bacc("TRN2", target_bir_lowering=False, debug=False, num_devices=NC)

    table = nc.dram_tensor("table", [TABLE_ROWS, D], BF16, kind="ExternalInput").ap()
    idx16 = nc.dram_tensor("idx16", [128, NG * TOKG // 16], I16, kind="ExternalInput").ap()
    keysT = nc.dram_tensor("keysT", [D, BK], BF16, kind="ExternalInput").ap()
    Umat = nc.dram_tensor("Umat", [D, D], BF16, kind="ExternalInput").ap()
    Vmat = nc.dram_tensor("Vmat", [D, D], BF16, kind="ExternalInput").ap()
    Wmat = nc.dram_tensor("Wmat", [D, D], BF16, kind="ExternalInput").ap()
    mbig = nc.dram_tensor("mbig", [128, NG, BK], BF16, kind="ExternalInput").ap()
    kvtf = nc.dram_tensor("kvtf", [128, 2, BL, K], BF16, kind="ExternalInput").ap()
    hout = nc.dram_tensor("hout", [BK, D], F32, kind="ExternalOutput").ap()

    with tile.TileContext(nc) as tc:
        _emit(nc, tc, table, idx16, keysT, Umat, Vmat, Wmat, mbig, kvtf, hout)
    nc.compile()
    return nc


def _emit(nc, tc, table, idx16, keysT, Umat, Vmat, Wmat, mbig, kvtf, hout):
    from contextlib import ExitStack

    ctx = ExitStack()
    const = ctx.enter_context(tc.tile_pool(name="const", bufs=1))
    persist = ctx.enter_context(tc.tile_pool(name="persist", bufs=1))
    gpool = ctx.enter_context(tc.tile_pool(name="g", bufs=2))
    work = ctx.enter_context(tc.tile_pool(name="work", bufs=4))
    hpool = ctx.enter_context(tc.tile_pool(name="h", bufs=3))
    # PSUM budget (8 banks): psH = pshG double-buffered x2 groups (4 banks,
    # with the step's sumsq row overlaid into bank rows after relusig reads
    # it); psM = psg+gate-bcast [128,256] x2 groups (1 bank); psB = inv
    # bcast [128,256] x2 groups (1 bank); psG = gather scratch (2 banks) so
    # the gather pipeline overlaps the scan instead of serializing on scan
    # PSUM tags.
    psH = ctx.enter_context(tc.tile_pool(name="psH", bufs=1, space="PSUM"))
    psM = ctx.enter_context(tc.tile_pool(name="psM", bufs=1, space="PSUM"))
    psB = ctx.enter_context(tc.tile_pool(name="psB", bufs=1, space="PSUM"))
    psG = ctx.enter_context(tc.tile_pool(name="psG", bufs=1, space="PSUM"))

    # ---- constants into SBUF ----
    sb_idx = const.tile([128, NG * TOKG // 16], I16)
    nc.sync.dma_start(out=sb_idx[:, 0:TOKG // 16], in_=idx16[:, 0:TOKG // 16])
    nc.sync.dma_start(out=sb_idx[:, TOKG // 16:], in_=idx16[:, TOKG // 16:])
    kT = [const.tile([128, BK], BF16, tag=f"kT{j}", name=f"kT{j}") for j in range(2)]
    for j in range(2):
        nc.sync.dma_start(out=kT[j][:], in_=keysT[128 * j:128 * (j + 1), :])
    sbU = [const.tile([128, D], BF16, tag=f"sbU{j}", name=f"sbU{j}") for j in range(2)]
    sbV = [const.tile([128, D], BF16, tag=f"sbV{j}", name=f"sbV{j}") for j in range(2)]
    sbW = [const.tile([128, D], BF16, tag=f"sbW{j}", name=f"sbW{j}") for j in range(2)]
    for j in range(2):
        nc.sync.dma_start(out=sbU[j][:], in_=Umat[128 * j:128 * (j + 1), :])
        nc.sync.dma_start(out=sbV[j][:], in_=Vmat[128 * j:128 * (j + 1), :])
        nc.sync.dma_start(out=sbW[j][:], in_=Wmat[128 * j:128 * (j + 1), :])
    sb_mb = const.tile([128, NG, BK], BF16)
    nc.sync.dma_start(out=sb_mb[:], in_=mbig[:])

    I128 = const.tile([128, 128], BF16)
    make_identity(nc, I128[:])
    ones8 = const.tile([8, 128], BF16)
    nc.vector.memset(ones8[:], 1.0)
    ones128 = const.tile([128, 1], BF16)
    nc.vector.memset(ones128[:], 1.0)
    ones1 = const.tile([1, 128], BF16)
    nc.vector.memset(ones1[:], 1.0)
    epsap = const.tile([1, 1], F32)
    nc.vector.memset(epsap[:], EPS)
    # word-sum reducers: Ablk[i][p, m] = 1 iff m == 4*i + p//32.
    Ablk = []
    for i in range(16):
        a = const.tile([128, 64], BF16, tag=f"Ablk{i}", name=f"Ablk{i}")
        nc.vector.memset(a[:], 0.0)
        for q in range(4):
            nc.vector.memset(a[32 * q:32 * (q + 1), 4 * i + q:4 * i + q + 1], 1.0)
        Ablk.append(a)

    # ---- persistent intermediates ----
    ET = [persist.tile([128, NG * 128], BF16, tag=f"ET{j}", name=f"ET{j}") for j in range(2)]
    eWc = persist.tile([128, 2, NG * 128], BF16, tag="eWc", name="eWc")
    # kvt = keys V, host-computed; shaped [128, 2(de half), BL, K]
    kvt = persist.tile([128, 2, BL, K], BF16, tag="kvt", name="kvt")
    gk0m = persist.tile([128, NG, BK], BF16, tag="gk0m", name="gk0m")

    def gather_dma(g, split=1):
        G = gpool.tile([128, L, D], BF16, tag="G")
        HT = TOKG // split
        for hseg in range(split):
            nc.gpsimd.dma_gather(
                out_ap=G[:, (L // split) * hseg:(L // split) * (hseg + 1), :],
                in_ap=table[:],
                idxs_ap=sb_idx[:, (TOKG // 16) * g + (HT // 16) * hseg:
                               (TOKG // 16) * g + (HT // 16) * (hseg + 1)],
                num_idxs=HT, num_idxs_reg=HT, elem_size=D, single_packet=False,
            )
        psE = psG.tile([128, D], F32, tag="pgs", name="psE")
        return G, psE

    def gather_wordsum(G, psE, cs):
        for c in cs:
            j, i = c // 16, c % 16
            nc.tensor.matmul(psE[64 * j:64 * (j + 1), :], lhsT=Ablk[i][:],
                             rhs=G[:, c, :], start=(i == 0), stop=(i == 15))

    def gather_finish(g, psE):
        enc = work.tile([128, D], BF16, tag="enc")
        nc.scalar.copy(out=enc[:], in_=psE[:])
        for j in range(2):
            pt = psG.tile([128, 128], BF16, tag="pgs", name="pt")
            nc.tensor.transpose(pt[:], enc[:, 128 * j:128 * (j + 1)], I128[:])
            nc.vector.tensor_copy(out=ET[j][:, 128 * g:128 * (g + 1)], in_=pt[:])
        for m in range(2):
            pw = psG.tile([128, 128], F32, tag="pgs", name="pw")
            nc.tensor.matmul(pw[:], lhsT=sbW[0][:, 128 * m:128 * (m + 1)],
                             rhs=ET[0][:, 128 * g:128 * (g + 1)], start=True, stop=False)
            nc.tensor.matmul(pw[:], lhsT=sbW[1][:, 128 * m:128 * (m + 1)],
                             rhs=ET[1][:, 128 * g:128 * (g + 1)], start=False, stop=True)
            nc.vector.tensor_copy(out=eWc[:, m, 128 * g:128 * (g + 1)], in_=pw[:])
        pgk = psG.tile([128, BK], F32, tag="pgk", name="pgk")
        nc.tensor.matmul(pgk[:], lhsT=ET[0][:, 128 * g:128 * (g + 1)], rhs=kT[0][:],
                         start=True, stop=False)
        nc.tensor.matmul(pgk[:], lhsT=ET[1][:, 128 * g:128 * (g + 1)], rhs=kT[1][:],
                         start=False, stop=True)
        nc.vector.tensor_tensor(out=gk0m[:, g, :], in0=pgk[:],
                                in1=sb_mb[:, g, :], op=OP.add)

    # ---- scan: two pipelined batch groups (b 0-7 | b 8-15) ----
    HB = BK // 2  # 256
    h = [hpool.tile([128, 2, HB], BF16, tag=f"h{gb}", name=f"h{gb}")
         for gb in range(2)]
    for gb in range(2):
        nc.vector.memset(h[gb][:], 0.0)

    def phase_a(t, gb):
        """Injections, h-dependent matmuls, gate, relu*sigmoid, upd, sq."""
        g, ds = t // 8, t % 8
        cg = 128 * g + 16 * ds + 8 * gb
        bks = slice(HB * gb, HB * (gb + 1))
        hg = h[gb]

        # h-independent PSUM preloads
        pshG = psH.tile([128, 2, HB], F32, tag=f"psh{gb}", name=f"psh{gb}")
        nc.tensor.matmul(pshG[:, :, :], lhsT=I128[:],
                         rhs=kvt[:, :, 8 * gb:8 * gb + 8, :], start=True, stop=False)
        for m in range(2):
            ew_bc = eWc[:, m, cg:cg + 8].unsqueeze(2).broadcast_to([128, 8, 32])
            nc.tensor.matmul(pshG[:, m, :], lhsT=I128[:], rhs=ew_bc,
                             start=False, stop=False)
        psMt = psM.tile([128, BK], F32, tag=f"psm{gb}", name=f"psm{gb}")
        psg = psMt[0:8, 0:HB]
        off = 16 * ds + 8 * gb
        nc.tensor.matmul(psg, lhsT=I128[:, off:off + 8],
                         rhs=gk0m[:, g, bks], start=True, stop=False)

        # h-dependent matmuls
        nc.tensor.matmul(psg, lhsT=ET[0][:, cg:cg + 8], rhs=hg[:, 0, :],
                         start=False, stop=False)
        nc.tensor.matmul(psg, lhsT=ET[1][:, cg:cg + 8], rhs=hg[:, 1, :],
                         start=False, stop=True)
        for m in range(2):
            nc.tensor.matmul(pshG[:, m, :], lhsT=sbU[0][:, 128 * m:128 * (m + 1)],
                             rhs=hg[:, 0, :], start=False, stop=False)
            nc.tensor.matmul(pshG[:, m, :], lhsT=sbU[1][:, 128 * m:128 * (m + 1)],
                             rhs=hg[:, 1, :], start=False, stop=(m == 1))

        # gate: eg = exp(-logit); sigmoid folded into RELUSIG below. The
        # gate broadcast overwrites the psg columns (WAR after eg reads).
        eg = work.tile([8, HB], BF16, tag=f"eg{gb}", name=f"eg{gb}")
        nc.scalar.activation(eg[:], psg, AF.Exp, scale=-1.0)
        nc.tensor.matmul(psMt[:, 0:HB], lhsT=ones8[:], rhs=eg[:],
                         start=True, stop=True)

        # u = relu(psh) * sigmoid; upd = u + h; sq = upd^2
        r = work.tile([128, 2, HB], BF16, tag=f"r{gb}", name=f"r{gb}")
        nc.scalar.activation(r[:], pshG[:, :, :], AF.Relu)
        u = work.tile([128, 2, HB], BF16, tag=f"u{gb}", name=f"u{gb}")
        nc.vector._custom_dve(
            _MULSIG, out=u[:], in0=r[:],
            in1=psMt[:, 0:HB].unsqueeze(1).broadcast_to([128, 2, HB]),
            s0=float(_RS_C0), s1=float(_RS_C1))
        upd = work.tile([128, 2, HB], BF16, tag=f"upd{gb}", name=f"upd{gb}")
        nc.vector.tensor_tensor(out=upd[:], in0=u[:], in1=hg[:], op=OP.add)
        sq = work.tile([128, 2, HB], BF16, tag=f"sq{gb}", name=f"sq{gb}")
        nc.vector.tensor_tensor(out=sq[:], in0=upd[:], in1=upd[:], op=OP.mult)
        return psMt, upd, sq

    def phase_b(t, gb, psMt, upd, sq):
        """Norm tail: sumsq, rsqrt via ln/exp, inv broadcast, hn."""
        pss = psMt[0:1, HB:HB + HB]
        nc.tensor.matmul(pss, lhsT=ones128[:], rhs=sq[:, 0, :],
                         start=True, stop=False)
        nc.tensor.matmul(pss, lhsT=ones128[:], rhs=sq[:, 1, :],
                         start=False, stop=True)
        lns = psMt[32:33, HB:HB + HB]
        nc.scalar.activation(lns, pss, AF.Ln, bias=epsap[:])
        inv = work.tile([1, HB], BF16, tag=f"inv{gb}", name=f"inv{gb}")
        nc.scalar.activation(inv[:], lns, AF.Exp, scale=-0.5)
        psBI = psB.tile([128, HB], F32, tag=f"psb{gb}", name=f"psb{gb}")
        nc.tensor.matmul(psBI[:, :], lhsT=ones1[:], rhs=inv[:],
                         start=True, stop=True)
        hni = hpool.tile([128, 2, HB], BF16, tag=f"h{gb}", name=f"hn{gb}")
        nc.vector.tensor_tensor(
            out=hni[:], in0=upd[:],
            in1=psBI[:, :].unsqueeze(1).broadcast_to([128, 2, HB]),
            op=OP.mult)
        return hni

    # PE p-state warm-up: keep the PE continuously busy while the first
    # gather's DMA is in flight so the word-sum runs at full clock.
    warm = psB.tile([128, HB], F32, tag="psb1", name="warm")
    for w in range(24):
        nc.tensor.matmul(warm[:, :], lhsT=I128[:], rhs=kT[0][:, 0:HB],
                         start=(w == 0), stop=(w == 23))

    def gather_group(g, split=1):
        G, psE = gather_dma(g, split=split)
        gather_wordsum(G, psE, range(L))
        gather_finish(g, psE)

    gather_group(0, split=2)
    nc.sync.dma_start(out=kvt[:], in_=kvtf[:])
    gather_group(1)
    # Groups 2..7 stream under the scan: DMA issued at the block start, the
    # word-sum spread 4 slots per step, finalize at the block end.
    pend = [None]
    for t in range(S):
        ti = t % 8
        if ti == 0 and t // 8 + 2 < NG:
            pend[0] = (t // 8 + 2,) + gather_dma(t // 8 + 2)
        if pend[0] is not None:
            gp, G, psE = pend[0]
            gather_wordsum(G, psE, range(4 * ti, 4 * ti + 4))
            if ti == 7:
                gather_finish(gp, psE)
                pend[0] = None
        st = [phase_a(t, 0), phase_a(t, 1)]
        h = [phase_b(t, gb, *st[gb]) for gb in range(2)]

    # ---- output: transpose h^T [256, 512] -> [512, 256] fp32 ----
    for q in range(4):
        gb, half = q // 2, q % 2
        ho = work.tile([128, D], F32, tag="ho")
        for j in range(2):
            pt = psG.tile([128, 128], BF16, tag="pgs", name="ptout")
            nc.tensor.transpose(pt[:], h[gb][:, j, 128 * half:128 * half + 128],
                                I128[:])
            nc.vector.tensor_copy(out=ho[:, 128 * j:128 * (j + 1)], in_=pt[:])
        nc.sync.dma_start(out=hout[128 * q:128 * (q + 1), :], in_=ho[:])

    ctx.close()


def _prep_core(pr, mask, keys_c, emb):
    """Host-side marshaling for one core's shard."""
    uniq, inv = np.unique(pr, return_inverse=True)
    assert len(uniq) <= TABLE_ROWS
    table = np.zeros((TABLE_ROWS, D), dtype=ml_dtypes.bfloat16)
    table[: len(uniq)] = emb[uniq].astype(ml_dtypes.bfloat16)
    ranks = inv.reshape(BL, S, L).astype(np.int16)

    # token order per group g: i = (ds*16 + b)*32 + w
    idx_groups = []
    for g in range(NG):
        blk = ranks[:, 8 * g:8 * (g + 1), :]          # [b, ds, w]
        lst = blk.transpose(1, 0, 2).reshape(-1)      # [(ds, b, w)] length 4096
        idx_groups.append(np.tile(lst.reshape(TOKG // 16, 16).T, (8, 1)))
    idx16 = np.concatenate(idx_groups, axis=1).astype(np.int16)  # [128, NG*256]

    keysT = np.ascontiguousarray(
        keys_c.reshape(BK, D).T).astype(ml_dtypes.bfloat16)      # [256, 512]


    # mbig[r=(ds,gb,b), g, c=(bglob,k)] gate-logit offsets:
    #   0    where the gate is live (diagonal batch, unmasked step),
    #   -35  diagonal but step-masked  (exp(-logit) huge -> sigmoid ~ 0),
    #   +35  off-diagonal              (exp(-logit) ~ 0, vanishes in the sum).
    m = mask.astype(bool)                                        # [16, 64]
    mb = np.full((128, NG, BK), 35.0, np.float32)
    r = np.arange(128)
    ds_, gb_, b_ = r // 16, (r % 16) // 8, r % 8
    bglob = 8 * gb_ + b_
    for g in range(NG):
        t_ = 8 * g + ds_                                         # [128]
        ok_row = m[bglob, t_]                                    # [128]
        cols = (np.arange(BK) // K)[None, :] == bglob[:, None]   # [128, BK]
        gm_ = mb[:, g, :]
        gm_[cols] = np.where(np.repeat(ok_row, K), 0.0, -35.0)
        mb[:, g, :] = gm_
    return table, idx16, keysT, mb.astype(ml_dtypes.bfloat16)


def kernel(prgrph, prgrph_mask, keys, embedding_matrix, U, V, W):
    prgrph = np.asarray(prgrph)
    prgrph_mask = np.asarray(prgrph_mask)
    keys = np.asarray(keys, dtype=np.float32)
    emb = np.asarray(embedding_matrix, dtype=np.float32)
    U = np.asarray(U, dtype=np.float32)
    V = np.asarray(V, dtype=np.float32)
    W = np.asarray(W, dtype=np.float32)

    if "nc" not in _CACHED:
        _CACHED["nc"] = _build_program()
    nc = _CACHED["nc"]

    Ub, Vb, Wb = (x.astype(ml_dtypes.bfloat16) for x in (U, V, W))

    in_maps = []
    for c in range(NC):
        sl = slice(BL * c, BL * (c + 1))
        table, idx16, keysT, mb = _prep_core(
            prgrph[sl], prgrph_mask[sl, :, 0], keys[sl], emb)
        kv = keys[sl].reshape(BK, D) @ V                       # [bk, de]
        kvtf = np.ascontiguousarray(
            kv.T.reshape(2, 128, BL, K).transpose(1, 0, 2, 3)).astype(ml_dtypes.bfloat16)
        in_maps.append({
            "table": table, "idx16": idx16, "keysT": keysT,
            "Umat": Ub, "Vmat": Vb, "Wmat": Wb,
            "mbig": mb, "kvtf": kvtf,
        })

    res = run_bass_kernel_spmd(nc, in_maps, core_ids=list(range(NC)))
    out = np.concatenate(
        [res.results[c]["hout"].reshape(BL, K, D) for c in range(NC)], axis=0)
    return out.astype(np.float32)


# revision 45
# speedup vs baseline: 1.0557x; 1.0557x over previous
"""Trainium2 Bass kernel for nn_BasicRecurrentEntityEncoder.

Full-input contract: kernel(**inputs) takes the complete (unsharded) numpy
inputs and returns the full [B, K, D] float32 output. Internally the batch
is sharded over 8 NeuronCores (data parallel, no collectives), the embedding
bag-of-words gather runs through dma_gather against a per-core compacted
bf16 table, and the 64-step entity recurrence runs in a transposed
[D, (b,k)] layout with bf16 matmul operands.

Structure (625.8us baseline -> 457.2us):
  - Gather pipeline fully overlapped with the scan: groups 0-1 up front
    (group 0's dma split in two so the word-sum starts early, behind a PE
    p-state warm-up), groups 2-7 streamed under scan block g-2 with the
    word-sum spread 4 slot-matmuls per step. Dedicated gather PSUM banks
    (psG) keep it off the scan's PSUM tags.
  - e.keys gate logits precomputed per group ([128,512] matmul) with the
    sentence mask and block-diagonal batch mask folded in host-side as
    +-35 logit offsets (gk0m); per step they are matmul-injected into the
    gate bank, so the gate path is inject + 2 E_t^T h matmuls + one
    ScalarE exp. The exp(-logit) rows are broadcast-SUMMED by a ones8
    matmul (masked entries vanish), and the sigmoid reciprocal is fused
    into the custom DVE op MULSIG: u = relu(psh) * 1/(1+sum exp), with
    relu on ScalarE (HW allows only one PSUM operand per DVE op).
  - kvt = keys V is computed on the host; kvt + eW broadcasts are PSUM
    preloads (h-independent), U^T h accumulates on top.
  - upd/sq are 2x-mode bf16 tensor_tensor ops; norm rsqrt = exp(-.5 ln)
    on ScalarE with ln written back to PSUM (cheaper access), inv
    broadcast by a ones1 matmul, h_new = upd * inv on DVE.
  - Two batch groups (b 0-7 | b 8-15) with separate PSUM banks pipeline
    the serial dependency chain across engines.
"""

import sys

if "/opt/trn_rl_repo" not in sys.path:
    sys.path.insert(0, "/opt/trn_rl_repo")

import numpy as np
import ml_dtypes

from concourse import bacc, mybir
import concourse.bass as bass
import concourse.tile as tile
from concourse.bass_utils import run_bass_kernel_spmd
from concourse.masks import make_identity

# Force every ScalarE activation onto the one table set that covers all the
# functions this kernel uses (relu/exp/ln/copy/identity). The default
# chooser greedily picks the first set per function, inserting ~550ns table
# reloads on the critical path. Padding the dict keeps act_func_set_id
# indices aligned with act_info.json while making only the all-covering set
# usable.
_ONE_SET = "natural_log_exp_and_others"


import concourse.hw_specs as _hw_specs
_ORIG_TABLES = _hw_specs.get_activation_tables


def _patched_tables(module_arch):
    real = _ORIG_TABLES(module_arch)
    names = list(real.keys())
    assert _ONE_SET in names, names
    out = {}
    for n in names:
        if n == _ONE_SET:
            out[n] = real[n]
            break
        out[n] = set()
    return out


def _install_table_patch():
    import functools
    cached = functools.cache(_patched_tables)
    bacc.get_activation_tables = cached
    _hw_specs.get_activation_tables = cached


_install_table_patch()

# Custom DVE op #1: out ~= 1/(1 + in0) in ONE VectorE instruction (8 ALU
# stages): u = in0+1; seed y0 = bitcast(~bits(u)); t = u*y0 lands in
# [-4.5, -4] for any positive u; quadratic minimax fixup P(t) ~= 1/t gives
# out = y0*P(t) at ~1e-5 relative error.
import concourse.dve_ops as _dve_ops
from concourse.dve_spec import AluOp as _AluOp, Bin as _Bin, Spec as _Spec
from concourse.dve_spec import C0 as _C0, C1 as _C1, C2 as _C2, One as _One
from concourse.dve_spec import Src0 as _Src0, Src1 as _Src1, relu as _relu
from concourse.dve_spec import lower as _dve_lower
from concourse.dve_spec import _has_src1 as _dve_has_src1
from concourse.dve_uop import DveOpSpec as _DveOpSpec

# MULSIG fuses u = r * sigmoid(logit): in0 = relu(psh) (SBUF bf16, relu on
# ScalarE -- HW allows only ONE PSUM input per DVE op so the gate broadcast
# keeps the PSUM slot), in1 = sum_sent exp(-logit) from the ones8 matmul.
# out = in0 * 1/(1+in1) via the bitwise-NOT seed and a LINEAR fixup
# P(t) = c0 + c1*t on t in [-4.5, -4] (max sigmoid abs err ~1.4e-3).
_RS_C0, _RS_C1 = -0.47250233, -0.05572371


def _mulsig_ref(in0, in1, c0, c1, c2):
    u = (np.asarray(in1, np.float32) + np.float32(1.0)).astype(np.float32)
    y0 = (~u.view(np.int32)).view(np.float32)
    t = (u * y0).astype(np.float32)
    sig = (y0 * (np.float32(c0) + np.float32(c1) * t)).astype(np.float32)
    return np.asarray(in0, np.float32) * sig


def _register_op(name, spec):
    row = 1 + len(_dve_ops.OPS)
    assert row < 0x20
    shas = {}
    for ver in ("v3", "v4"):
        s = _DveOpSpec(name=name, opcode=row, uops=_dve_lower(spec, ver=ver),
                       rd1_en=_dve_has_src1(spec))
        shas[ver] = s.sha(ver)
    op = _dve_ops.DveOp(name, spec, subdim=False, uops_sha=shas)
    _dve_ops.OPS.append(op)
    _dve_ops._SUB_OPCODE_FOR_NAME[name] = row
    _dve_ops.CUSTOM_DVE_SPECS[name] = spec
    return op


def _relusig_ref(in0, in1, c0, c1, c2):
    r = np.maximum(np.nan_to_num(np.asarray(in0, np.float32), nan=0.0), 0.0)
    return _mulsig_ref(r, in1, c0, c1, c2)


def _make_mulsig():
    u = _Bin(_AluOp.ADD, _Src1, _One)
    y0 = _Bin(_AluOp.BITWISE_NOT, u, u)
    t = u * y0
    sig = y0 * (_C0 + _C1 * t)
    return _register_op("MULSIG_ANT",
                        _Spec(body=_Src0 * sig, reference=_mulsig_ref))


def _make_relusig():
    u = _Bin(_AluOp.ADD, _Src1, _One)
    y0 = _Bin(_AluOp.BITWISE_NOT, u, u)
    t = u * y0
    sig = y0 * (_C0 + _C1 * t)
    return _register_op("RELUSIG_ANT",
                        _Spec(body=_relu(_Src0) * sig, reference=_relusig_ref))


def _addsq_ref(in0, in1, c0, c1, c2):
    a = (np.asarray(in0, np.float32) + np.asarray(in1, np.float32)).astype(np.float32)
    return a * a


def _make_addsq():
    a = _Bin(_AluOp.ADD, _Src0, _Src1)
    return _register_op("ADDSQ_ANT",
                        _Spec(body=a * a, reference=_addsq_ref))


_MULSIG = _make_mulsig()
_RELUSIG = _make_relusig()
_ADDSQ = _make_addsq()

F32 = mybir.dt.float32
BF16 = mybir.dt.bfloat16
I16 = mybir.dt.int16
AF = mybir.ActivationFunctionType
OP = mybir.AluOpType

B, S, L, K, D = 128, 64, 32, 32, 256
NC = 8
BL = B // NC              # 16 batch rows per core
BK = BL * K               # 512 = free dim of the state
NG = 8                    # gather groups per core (128 sentences each)
TOKG = 128 * L            # 4096 tokens per group
TABLE_ROWS = 32768        # compacted per-core vocab (unique ids <= 32768)
EPS = 1e-12

_CACHED = {}


def _build_program():
    nc = bacc.Bacc("TRN2", target_bir_lowering=False, debug=False, num_devices=NC)

    table = nc.dram_tensor("table", [TABLE_ROWS, D], BF16, kind="ExternalInput").ap()
    idx16 = nc.dram_tensor("idx16", [128, NG * TOKG // 16], I16, kind="ExternalInput").ap()
    keysT = nc.dram_tensor("keysT", [D, BK], BF16, kind="ExternalInput").ap()
    Umat = nc.dram_tensor("Umat", [D, D], BF16, kind="ExternalInput").ap()
    Vmat = nc.dram_tensor("Vmat", [D, D], BF16, kind="ExternalInput").ap()
    Wmat = nc.dram_tensor("Wmat", [D, D], BF16, kind="ExternalInput").ap()
    mbig = nc.dram_tensor("mbig", [128, NG, BK], BF16, kind="ExternalInput").ap()
    kvtf = nc.dram_tensor("kvtf", [128, 2, BL, K], BF16, kind="ExternalInput").ap()
    hout = nc.dram_tensor("hout", [BK, D], F32, kind="ExternalOutput").ap()

    with tile.TileContext(nc) as tc:
        _emit(nc, tc, table, idx16, keysT, Umat, Vmat, Wmat, mbig, kvtf, hout)
    nc.compile()
    return nc


def _emit(nc, tc, table, idx16, keysT, Umat, Vmat, Wmat, mbig, kvtf, hout):
    from contextlib import ExitStack

    ctx = ExitStack()
    const = ctx.enter_context(tc.tile_pool(name="const", bufs=1))
    persist = ctx.enter_context(tc.tile_pool(name="persist", bufs=1))
    gpool = ctx.enter_context(tc.tile_pool(name="g", bufs=2))
    work = ctx.enter_context(tc.tile_pool(name="work", bufs=4))
    hpool = ctx.enter_context(tc.tile_pool(name="h", bufs=3))
    # PSUM budget (8 banks): psH = pshG double-buffered x2 groups (4 banks,
    # with the step's sumsq row overlaid into bank rows after relusig reads
    # it); psM = psg+gate-bcast [128,256] x2 groups (1 bank); psB = inv
    # bcast [128,256] x2 groups (1 bank); psG = gather scratch (2 banks) so
    # the gather pipeline overlaps the scan instead of serializing on scan
    # PSUM tags.
    psH = ctx.enter_context(tc.tile_pool(name="psH", bufs=1, space="PSUM"))
    psM = ctx.enter_context(tc.tile_pool(name="psM", bufs=1, space="PSUM"))
    psB = ctx.enter_context(tc.tile_pool(name="psB", bufs=1, space="PSUM"))
    psG = ctx.enter_context(tc.tile_pool(name="psG", bufs=1, space="PSUM"))

    # ---- constants into SBUF ----
    sb_idx = const.tile([128, NG * TOKG // 16], I16)
    nc.sync.dma_start(out=sb_idx[:, 0:TOKG // 16], in_=idx16[:, 0:TOKG // 16])
    nc.sync.dma_start(out=sb_idx[:, TOKG // 16:], in_=idx16[:, TOKG // 16:])
    kT = [const.tile([128, BK], BF16, tag=f"kT{j}", name=f"kT{j}") for j in range(2)]
    for j in range(2):
        nc.sync.dma_start(out=kT[j][:], in_=keysT[128 * j:128 * (j + 1), :])
    sbU = [const.tile([128, D], BF16, tag=f"sbU{j}", name=f"sbU{j}") for j in range(2)]
    sbV = [const.tile([128, D], BF16, tag=f"sbV{j}", name=f"sbV{j}") for j in range(2)]
    sbW = [const.tile([128, D], BF16, tag=f"sbW{j}", name=f"sbW{j}") for j in range(2)]
    for j in range(2):
        nc.sync.dma_start(out=sbU[j][:], in_=Umat[128 * j:128 * (j + 1), :])
        nc.sync.dma_start(out=sbV[j][:], in_=Vmat[128 * j:128 * (j + 1), :])
        nc.sync.dma_start(out=sbW[j][:], in_=Wmat[128 * j:128 * (j + 1), :])
    sb_mb = const.tile([128, NG, BK], BF16)
    nc.sync.dma_start(out=sb_mb[:], in_=mbig[:])

    I128 = const.tile([128, 128], BF16)
    make_identity(nc, I128[:])
    ones8 = const.tile([8, 128], BF16)
    nc.vector.memset(ones8[:], 1.0)
    ones128 = const.tile([128, 1], BF16)
    nc.vector.memset(ones128[:], 1.0)
    ones1 = const.tile([1, 128], BF16)
    nc.vector.memset(ones1[:], 1.0)
    epsap = const.tile([1, 1], F32)
    nc.vector.memset(epsap[:], EPS)
    # word-sum reducers: Ablk[i][p, m] = 1 iff m == 4*i + p//32.
    Ablk = []
    for i in range(16):
        a = const.tile([128, 64], BF16, tag=f"Ablk{i}", name=f"Ablk{i}")
        nc.vector.memset(a[:], 0.0)
        for q in range(4):
            nc.vector.memset(a[32 * q:32 * (q + 1), 4 * i + q:4 * i + q + 1], 1.0)
        Ablk.append(a)

    # ---- persistent intermediates ----
    ET = [persist.tile([128, NG * 128], BF16, tag=f"ET{j}", name=f"ET{j}") for j in range(2)]
    eWc = persist.tile([128, 2, NG * 128], BF16, tag="eWc", name="eWc")
    # kvt = keys V, host-computed; shaped [128, 2(de half), BL, K]
    kvt = persist.tile([128, 2, BL, K], BF16, tag="kvt", name="kvt")
    gk0m = persist.tile([128, NG, BK], BF16, tag="gk0m", name="gk0m")

    def gather_dma(g, split=1):
        G = gpool.tile([128, L, D], BF16, tag="G")
        HT = TOKG // split
        for hseg in range(split):
            nc.gpsimd.dma_gather(
                out_ap=G[:, (L // split) * hseg:(L // split) * (hseg + 1), :],
                in_ap=table[:],
                idxs_ap=sb_idx[:, (TOKG // 16) * g + (HT // 16) * hseg:
                               (TOKG // 16) * g + (HT // 16) * (hseg + 1)],
                num_idxs=HT, num_idxs_reg=HT, elem_size=D, single_packet=False,
            )
        psE = psG.tile([128, D], F32, tag="pgs", name="psE")
        return G, psE

    def gather_wordsum(G, psE, cs):
        for c in cs:
            j, i = c // 16, c % 16
            nc.tensor.matmul(psE[64 * j:64 * (j + 1), :], lhsT=Ablk[i][:],
                             rhs=G[:, c, :], start=(i == 0), stop=(i == 15))

    def gather_finish(g, psE):
        enc = work.tile([128, D], BF16, tag="enc")
        nc.scalar.copy(out=enc[:], in_=psE[:])
        for j in range(2):
            pt = psG.tile([128, 128], BF16, tag="pgs", name="pt")
            nc.tensor.transpose(pt[:], enc[:, 128 * j:128 * (j + 1)], I128[:])
            nc.vector.tensor_copy(out=ET[j][:, 128 * g:128 * (g + 1)], in_=pt[:])
        for m in range(2):
            pw = psG.tile([128, 128], F32, tag="pgs", name="pw")
            nc.tensor.matmul(pw[:], lhsT=sbW[0][:, 128 * m:128 * (m + 1)],
                             rhs=ET[0][:, 128 * g:128 * (g + 1)], start=True, stop=False)
            nc.tensor.matmul(pw[:], lhsT=sbW[1][:, 128 * m:128 * (m + 1)],
                             rhs=ET[1][:, 128 * g:128 * (g + 1)], start=False, stop=True)
            nc.vector.tensor_copy(out=eWc[:, m, 128 * g:128 * (g + 1)], in_=pw[:])
        pgk = psG.tile([128, BK], F32, tag="pgk", name="pgk")
        nc.tensor.matmul(pgk[:], lhsT=ET[0][:, 128 * g:128 * (g + 1)], rhs=kT[0][:],
                         start=True, stop=False)
        nc.tensor.matmul(pgk[:], lhsT=ET[1][:, 128 * g:128 * (g + 1)], rhs=kT[1][:],
                         start=False, stop=True)
        nc.vector.tensor_tensor(out=gk0m[:, g, :], in0=pgk[:],
                                in1=sb_mb[:, g, :], op=OP.add)

    # ---- scan: two pipelined batch groups (b 0-7 | b 8-15) ----
    HB = BK // 2  # 256
    h = [hpool.tile([128, 2, HB], BF16, tag=f"h{gb}", name=f"h{gb}")
         for gb in range(2)]
    for gb in range(2):
        nc.vector.memset(h[gb][:], 0.0)

    def phase_a(t, gb):
        """Injections, h-dependent matmuls, gate, relu*sigmoid, upd, sq."""
        g, ds = t // 8, t % 8
        cg = 128 * g + 16 * ds + 8 * gb
        bks = slice(HB * gb, HB * (gb + 1))
        hg = h[gb]

        # h-independent PSUM preloads
        pshG = psH.tile([128, 2, HB], F32, tag=f"psh{gb}", name=f"psh{gb}")
        nc.tensor.matmul(pshG[:, :, :], lhsT=I128[:],
                         rhs=kvt[:, :, 8 * gb:8 * gb + 8, :], start=True, stop=False)
        for m in range(2):
            ew_bc = eWc[:, m, cg:cg + 8].unsqueeze(2).broadcast_to([128, 8, 32])
            nc.tensor.matmul(pshG[:, m, :], lhsT=I128[:], rhs=ew_bc,
                             start=False, stop=False)
        psMt = psM.tile([128, BK], F32, tag=f"psm{gb}", name=f"psm{gb}")
        psg = psMt[0:8, 0:HB]
        off = 16 * ds + 8 * gb
        nc.tensor.matmul(psg, lhsT=I128[:, off:off + 8],
                         rhs=gk0m[:, g, bks], start=True, stop=False)

        # h-dependent matmuls
        nc.tensor.matmul(psg, lhsT=ET[0][:, cg:cg + 8], rhs=hg[:, 0, :],
                         start=False, stop=False)
        nc.tensor.matmul(psg, lhsT=ET[1][:, cg:cg + 8], rhs=hg[:, 1, :],
                         start=False, stop=True)
        for m in range(2):
            nc.tensor.matmul(pshG[:, m, :], lhsT=sbU[0][:, 128 * m:128 * (m + 1)],
                             rhs=hg[:, 0, :], start=False, stop=False)
            nc.tensor.matmul(pshG[:, m, :], lhsT=sbU[1][:, 128 * m:128 * (m + 1)],
                             rhs=hg[:, 1, :], start=False, stop=(m == 1))

        # gate: eg = exp(-logit); sigmoid folded into RELUSIG below. The
        # gate broadcast overwrites the psg columns (WAR after eg reads).
        eg = work.tile([8, HB], BF16, tag=f"eg{gb}", name=f"eg{gb}")
        nc.scalar.activation(eg[:], psg, AF.Exp, scale=-1.0)
        nc.tensor.matmul(psMt[:, 0:HB], lhsT=ones8[:], rhs=eg[:],
                         start=True, stop=True)

        # u = relu(psh) * sigmoid, split by de-half so the second relu
        # half overlaps the first MULSIG half.
        r = work.tile([128, 2, HB], BF16, tag=f"r{gb}", name=f"r{gb}")
        u = work.tile([128, 2, HB], BF16, tag=f"u{gb}", name=f"u{gb}")
        for m in range(2):
            nc.scalar.activation(r[:, m, :], pshG[:, m, :], AF.Relu)
            nc.vector._custom_dve(
                _MULSIG, out=u[:, m, :], in0=r[:, m, :],
                in1=psMt[:, 0:HB],
                s0=float(_RS_C0), s1=float(_RS_C1))
        # sq = (u+h)^2 fused (keeps the sumsq path one uninterruptible DVE
        # op); upd itself is only needed by hn much later.
        sq = work.tile([128, 2, HB], BF16, tag=f"sq{gb}", name=f"sq{gb}")
        nc.vector._custom_dve(_ADDSQ, out=sq[:], in0=u[:], in1=hg[:])
        return psMt, (u, hg), sq

    def phase_b(t, gb, psMt, uh, sq):
        """Norm tail: sumsq, rsqrt via ln/exp, inv broadcast, hn."""
        u, hg = uh
        upd = work.tile([128, 2, HB], BF16, tag=f"upd{gb}", name=f"upd{gb}")
        nc.vector.tensor_tensor(out=upd[:], in0=u[:], in1=hg[:], op=OP.add)
        pss = psMt[0:1, HB:HB + HB]
        nc.tensor.matmul(pss, lhsT=ones128[:], rhs=sq[:, 0, :],
                         start=True, stop=False)
        nc.tensor.matmul(pss, lhsT=ones128[:], rhs=sq[:, 1, :],
                         start=False, stop=True)
        lns = psMt[32:33, HB:HB + HB]
        nc.scalar.activation(lns, pss, AF.Ln, bias=epsap[:])
        inv = work.tile([1, HB], BF16, tag=f"inv{gb}", name=f"inv{gb}")
        nc.scalar.activation(inv[:], lns, AF.Exp, scale=-0.5)
        psBI = psB.tile([128, HB], F32, tag=f"psb{gb}", name=f"psb{gb}")
        nc.tensor.matmul(psBI[:, :], lhsT=ones1[:], rhs=inv[:],
                         start=True, stop=True)
        hni = hpool.tile([128, 2, HB], BF16, tag=f"h{gb}", name=f"hn{gb}")
        nc.vector.tensor_tensor(
            out=hni[:], in0=upd[:],
            in1=psBI[:, :].unsqueeze(1).broadcast_to([128, 2, HB]),
            op=OP.mult)
        return hni

    # PE p-state warm-up: keep the PE continuously busy while the first
    # gather's DMA is in flight so the word-sum runs at full clock.
    warm = psB.tile([128, HB], F32, tag="psb1", name="warm")
    for w in range(24):
        nc.tensor.matmul(warm[:, :], lhsT=I128[:], rhs=kT[0][:, 0:HB],
                         start=(w == 0), stop=(w == 23))

    def gather_group(g, split=1):
        G, psE = gather_dma(g, split=split)
        gather_wordsum(G, psE, range(L))
        gather_finish(g, psE)

    gather_group(0, split=2)
    nc.sync.dma_start(out=kvt[:], in_=kvtf[:])
    gather_group(1)
    # Groups 2..7 stream under the scan: DMA issued at the block start, the
    # word-sum spread 4 slots per step, finalize at the block end.
    pend = [None]
    for t in range(S):
        ti = t % 8
        if ti == 0 and t // 8 + 2 < NG:
            pend[0] = (t // 8 + 2,) + gather_dma(t // 8 + 2)
        if pend[0] is not None:
            gp, G, psE = pend[0]
            gather_wordsum(G, psE, range(4 * ti, 4 * ti + 4))
            if ti == 7:
                gather_finish(gp, psE)
                pend[0] = None
        st = [phase_a(t, 0), phase_a(t, 1)]
        h = [phase_b(t, gb, *st[gb]) for gb in range(2)]

    # ---- output: transpose h^T [256, 512] -> [512, 256] fp32 ----
    for q in range(4):
        gb, half = q // 2, q % 2
        ho = work.tile([128, D], F32, tag="ho")
        for j in range(2):
            pt = psG.tile([128, 128], BF16, tag="pgs", name="ptout")
            nc.tensor.transpose(pt[:], h[gb][:, j, 128 * half:128 * half + 128],
                                I128[:])
            nc.vector.tensor_copy(out=ho[:, 128 * j:128 * (j + 1)], in_=pt[:])
        nc.sync.dma_start(out=hout[128 * q:128 * (q + 1), :], in_=ho[:])

    ctx.close()


def _prep_core(pr, mask, keys_c, emb):
    """Host-side marshaling for one core's shard."""
    uniq, inv = np.unique(pr, return_inverse=True)
    assert len(uniq) <= TABLE_ROWS
    table = np.zeros((TABLE_ROWS, D), dtype=ml_dtypes.bfloat16)
    table[: len(uniq)] = emb[uniq].astype(ml_dtypes.bfloat16)
    ranks = inv.reshape(BL, S, L).astype(np.int16)

    # token order per group g: i = (ds*16 + b)*32 + w
    idx_groups = []
    for g in range(NG):
        blk = ranks[:, 8 * g:8 * (g + 1), :]          # [b, ds, w]
        lst = blk.transpose(1, 0, 2).reshape(-1)      # [(ds, b, w)] length 4096
        idx_groups.append(np.tile(lst.reshape(TOKG // 16, 16).T, (8, 1)))
    idx16 = np.concatenate(idx_groups, axis=1).astype(np.int16)  # [128, NG*256]

    keysT = np.ascontiguousarray(
        keys_c.reshape(BK, D).T).astype(ml_dtypes.bfloat16)      # [256, 512]


    # mbig[r=(ds,gb,b), g, c=(bglob,k)] gate-logit offsets:
    #   0    where the gate is live (diagonal batch, unmasked step),
    #   -35  diagonal but step-masked  (exp(-logit) huge -> sigmoid ~ 0),
    #   +35  off-diagonal              (exp(-logit) ~ 0, vanishes in the sum).
    m = mask.astype(bool)                                        # [16, 64]
    mb = np.full((128, NG, BK), 35.0, np.float32)
    r = np.arange(128)
    ds_, gb_, b_ = r // 16, (r % 16) // 8, r % 8
    bglob = 8 * gb_ + b_
    for g in range(NG):
        t_ = 8 * g + ds_                                         # [128]
        ok_row = m[bglob, t_]                                    # [128]
        cols = (np.arange(BK) // K)[None, :] == bglob[:, None]   # [128, BK]
        gm_ = mb[:, g, :]
        gm_[cols] = np.where(np.repeat(ok_row, K), 0.0, -35.0)
        mb[:, g, :] = gm_
    return table, idx16, keysT, mb.astype(ml_dtypes.bfloat16)


def kernel(prgrph, prgrph_mask, keys, embedding_matrix, U, V, W):
    prgrph = np.asarray(prgrph)
    prgrph_mask = np.asarray(prgrph_mask)
    keys = np.asarray(keys, dtype=np.float32)
    emb = np.asarray(embedding_matrix, dtype=np.float32)
    U = np.asarray(U, dtype=np.float32)
    V = np.asarray(V, dtype=np.float32)
    W = np.asarray(W, dtype=np.float32)

    if "nc" not in _CACHED:
        _CACHED["nc"] = _build_program()
    nc = _CACHED["nc"]

    Ub, Vb, Wb = (x.astype(ml_dtypes.bfloat16) for x in (U, V, W))

    in_maps = []
    for c in range(NC):
        sl = slice(BL * c, BL * (c + 1))
        table, idx16, keysT, mb = _prep_core(
            prgrph[sl], prgrph_mask[sl, :, 0], keys[sl], emb)
        kv = keys[sl].reshape(BK, D) @ V                       # [bk, de]
        kvtf = np.ascontiguousarray(
            kv.T.reshape(2, 128, BL, K).transpose(1, 0, 2, 3)).astype(ml_dtypes.bfloat16)
        in_maps.append({
            "table": table, "idx16": idx16, "keysT": keysT,
            "Umat": Ub, "Vmat": Vb, "Wmat": Wb,
            "mbig": mb, "kvtf": kvtf,
        })

    res = run_bass_kernel_spmd(nc, in_maps, core_ids=list(range(NC)))
    out = np.concatenate(
        [res.results[c]["hout"].reshape(BL, K, D) for c in range(NC)], axis=0)
    return out.astype(np.float32)


# revision 55
# speedup vs baseline: 1.0649x; 1.0088x over previous
"""Trainium2 Bass kernel for nn_BasicRecurrentEntityEncoder.

Full-input contract: kernel(**inputs) takes the complete (unsharded) numpy
inputs and returns the full [B, K, D] float32 output. Internally the batch
is sharded over 8 NeuronCores (data parallel, no collectives), the embedding
bag-of-words gather runs through dma_gather against a per-core compacted
bf16 table, and the 64-step entity recurrence runs in a transposed
[D, (b,k)] layout with bf16 matmul operands.

Structure (625.8us baseline -> 433.1us):
  - Gather pipeline fully overlapped with the scan: groups 0-1 up front
    (group 0's dma split in two so the word-sum starts early, behind a PE
    p-state warm-up), groups 2-7 streamed under scan block g-2 with the
    word-sum spread 4 slot-matmuls per step. Dedicated gather PSUM banks
    (psG) keep it off the scan's PSUM tags.
  - e.keys gate logits precomputed per group ([128,512] matmul) with the
    sentence mask and block-diagonal batch mask folded in host-side as
    +-35 logit offsets (gk0m); per step they are matmul-injected into the
    gate bank, so the gate path is inject + 2 E_t^T h matmuls + one
    ScalarE exp. The exp(-logit) rows are broadcast-SUMMED by a ones8
    matmul (masked entries vanish), and the sigmoid reciprocal is fused
    into the custom DVE op MULSIG: u = relu(psh) * 1/(1+sum exp), with
    relu on ScalarE (HW allows only one PSUM operand per DVE op).
  - kvt = keys V is computed on the host; kvt + eW broadcasts are PSUM
    preloads (h-independent), U^T h accumulates on top.
  - sq = (u+h)^2 fused into the 2-stage custom DVE op ADDSQ so the sumsq
    path is one uninterruptible op (the other group's MULSIG used to land
    between upd and sq); upd = u+h (2x tensor_tensor) is deferred to the
    norm phase since only hn consumes it. relu/MULSIG are split by de-half
    so ScalarE and DVE pipeline. Norm rsqrt = exp(-.5 ln)
    on ScalarE with ln written back to PSUM (cheaper access), inv
    broadcast by a ones1 matmul, h_new = upd * inv on DVE.
  - Two batch groups (b 0-7 | b 8-15) with separate PSUM banks pipeline
    the serial dependency chain across engines.
"""

import sys

if "/opt/trn_rl_repo" not in sys.path:
    sys.path.insert(0, "/opt/trn_rl_repo")

import numpy as np
import ml_dtypes

from concourse import bacc, mybir
import concourse.bass as bass
import concourse.tile as tile
from concourse.bass_utils import run_bass_kernel_spmd
from concourse.masks import make_identity

# Force every ScalarE activation onto the one table set that covers all the
# functions this kernel uses (relu/exp/ln/copy/identity). The default
# chooser greedily picks the first set per function, inserting ~550ns table
# reloads on the critical path. Padding the dict keeps act_func_set_id
# indices aligned with act_info.json while making only the all-covering set
# usable.
_ONE_SET = "natural_log_exp_and_others"


import concourse.hw_specs as _hw_specs
_ORIG_TABLES = _hw_specs.get_activation_tables


def _patched_tables(module_arch):
    real = _ORIG_TABLES(module_arch)
    names = list(real.keys())
    assert _ONE_SET in names, names
    out = {}
    for n in names:
        if n == _ONE_SET:
            out[n] = real[n]
            break
        out[n] = set()
    return out


def _install_table_patch():
    import functools
    cached = functools.cache(_patched_tables)
    bacc.get_activation_tables = cached
    _hw_specs.get_activation_tables = cached


_install_table_patch()

# Custom DVE op #1: out ~= 1/(1 + in0) in ONE VectorE instruction (8 ALU
# stages): u = in0+1; seed y0 = bitcast(~bits(u)); t = u*y0 lands in
# [-4.5, -4] for any positive u; quadratic minimax fixup P(t) ~= 1/t gives
# out = y0*P(t) at ~1e-5 relative error.
import concourse.dve_ops as _dve_ops
from concourse.dve_spec import AluOp as _AluOp, Bin as _Bin, Spec as _Spec
from concourse.dve_spec import C0 as _C0, C1 as _C1, C2 as _C2, One as _One
from concourse.dve_spec import Src0 as _Src0, Src1 as _Src1, relu as _relu
from concourse.dve_spec import lower as _dve_lower
from concourse.dve_spec import _has_src1 as _dve_has_src1
from concourse.dve_uop import DveOpSpec as _DveOpSpec

# MULSIG fuses u = r * sigmoid(logit): in0 = relu(psh) (SBUF bf16, relu on
# ScalarE -- HW allows only ONE PSUM input per DVE op so the gate broadcast
# keeps the PSUM slot), in1 = sum_sent exp(-logit) from the ones8 matmul.
# out = in0 * 1/(1+in1) via the bitwise-NOT seed and a LINEAR fixup
# P(t) = c0 + c1*t on t in [-4.5, -4] (max sigmoid abs err ~1.4e-3).
_RS_C0, _RS_C1 = -0.47250233, -0.05572371


def _mulsig_ref(in0, in1, c0, c1, c2):
    u = (np.asarray(in1, np.float32) + np.float32(1.0)).astype(np.float32)
    y0 = (~u.view(np.int32)).view(np.float32)
    t = (u * y0).astype(np.float32)
    sig = (y0 * (np.float32(c0) + np.float32(c1) * t)).astype(np.float32)
    return np.asarray(in0, np.float32) * sig


def _register_op(name, spec):
    row = 1 + len(_dve_ops.OPS)
    assert row < 0x20
    shas = {}
    for ver in ("v3", "v4"):
        s = _DveOpSpec(name=name, opcode=row, uops=_dve_lower(spec, ver=ver),
                       rd1_en=_dve_has_src1(spec))
        shas[ver] = s.sha(ver)
    op = _dve_ops.DveOp(name, spec, subdim=False, uops_sha=shas)
    _dve_ops.OPS.append(op)
    _dve_ops._SUB_OPCODE_FOR_NAME[name] = row
    _dve_ops.CUSTOM_DVE_SPECS[name] = spec
    return op


def _relusig_ref(in0, in1, c0, c1, c2):
    r = np.maximum(np.nan_to_num(np.asarray(in0, np.float32), nan=0.0), 0.0)
    return _mulsig_ref(r, in1, c0, c1, c2)


def _make_mulsig():
    u = _Bin(_AluOp.ADD, _Src1, _One)
    y0 = _Bin(_AluOp.BITWISE_NOT, u, u)
    t = u * y0
    sig = y0 * (_C0 + _C1 * t)
    return _register_op("MULSIG_ANT",
                        _Spec(body=_Src0 * sig, reference=_mulsig_ref))


def _make_relusig():
    u = _Bin(_AluOp.ADD, _Src1, _One)
    y0 = _Bin(_AluOp.BITWISE_NOT, u, u)
    t = u * y0
    sig = y0 * (_C0 + _C1 * t)
    return _register_op("RELUSIG_ANT",
                        _Spec(body=_relu(_Src0) * sig, reference=_relusig_ref))


def _addsq_ref(in0, in1, c0, c1, c2):
    a = (np.asarray(in0, np.float32) + np.asarray(in1, np.float32)).astype(np.float32)
    return a * a


def _make_addsq():
    a = _Bin(_AluOp.ADD, _Src0, _Src1)
    return _register_op("ADDSQ_ANT",
                        _Spec(body=a * a, reference=_addsq_ref))


_MULSIG = _make_mulsig()
_RELUSIG = _make_relusig()
_ADDSQ = _make_addsq()

F32 = mybir.dt.float32
BF16 = mybir.dt.bfloat16
I16 = mybir.dt.int16
AF = mybir.ActivationFunctionType
OP = mybir.AluOpType

B, S, L, K, D = 128, 64, 32, 32, 256
NC = 8
BL = B // NC              # 16 batch rows per core
BK = BL * K               # 512 = free dim of the state
NG = 8                    # gather groups per core (128 sentences each)
TOKG = 128 * L            # 4096 tokens per group
TABLE_ROWS = 32768        # compacted per-core vocab (unique ids <= 32768)
EPS = 1e-12

_CACHED = {}


def _build_program():
    nc = bacc.Bacc("TRN2", target_bir_lowering=False, debug=False, num_devices=NC)

    table = nc.dram_tensor("table", [TABLE_ROWS, D], BF16, kind="ExternalInput").ap()
    idx16 = nc.dram_tensor("idx16", [128, NG * TOKG // 16], I16, kind="ExternalInput").ap()
    keysT = nc.dram_tensor("keysT", [D, BK], BF16, kind="ExternalInput").ap()
    Umat = nc.dram_tensor("Umat", [D, D], BF16, kind="ExternalInput").ap()
    Vmat = nc.dram_tensor("Vmat", [D, D], BF16, kind="ExternalInput").ap()
    Wmat = nc.dram_tensor("Wmat", [D, D], BF16, kind="ExternalInput").ap()
    mbig = nc.dram_tensor("mbig", [128, NG, BK], BF16, kind="ExternalInput").ap()
    kvtf = nc.dram_tensor("kvtf", [128, 2, BL, K], BF16, kind="ExternalInput").ap()
    hout = nc.dram_tensor("hout", [BK, D], F32, kind="ExternalOutput").ap()

    with tile.TileContext(nc) as tc:
        _emit(nc, tc, table, idx16, keysT, Umat, Vmat, Wmat, mbig, kvtf, hout)
    nc.compile()
    return nc


def _emit(nc, tc, table, idx16, keysT, Umat, Vmat, Wmat, mbig, kvtf, hout):
    from contextlib import ExitStack

    ctx = ExitStack()
    const = ctx.enter_context(tc.tile_pool(name="const", bufs=1))
    persist = ctx.enter_context(tc.tile_pool(name="persist", bufs=1))
    gpool = ctx.enter_context(tc.tile_pool(name="g", bufs=2))
    work = ctx.enter_context(tc.tile_pool(name="work", bufs=4))
    hpool = ctx.enter_context(tc.tile_pool(name="h", bufs=3))
    # PSUM budget (8 banks): psH = pshG double-buffered x2 groups (4 banks,
    # with the step's sumsq row overlaid into bank rows after relusig reads
    # it); psM = psg+gate-bcast [128,256] x2 groups (1 bank); psB = inv
    # bcast [128,256] x2 groups (1 bank); psG = gather scratch (2 banks) so
    # the gather pipeline overlaps the scan instead of serializing on scan
    # PSUM tags.
    psH = ctx.enter_context(tc.tile_pool(name="psH", bufs=1, space="PSUM"))
    psM = ctx.enter_context(tc.tile_pool(name="psM", bufs=1, space="PSUM"))
    psB = ctx.enter_context(tc.tile_pool(name="psB", bufs=1, space="PSUM"))
    psG = ctx.enter_context(tc.tile_pool(name="psG", bufs=1, space="PSUM"))

    # ---- constants into SBUF ----
    sb_idx = const.tile([128, NG * TOKG // 16], I16)
    nc.sync.dma_start(out=sb_idx[:, 0:TOKG // 16], in_=idx16[:, 0:TOKG // 16])
    nc.sync.dma_start(out=sb_idx[:, TOKG // 16:], in_=idx16[:, TOKG // 16:])
    kT = [const.tile([128, BK], BF16, tag=f"kT{j}", name=f"kT{j}") for j in range(2)]
    for j in range(2):
        nc.sync.dma_start(out=kT[j][:], in_=keysT[128 * j:128 * (j + 1), :])
    sbU = [const.tile([128, D], BF16, tag=f"sbU{j}", name=f"sbU{j}") for j in range(2)]
    sbV = [const.tile([128, D], BF16, tag=f"sbV{j}", name=f"sbV{j}") for j in range(2)]
    sbW = [const.tile([128, D], BF16, tag=f"sbW{j}", name=f"sbW{j}") for j in range(2)]
    for j in range(2):
        nc.sync.dma_start(out=sbU[j][:], in_=Umat[128 * j:128 * (j + 1), :])
        nc.sync.dma_start(out=sbV[j][:], in_=Vmat[128 * j:128 * (j + 1), :])
        nc.sync.dma_start(out=sbW[j][:], in_=Wmat[128 * j:128 * (j + 1), :])
    sb_mb = const.tile([128, NG, BK], BF16)
    nc.sync.dma_start(out=sb_mb[:], in_=mbig[:])

    I128 = const.tile([128, 128], BF16)
    make_identity(nc, I128[:])
    ones8 = const.tile([8, 128], BF16)
    nc.vector.memset(ones8[:], 1.0)
    ones128 = const.tile([128, 1], BF16)
    nc.vector.memset(ones128[:], 1.0)
    ones1 = const.tile([1, 128], BF16)
    nc.vector.memset(ones1[:], 1.0)
    epsap = const.tile([1, 1], F32)
    nc.vector.memset(epsap[:], EPS)
    # word-sum reducers: Ablk[i][p, m] = 1 iff m == 4*i + p//32.
    Ablk = []
    for i in range(16):
        a = const.tile([128, 64], BF16, tag=f"Ablk{i}", name=f"Ablk{i}")
        nc.vector.memset(a[:], 0.0)
        for q in range(4):
            nc.vector.memset(a[32 * q:32 * (q + 1), 4 * i + q:4 * i + q + 1], 1.0)
        Ablk.append(a)

    # ---- persistent intermediates ----
    ET = [persist.tile([128, NG * 128], BF16, tag=f"ET{j}", name=f"ET{j}") for j in range(2)]
    eWc = persist.tile([128, 2, NG * 128], BF16, tag="eWc", name="eWc")
    # kvt = keys V, host-computed; shaped [128, 2(de half), BL, K]
    kvt = persist.tile([128, 2, BL, K], BF16, tag="kvt", name="kvt")
    gk0m = persist.tile([128, NG, BK], BF16, tag="gk0m", name="gk0m")

    def gather_dma(g, split=1, petag="pgs"):
        G = gpool.tile([128, L, D], BF16, tag="G")
        HT = TOKG // split
        for hseg in range(split):
            nc.gpsimd.dma_gather(
                out_ap=G[:, (L // split) * hseg:(L // split) * (hseg + 1), :],
                in_ap=table[:],
                idxs_ap=sb_idx[:, (TOKG // 16) * g + (HT // 16) * hseg:
                               (TOKG // 16) * g + (HT // 16) * (hseg + 1)],
                num_idxs=HT, num_idxs_reg=HT, elem_size=D, single_packet=False,
            )
        psE = psG.tile([128, D], F32, tag=petag, name="psE")
        return G, psE

    def gather_wordsum(G, psE, cs):
        for c in cs:
            j, i = c // 16, c % 16
            nc.tensor.matmul(psE[64 * j:64 * (j + 1), :], lhsT=Ablk[i][:],
                             rhs=G[:, c, :], start=(i == 0), stop=(i == 15))

    def gather_finish(g, psE):
        enc = work.tile([128, D], BF16, tag="enc")
        nc.scalar.copy(out=enc[:], in_=psE[:])
        for j in range(2):
            pt = psG.tile([128, 128], BF16, tag="pgs", name="pt")
            nc.tensor.transpose(pt[:], enc[:, 128 * j:128 * (j + 1)], I128[:])
            nc.vector.tensor_copy(out=ET[j][:, 128 * g:128 * (g + 1)], in_=pt[:])
        for m in range(2):
            pw = psG.tile([128, 128], F32, tag="pgs", name="pw")
            nc.tensor.matmul(pw[:], lhsT=sbW[0][:, 128 * m:128 * (m + 1)],
                             rhs=ET[0][:, 128 * g:128 * (g + 1)], start=True, stop=False)
            nc.tensor.matmul(pw[:], lhsT=sbW[1][:, 128 * m:128 * (m + 1)],
                             rhs=ET[1][:, 128 * g:128 * (g + 1)], start=False, stop=True)
            nc.vector.tensor_copy(out=eWc[:, m, 128 * g:128 * (g + 1)], in_=pw[:])
        pgk = psG.tile([128, BK], F32, tag="pgk", name="pgk")
        nc.tensor.matmul(pgk[:], lhsT=ET[0][:, 128 * g:128 * (g + 1)], rhs=kT[0][:],
                         start=True, stop=False)
        nc.tensor.matmul(pgk[:], lhsT=ET[1][:, 128 * g:128 * (g + 1)], rhs=kT[1][:],
                         start=False, stop=True)
        nc.vector.tensor_tensor(out=gk0m[:, g, :], in0=pgk[:],
                                in1=sb_mb[:, g, :], op=OP.add)

    # ---- scan: two pipelined batch groups (b 0-7 | b 8-15) ----
    HB = BK // 2  # 256
    h = [hpool.tile([128, 2, HB], BF16, tag=f"h{gb}", name=f"h{gb}")
         for gb in range(2)]
    for gb in range(2):
        nc.vector.memset(h[gb][:], 0.0)

    def phase_a(t, gb):
        """Injections, h-dependent matmuls, gate, relu*sigmoid, upd, sq."""
        g, ds = t // 8, t % 8
        cg = 128 * g + 16 * ds + 8 * gb
        bks = slice(HB * gb, HB * (gb + 1))
        hg = h[gb]

        # h-independent PSUM preloads
        pshG = psH.tile([128, 2, HB], F32, tag=f"psh{gb}", name=f"psh{gb}")
        nc.tensor.matmul(pshG[:, :, :], lhsT=I128[:],
                         rhs=kvt[:, :, 8 * gb:8 * gb + 8, :], start=True, stop=False)
        for m in range(2):
            ew_bc = eWc[:, m, cg:cg + 8].unsqueeze(2).broadcast_to([128, 8, 32])
            nc.tensor.matmul(pshG[:, m, :], lhsT=I128[:], rhs=ew_bc,
                             start=False, stop=False)
        psMt = psM.tile([128, BK], F32, tag=f"psm{gb}", name=f"psm{gb}")
        psg = psMt[0:8, 0:HB]
        off = 16 * ds + 8 * gb
        nc.tensor.matmul(psg, lhsT=I128[:, off:off + 8],
                         rhs=gk0m[:, g, bks], start=True, stop=False)

        # h-dependent matmuls
        nc.tensor.matmul(psg, lhsT=ET[0][:, cg:cg + 8], rhs=hg[:, 0, :],
                         start=False, stop=False)
        nc.tensor.matmul(psg, lhsT=ET[1][:, cg:cg + 8], rhs=hg[:, 1, :],
                         start=False, stop=True)
        for m in range(2):
            nc.tensor.matmul(pshG[:, m, :], lhsT=sbU[0][:, 128 * m:128 * (m + 1)],
                             rhs=hg[:, 0, :], start=False, stop=False)
            nc.tensor.matmul(pshG[:, m, :], lhsT=sbU[1][:, 128 * m:128 * (m + 1)],
                             rhs=hg[:, 1, :], start=False, stop=(m == 1))

        # gate: eg = exp(-logit); sigmoid folded into RELUSIG below. The
        # gate broadcast overwrites the psg columns (WAR after eg reads).
        eg = work.tile([8, HB], BF16, tag=f"eg{gb}", name=f"eg{gb}")
        nc.scalar.activation(eg[:], psg, AF.Exp, scale=-1.0)
        nc.tensor.matmul(psMt[:, 0:HB], lhsT=ones8[:], rhs=eg[:],
                         start=True, stop=True)

        # u = relu(psh) * sigmoid, split by de-half so the second relu
        # half overlaps the first MULSIG half.
        r = work.tile([128, 2, HB], BF16, tag=f"r{gb}", name=f"r{gb}")
        u = work.tile([128, 2, HB], BF16, tag=f"u{gb}", name=f"u{gb}")
        for m in range(2):
            nc.scalar.activation(r[:, m, :], pshG[:, m, :], AF.Relu)
            nc.vector._custom_dve(
                _MULSIG, out=u[:, m, :], in0=r[:, m, :],
                in1=psMt[:, 0:HB],
                s0=float(_RS_C0), s1=float(_RS_C1))
        # sq = (u+h)^2 fused (keeps the sumsq path one uninterruptible DVE
        # op); upd itself is only needed by hn much later.
        sq = work.tile([128, 2, HB], BF16, tag=f"sq{gb}", name=f"sq{gb}")
        nc.vector._custom_dve(_ADDSQ, out=sq[:], in0=u[:], in1=hg[:])
        return psMt, (u, hg), sq

    def phase_b(t, gb, psMt, uh, sq):
        """Norm tail: sumsq, rsqrt via ln/exp, inv broadcast, hn."""
        u, hg = uh
        upd = work.tile([128, 2, HB], BF16, tag=f"upd{gb}", name=f"upd{gb}")
        nc.vector.tensor_tensor(out=upd[:], in0=u[:], in1=hg[:], op=OP.add)
        pss = psMt[0:1, HB:HB + HB]
        nc.tensor.matmul(pss, lhsT=ones128[:], rhs=sq[:, 0, :],
                         start=True, stop=False)
        nc.tensor.matmul(pss, lhsT=ones128[:], rhs=sq[:, 1, :],
                         start=False, stop=True)
        lns = psMt[32:33, HB:HB + HB]
        nc.scalar.activation(lns, pss, AF.Ln, bias=epsap[:])
        inv = work.tile([1, HB], BF16, tag=f"inv{gb}", name=f"inv{gb}")
        nc.scalar.activation(inv[:], lns, AF.Exp, scale=-0.5)
        psBI = psB.tile([128, HB], F32, tag=f"psb{gb}", name=f"psb{gb}")
        nc.tensor.matmul(psBI[:, :], lhsT=ones1[:], rhs=inv[:],
                         start=True, stop=True)
        hni = hpool.tile([128, 2, HB], BF16, tag=f"h{gb}", name=f"hn{gb}")
        nc.vector.tensor_tensor(
            out=hni[:], in0=upd[:],
            in1=psBI[:, :].unsqueeze(1).broadcast_to([128, 2, HB]),
            op=OP.mult)
        return hni

    # PE p-state warm-up: keep the PE continuously busy while the first
    # gather's DMA is in flight so the word-sum runs at full clock.
    warm = psB.tile([128, HB], F32, tag="psb1", name="warm")
    for w in range(24):
        nc.tensor.matmul(warm[:, :], lhsT=I128[:], rhs=kT[0][:, 0:HB],
                         start=(w == 0), stop=(w == 23))

    def gather_group(g, split=1):
        G, psE = gather_dma(g, split=split)
        gather_wordsum(G, psE, range(L))
        gather_finish(g, psE)

    gather_group(0, split=2)
    nc.sync.dma_start(out=kvt[:], in_=kvtf[:])
    # Groups 1..7 stream under the scan: DMA issued up front / at the block
    # start, the word-sum spread 4 slots per step, finalize at the block
    # end (group g's outputs are first needed at step 8*(g-1)). Group 1's
    # psE parks in the pgk bank so it can coexist with group 2's.
    pend = [(1, 0) + gather_dma(1, petag="pgk")]
    for t in range(S):
        ti = t % 8
        if ti == 0 and t // 8 + 2 < NG:
            pend.append((t // 8 + 2, t) + gather_dma(t // 8 + 2))
        for ent in pend[:]:
            gp, t0, G, psE = ent
            k = t - t0
            gather_wordsum(G, psE, range(4 * k, 4 * k + 4))
            if k == 7:
                gather_finish(gp, psE)
                pend.remove(ent)
        st = [phase_a(t, 0), phase_a(t, 1)]
        h = [phase_b(t, gb, *st[gb]) for gb in range(2)]

    # ---- output: transpose h^T [256, 512] -> [512, 256] fp32 ----
    for q in range(4):
        gb, half = q // 2, q % 2
        ho = work.tile([128, D], F32, tag="ho")
        for j in range(2):
            pt = psG.tile([128, 128], BF16, tag="pgs", name="ptout")
            nc.tensor.transpose(pt[:], h[gb][:, j, 128 * half:128 * half + 128],
                                I128[:])
            nc.vector.tensor_copy(out=ho[:, 128 * j:128 * (j + 1)], in_=pt[:])
        nc.sync.dma_start(out=hout[128 * q:128 * (q + 1), :], in_=ho[:])

    ctx.close()


def _prep_core(pr, mask, keys_c, emb):
    """Host-side marshaling for one core's shard."""
    uniq, inv = np.unique(pr, return_inverse=True)
    assert len(uniq) <= TABLE_ROWS
    table = np.zeros((TABLE_ROWS, D), dtype=ml_dtypes.bfloat16)
    table[: len(uniq)] = emb[uniq].astype(ml_dtypes.bfloat16)
    ranks = inv.reshape(BL, S, L).astype(np.int16)

    # token order per group g: i = (ds*16 + b)*32 + w
    idx_groups = []
    for g in range(NG):
        blk = ranks[:, 8 * g:8 * (g + 1), :]          # [b, ds, w]
        lst = blk.transpose(1, 0, 2).reshape(-1)      # [(ds, b, w)] length 4096
        idx_groups.append(np.tile(lst.reshape(TOKG // 16, 16).T, (8, 1)))
    idx16 = np.concatenate(idx_groups, axis=1).astype(np.int16)  # [128, NG*256]

    keysT = np.ascontiguousarray(
        keys_c.reshape(BK, D).T).astype(ml_dtypes.bfloat16)      # [256, 512]


    # mbig[r=(ds,gb,b), g, c=(bglob,k)] gate-logit offsets:
    #   0    where the gate is live (diagonal batch, unmasked step),
    #   -35  diagonal but step-masked  (exp(-logit) huge -> sigmoid ~ 0),
    #   +35  off-diagonal              (exp(-logit) ~ 0, vanishes in the sum).
    m = mask.astype(bool)                                        # [16, 64]
    mb = np.full((128, NG, BK), 35.0, np.float32)
    r = np.arange(128)
    ds_, gb_, b_ = r // 16, (r % 16) // 8, r % 8
    bglob = 8 * gb_ + b_
    for g in range(NG):
        t_ = 8 * g + ds_                                         # [128]
        ok_row = m[bglob, t_]                                    # [128]
        cols = (np.arange(BK) // K)[None, :] == bglob[:, None]   # [128, BK]
        gm_ = mb[:, g, :]
        gm_[cols] = np.where(np.repeat(ok_row, K), 0.0, -35.0)
        mb[:, g, :] = gm_
    return table, idx16, keysT, mb.astype(ml_dtypes.bfloat16)


def kernel(prgrph, prgrph_mask, keys, embedding_matrix, U, V, W):
    prgrph = np.asarray(prgrph)
    prgrph_mask = np.asarray(prgrph_mask)
    keys = np.asarray(keys, dtype=np.float32)
    emb = np.asarray(embedding_matrix, dtype=np.float32)
    U = np.asarray(U, dtype=np.float32)
    V = np.asarray(V, dtype=np.float32)
    W = np.asarray(W, dtype=np.float32)

    if "nc" not in _CACHED:
        _CACHED["nc"] = _build_program()
    nc = _CACHED["nc"]

    Ub, Vb, Wb = (x.astype(ml_dtypes.bfloat16) for x in (U, V, W))

    in_maps = []
    for c in range(NC):
        sl = slice(BL * c, BL * (c + 1))
        table, idx16, keysT, mb = _prep_core(
            prgrph[sl], prgrph_mask[sl, :, 0], keys[sl], emb)
        kv = keys[sl].reshape(BK, D) @ V                       # [bk, de]
        kvtf = np.ascontiguousarray(
            kv.T.reshape(2, 128, BL, K).transpose(1, 0, 2, 3)).astype(ml_dtypes.bfloat16)
        in_maps.append({
            "table": table, "idx16": idx16, "keysT": keysT,
            "Umat": Ub, "Vmat": Vb, "Wmat": Wb,
            "mbig": mb, "kvtf": kvtf,
        })

    res = run_bass_kernel_spmd(nc, in_maps, core_ids=list(range(NC)))
    out = np.concatenate(
        [res.results[c]["hout"].reshape(BL, K, D) for c in range(NC)], axis=0)
    return out.astype(np.float32)


# revision 68
# speedup vs baseline: 1.0724x; 1.0070x over previous
"""Trainium2 Bass kernel for nn_BasicRecurrentEntityEncoder.

Full-input contract: kernel(**inputs) takes the complete (unsharded) numpy
inputs and returns the full [B, K, D] float32 output. Internally the batch
is sharded over 8 NeuronCores (data parallel, no collectives), the embedding
bag-of-words gather runs through dma_gather against a per-core compacted
bf16 table, and the 64-step entity recurrence runs in a transposed
[D, (b,k)] layout with bf16 matmul operands.

Structure (625.8us baseline -> 429.4us):
  - Gather pipeline fully overlapped with the scan: group 0 up front (dma
    split in two so the word-sum starts early, behind a PE p-state
    warm-up); groups 1-7 streamed under the scan with the word-sum spread
    4 slot-matmuls per step (a monolithic group-1 word-sum used to block
    step 0 on the in-order PE for 5us). Dedicated gather PSUM banks (psG)
    keep it off the scan's PSUM tags.
  - e.keys gate logits precomputed per group ([128,512] matmul) with the
    sentence mask and block-diagonal batch mask folded in host-side as
    +-35 logit offsets (gk0m); per step they are matmul-injected into the
    gate bank, so the gate path is inject + 2 E_t^T h matmuls + one
    ScalarE exp. The exp(-logit) rows are broadcast-SUMMED by a ones8
    matmul (masked entries vanish), and the sigmoid reciprocal is fused
    into the custom DVE op MULSIG: u = relu(psh) * 1/(1+sum exp), with
    relu on ScalarE (HW allows only one PSUM operand per DVE op).
  - kvt = keys V is computed on the host; kvt + eW broadcasts are PSUM
    preloads (h-independent), U^T h accumulates on top.
  - sq = (u+h)^2 fused into the 2-stage custom DVE op ADDSQ so the sumsq
    path is one uninterruptible op (the other group's MULSIG used to land
    between upd and sq); upd = u+h (2x tensor_tensor) is deferred to the
    norm phase since only hn consumes it. relu/MULSIG are split by de-half
    so ScalarE and DVE pipeline. Norm rsqrt = exp(-.5 ln)
    on ScalarE with ln written back to PSUM (cheaper access), inv
    broadcast by a ones1 matmul, h_new = upd * inv on DVE.
  - Two batch groups (b 0-7 | b 8-15) with separate PSUM banks pipeline
    the serial dependency chain across engines.
"""

import sys

if "/opt/trn_rl_repo" not in sys.path:
    sys.path.insert(0, "/opt/trn_rl_repo")

import numpy as np
import ml_dtypes

from concourse import bacc, mybir
import concourse.bass as bass
import concourse.tile as tile
from concourse.bass_utils import run_bass_kernel_spmd
from concourse.masks import make_identity

# Force every ScalarE activation onto the one table set that covers all the
# functions this kernel uses (relu/exp/ln/copy/identity). The default
# chooser greedily picks the first set per function, inserting ~550ns table
# reloads on the critical path. Padding the dict keeps act_func_set_id
# indices aligned with act_info.json while making only the all-covering set
# usable.
_ONE_SET = "natural_log_exp_and_others"


import concourse.hw_specs as _hw_specs
_ORIG_TABLES = _hw_specs.get_activation_tables


def _patched_tables(module_arch):
    real = _ORIG_TABLES(module_arch)
    names = list(real.keys())
    assert _ONE_SET in names, names
    out = {}
    for n in names:
        if n == _ONE_SET:
            out[n] = real[n]
            break
        out[n] = set()
    return out


def _install_table_patch():
    import functools
    cached = functools.cache(_patched_tables)
    bacc.get_activation_tables = cached
    _hw_specs.get_activation_tables = cached


_install_table_patch()

# Custom DVE op #1: out ~= 1/(1 + in0) in ONE VectorE instruction (8 ALU
# stages): u = in0+1; seed y0 = bitcast(~bits(u)); t = u*y0 lands in
# [-4.5, -4] for any positive u; quadratic minimax fixup P(t) ~= 1/t gives
# out = y0*P(t) at ~1e-5 relative error.
import concourse.dve_ops as _dve_ops
from concourse.dve_spec import AluOp as _AluOp, Bin as _Bin, Spec as _Spec
from concourse.dve_spec import C0 as _C0, C1 as _C1, C2 as _C2, One as _One
from concourse.dve_spec import Src0 as _Src0, Src1 as _Src1, relu as _relu
from concourse.dve_spec import lower as _dve_lower
from concourse.dve_spec import _has_src1 as _dve_has_src1
from concourse.dve_uop import DveOpSpec as _DveOpSpec

# MULSIG fuses u = r * sigmoid(logit): in0 = relu(psh) (SBUF bf16, relu on
# ScalarE -- HW allows only ONE PSUM input per DVE op so the gate broadcast
# keeps the PSUM slot), in1 = sum_sent exp(-logit) from the ones8 matmul.
# out = in0 * 1/(1+in1) via the bitwise-NOT seed and a LINEAR fixup
# P(t) = c0 + c1*t on t in [-4.5, -4] (max sigmoid abs err ~1.4e-3).
_RS_C0, _RS_C1 = -0.47250233, -0.05572371


def _mulsig_ref(in0, in1, c0, c1, c2):
    u = (np.asarray(in1, np.float32) + np.float32(1.0)).astype(np.float32)
    y0 = (~u.view(np.int32)).view(np.float32)
    t = (u * y0).astype(np.float32)
    sig = (y0 * (np.float32(c0) + np.float32(c1) * t)).astype(np.float32)
    return np.asarray(in0, np.float32) * sig


def _register_op(name, spec):
    row = 1 + len(_dve_ops.OPS)
    assert row < 0x20
    shas = {}
    for ver in ("v3", "v4"):
        s = _DveOpSpec(name=name, opcode=row, uops=_dve_lower(spec, ver=ver),
                       rd1_en=_dve_has_src1(spec))
        shas[ver] = s.sha(ver)
    op = _dve_ops.DveOp(name, spec, subdim=False, uops_sha=shas)
    _dve_ops.OPS.append(op)
    _dve_ops._SUB_OPCODE_FOR_NAME[name] = row
    _dve_ops.CUSTOM_DVE_SPECS[name] = spec
    return op


def _relusig_ref(in0, in1, c0, c1, c2):
    r = np.maximum(np.nan_to_num(np.asarray(in0, np.float32), nan=0.0), 0.0)
    return _mulsig_ref(r, in1, c0, c1, c2)


def _make_mulsig():
    u = _Bin(_AluOp.ADD, _Src1, _One)
    y0 = _Bin(_AluOp.BITWISE_NOT, u, u)
    t = u * y0
    sig = y0 * (_C0 + _C1 * t)
    return _register_op("MULSIG_ANT",
                        _Spec(body=_Src0 * sig, reference=_mulsig_ref))


def _make_relusig():
    u = _Bin(_AluOp.ADD, _Src1, _One)
    y0 = _Bin(_AluOp.BITWISE_NOT, u, u)
    t = u * y0
    sig = y0 * (_C0 + _C1 * t)
    return _register_op("RELUSIG_ANT",
                        _Spec(body=_relu(_Src0) * sig, reference=_relusig_ref))


def _addsq_ref(in0, in1, c0, c1, c2):
    a = (np.asarray(in0, np.float32) + np.asarray(in1, np.float32)).astype(np.float32)
    return a * a


def _make_addsq():
    a = _Bin(_AluOp.ADD, _Src0, _Src1)
    return _register_op("ADDSQ_ANT",
                        _Spec(body=a * a, reference=_addsq_ref))


_MULSIG = _make_mulsig()
_RELUSIG = _make_relusig()
_ADDSQ = _make_addsq()

F32 = mybir.dt.float32
BF16 = mybir.dt.bfloat16
I16 = mybir.dt.int16
AF = mybir.ActivationFunctionType
OP = mybir.AluOpType

B, S, L, K, D = 128, 64, 32, 32, 256
NC = 8
BL = B // NC              # 16 batch rows per core
BK = BL * K               # 512 = free dim of the state
NG = 8                    # gather groups per core (128 sentences each)
TOKG = 128 * L            # 4096 tokens per group
TABLE_ROWS = 32768        # compacted per-core vocab (unique ids <= 32768)
EPS = 1e-12

_CACHED = {}


def _build_program():
    nc = bacc.Bacc("TRN2", target_bir_lowering=False, debug=False, num_devices=NC)

    table = nc.dram_tensor("table", [TABLE_ROWS, D], BF16, kind="ExternalInput").ap()
    idx16 = nc.dram_tensor("idx16", [128, NG * TOKG // 16], I16, kind="ExternalInput").ap()
    keysT = nc.dram_tensor("keysT", [D, BK], BF16, kind="ExternalInput").ap()
    Umat = nc.dram_tensor("Umat", [D, D], BF16, kind="ExternalInput").ap()
    Vmat = nc.dram_tensor("Vmat", [D, D], BF16, kind="ExternalInput").ap()
    Wmat = nc.dram_tensor("Wmat", [D, D], BF16, kind="ExternalInput").ap()
    mbig = nc.dram_tensor("mbig", [128, NG, BK], BF16, kind="ExternalInput").ap()
    kvtf = nc.dram_tensor("kvtf", [128, 2, BL, K], BF16, kind="ExternalInput").ap()
    hout = nc.dram_tensor("hout", [BK, D], F32, kind="ExternalOutput").ap()

    with tile.TileContext(nc) as tc:
        _emit(nc, tc, table, idx16, keysT, Umat, Vmat, Wmat, mbig, kvtf, hout)
    nc.compile()
    return nc


def _emit(nc, tc, table, idx16, keysT, Umat, Vmat, Wmat, mbig, kvtf, hout):
    from contextlib import ExitStack

    ctx = ExitStack()
    const = ctx.enter_context(tc.tile_pool(name="const", bufs=1))
    persist = ctx.enter_context(tc.tile_pool(name="persist", bufs=1))
    gpool = ctx.enter_context(tc.tile_pool(name="g", bufs=2))
    work = ctx.enter_context(tc.tile_pool(name="work", bufs=4))
    hpool = ctx.enter_context(tc.tile_pool(name="h", bufs=3))
    # PSUM budget (8 banks): psH = pshG double-buffered x2 groups (4 banks,
    # with the step's sumsq row overlaid into bank rows after relusig reads
    # it); psM = psg+gate-bcast [128,256] x2 groups (1 bank); psB = inv
    # bcast [128,256] x2 groups (1 bank); psG = gather scratch (2 banks) so
    # the gather pipeline overlaps the scan instead of serializing on scan
    # PSUM tags.
    psH = ctx.enter_context(tc.tile_pool(name="psH", bufs=1, space="PSUM"))
    psM = ctx.enter_context(tc.tile_pool(name="psM", bufs=1, space="PSUM"))
    psB = ctx.enter_context(tc.tile_pool(name="psB", bufs=1, space="PSUM"))
    psG = ctx.enter_context(tc.tile_pool(name="psG", bufs=1, space="PSUM"))

    # ---- constants into SBUF ----
    sb_idx = const.tile([128, NG * TOKG // 16], I16)
    nc.sync.dma_start(out=sb_idx[:, 0:TOKG // 16], in_=idx16[:, 0:TOKG // 16])
    nc.sync.dma_start(out=sb_idx[:, TOKG // 16:], in_=idx16[:, TOKG // 16:])
    kT = [const.tile([128, BK], BF16, tag=f"kT{j}", name=f"kT{j}") for j in range(2)]
    for j in range(2):
        nc.sync.dma_start(out=kT[j][:], in_=keysT[128 * j:128 * (j + 1), :])
    sbU = [const.tile([128, D], BF16, tag=f"sbU{j}", name=f"sbU{j}") for j in range(2)]
    sbV = [const.tile([128, D], BF16, tag=f"sbV{j}", name=f"sbV{j}") for j in range(2)]
    sbW = [const.tile([128, D], BF16, tag=f"sbW{j}", name=f"sbW{j}") for j in range(2)]
    for j in range(2):
        nc.sync.dma_start(out=sbU[j][:], in_=Umat[128 * j:128 * (j + 1), :])
        nc.sync.dma_start(out=sbV[j][:], in_=Vmat[128 * j:128 * (j + 1), :])
        nc.sync.dma_start(out=sbW[j][:], in_=Wmat[128 * j:128 * (j + 1), :])
    sb_mb = const.tile([128, NG, BK], BF16)
    nc.sync.dma_start(out=sb_mb[:], in_=mbig[:])

    I128 = const.tile([128, 128], BF16)
    make_identity(nc, I128[:])
    ones8 = const.tile([8, 128], BF16)
    nc.vector.memset(ones8[:], 1.0)
    ones128 = const.tile([128, 1], BF16)
    nc.vector.memset(ones128[:], 1.0)
    ones1 = const.tile([1, 128], BF16)
    nc.vector.memset(ones1[:], 1.0)
    epsap = const.tile([1, 1], F32)
    nc.vector.memset(epsap[:], EPS)
    # word-sum reducers: Ablk[i][p, m] = 1 iff m == 4*i + p//32.
    Ablk = []
    for i in range(16):
        a = const.tile([128, 64], BF16, tag=f"Ablk{i}", name=f"Ablk{i}")
        nc.vector.memset(a[:], 0.0)
        for q in range(4):
            nc.vector.memset(a[32 * q:32 * (q + 1), 4 * i + q:4 * i + q + 1], 1.0)
        Ablk.append(a)

    # ---- persistent intermediates ----
    ET = [persist.tile([128, NG * 128], BF16, tag=f"ET{j}", name=f"ET{j}") for j in range(2)]
    eWc = persist.tile([128, 2, NG * 128], BF16, tag="eWc", name="eWc")
    # kvt = keys V, host-computed; shaped [128, 2(de half), BL, K]
    kvt = persist.tile([128, 2, BL, K], BF16, tag="kvt", name="kvt")
    gk0m = persist.tile([128, NG, BK], BF16, tag="gk0m", name="gk0m")

    def gather_dma(g, split=1, petag="pgs"):
        G = gpool.tile([128, L, D], BF16, tag="G")
        HT = TOKG // split
        for hseg in range(split):
            nc.gpsimd.dma_gather(
                out_ap=G[:, (L // split) * hseg:(L // split) * (hseg + 1), :],
                in_ap=table[:],
                idxs_ap=sb_idx[:, (TOKG // 16) * g + (HT // 16) * hseg:
                               (TOKG // 16) * g + (HT // 16) * (hseg + 1)],
                num_idxs=HT, num_idxs_reg=HT, elem_size=D, single_packet=False,
            )
        psE = psG.tile([128, D], F32, tag=petag, name="psE")
        return G, psE

    def gather_wordsum(G, psE, cs):
        for c in cs:
            j, i = c // 16, c % 16
            nc.tensor.matmul(psE[64 * j:64 * (j + 1), :], lhsT=Ablk[i][:],
                             rhs=G[:, c, :], start=(i == 0), stop=(i == 15))

    def gather_finish(g, psE):
        enc = work.tile([128, D], BF16, tag="enc")
        nc.scalar.copy(out=enc[:], in_=psE[:])
        for j in range(2):
            pt = psG.tile([128, 128], BF16, tag="pgs", name="pt")
            nc.tensor.transpose(pt[:], enc[:, 128 * j:128 * (j + 1)], I128[:])
            nc.vector.tensor_copy(out=ET[j][:, 128 * g:128 * (g + 1)], in_=pt[:])
        for m in range(2):
            pw = psG.tile([128, 128], F32, tag="pgs", name="pw")
            nc.tensor.matmul(pw[:], lhsT=sbW[0][:, 128 * m:128 * (m + 1)],
                             rhs=ET[0][:, 128 * g:128 * (g + 1)], start=True, stop=False)
            nc.tensor.matmul(pw[:], lhsT=sbW[1][:, 128 * m:128 * (m + 1)],
                             rhs=ET[1][:, 128 * g:128 * (g + 1)], start=False, stop=True)
            nc.vector.tensor_copy(out=eWc[:, m, 128 * g:128 * (g + 1)], in_=pw[:])
        pgk = psG.tile([128, BK], F32, tag="pgk", name="pgk")
        nc.tensor.matmul(pgk[:], lhsT=ET[0][:, 128 * g:128 * (g + 1)], rhs=kT[0][:],
                         start=True, stop=False)
        nc.tensor.matmul(pgk[:], lhsT=ET[1][:, 128 * g:128 * (g + 1)], rhs=kT[1][:],
                         start=False, stop=True)
        nc.vector.tensor_tensor(out=gk0m[:, g, :], in0=pgk[:],
                                in1=sb_mb[:, g, :], op=OP.add)

    # ---- scan: two pipelined batch groups (b 0-7 | b 8-15) ----
    HB = BK // 2  # 256
    h = [hpool.tile([128, 2, HB], BF16, tag=f"h{gb}", name=f"h{gb}")
         for gb in range(2)]
    for gb in range(2):
        nc.vector.memset(h[gb][:], 0.0)

    def phase_a(t, gb):
        """Injections, h-dependent matmuls, gate, relu*sigmoid, upd, sq."""
        g, ds = t // 8, t % 8
        cg = 128 * g + 16 * ds + 8 * gb
        bks = slice(HB * gb, HB * (gb + 1))
        hg = h[gb]

        # h-independent PSUM preloads
        pshG = psH.tile([128, 2, HB], F32, tag=f"psh{gb}", name=f"psh{gb}")
        nc.tensor.matmul(pshG[:, :, :], lhsT=I128[:],
                         rhs=kvt[:, :, 8 * gb:8 * gb + 8, :], start=True, stop=False)
        for m in range(2):
            ew_bc = eWc[:, m, cg:cg + 8].unsqueeze(2).broadcast_to([128, 8, 32])
            nc.tensor.matmul(pshG[:, m, :], lhsT=I128[:], rhs=ew_bc,
                             start=False, stop=False)
        psMt = psM.tile([128, BK], F32, tag=f"psm{gb}", name=f"psm{gb}")
        psg = psMt[0:8, 0:HB]
        off = 16 * ds + 8 * gb
        nc.tensor.matmul(psg, lhsT=I128[:, off:off + 8],
                         rhs=gk0m[:, g, bks], start=True, stop=False)

        # h-dependent matmuls
        nc.tensor.matmul(psg, lhsT=ET[0][:, cg:cg + 8], rhs=hg[:, 0, :],
                         start=False, stop=False)
        nc.tensor.matmul(psg, lhsT=ET[1][:, cg:cg + 8], rhs=hg[:, 1, :],
                         start=False, stop=True)
        for m in range(2):
            nc.tensor.matmul(pshG[:, m, :], lhsT=sbU[0][:, 128 * m:128 * (m + 1)],
                             rhs=hg[:, 0, :], start=False, stop=False)
            nc.tensor.matmul(pshG[:, m, :], lhsT=sbU[1][:, 128 * m:128 * (m + 1)],
                             rhs=hg[:, 1, :], start=False, stop=(m == 1))

        # gate: eg = exp(-logit); sigmoid folded into RELUSIG below. The
        # gate broadcast overwrites the psg columns (WAR after eg reads).
        eg = work.tile([8, HB], BF16, tag=f"eg{gb}", name=f"eg{gb}")
        nc.scalar.activation(eg[:], psg, AF.Exp, scale=-1.0)
        nc.tensor.matmul(psMt[:, 0:HB], lhsT=ones8[:], rhs=eg[:],
                         start=True, stop=True)

        # u = relu(psh) * sigmoid, split by de-half so the second relu
        # half overlaps the first MULSIG half.
        r = work.tile([128, 2, HB], BF16, tag=f"r{gb}", name=f"r{gb}")
        u = work.tile([128, 2, HB], BF16, tag=f"u{gb}", name=f"u{gb}")
        for m in range(2):
            nc.scalar.activation(r[:, m, :], pshG[:, m, :], AF.Relu)
            nc.vector._custom_dve(
                _MULSIG, out=u[:, m, :], in0=r[:, m, :],
                in1=psMt[:, 0:HB],
                s0=float(_RS_C0), s1=float(_RS_C1))
        # sq = (u+h)^2 fused (keeps the sumsq path one uninterruptible DVE
        # op); upd itself is only needed by hn much later.
        sq = work.tile([128, 2, HB], BF16, tag=f"sq{gb}", name=f"sq{gb}")
        nc.vector._custom_dve(_ADDSQ, out=sq[:], in0=u[:], in1=hg[:])
        return psMt, (u, hg), sq

    def phase_b(t, gb, psMt, uh, sq):
        """Norm tail: sumsq, rsqrt via ln/exp, inv broadcast, hn."""
        u, hg = uh
        upd = work.tile([128, 2, HB], BF16, tag=f"upd{gb}", name=f"upd{gb}")
        nc.vector.tensor_tensor(out=upd[:], in0=u[:], in1=hg[:], op=OP.add)
        pss = psMt[0:1, HB:HB + HB]
        nc.tensor.matmul(pss, lhsT=ones128[:], rhs=sq[:, 0, :],
                         start=True, stop=False)
        nc.tensor.matmul(pss, lhsT=ones128[:], rhs=sq[:, 1, :],
                         start=False, stop=True)
        lns = psMt[32:33, HB:HB + HB]
        nc.scalar.activation(lns, pss, AF.Ln, bias=epsap[:])
        inv = work.tile([1, HB], BF16, tag=f"inv{gb}", name=f"inv{gb}")
        nc.scalar.activation(inv[:], lns, AF.Exp, scale=-0.5)
        psBI = psB.tile([128, HB], F32, tag=f"psb{gb}", name=f"psb{gb}")
        nc.tensor.matmul(psBI[:, :], lhsT=ones1[:], rhs=inv[:],
                         start=True, stop=True)
        hni = hpool.tile([128, 2, HB], BF16, tag=f"h{gb}", name=f"hn{gb}")
        nc.vector.tensor_tensor(
            out=hni[:], in0=upd[:],
            in1=psBI[:, :].unsqueeze(1).broadcast_to([128, 2, HB]),
            op=OP.mult)
        return hni

    # PE p-state warm-up: keep the PE continuously busy while the first
    # gather's DMA is in flight so the word-sum runs at full clock.
    warm = psB.tile([128, HB], F32, tag="psb1", name="warm")
    for w in range(8):
        nc.tensor.matmul(warm[:, :], lhsT=I128[:], rhs=kT[0][:, 0:HB],
                         start=(w == 0), stop=(w == 7))

    def gather_group(g, split=1):
        G, psE = gather_dma(g, split=split)
        gather_wordsum(G, psE, range(L))
        gather_finish(g, psE)

    gather_group(0, split=4)
    nc.sync.dma_start(out=kvt[:], in_=kvtf[:])
    # Groups 1..7 stream under the scan: DMA issued up front / at the block
    # start, the word-sum spread 4 slots per step, finalize at the block
    # end (group g's outputs are first needed at step 8*(g-1)). Group 1's
    # psE parks in the pgk bank so it can coexist with group 2's.
    pend = [(1, 0) + gather_dma(1, petag="pgk")]
    for t in range(S):
        ti = t % 8
        if ti == 0 and t // 8 + 2 < NG:
            pend.append((t // 8 + 2, t) + gather_dma(t // 8 + 2))
        for ent in pend[:]:
            gp, t0, G, psE = ent
            k = t - t0
            gather_wordsum(G, psE, range(4 * k, 4 * k + 4))
            if k == 7:
                gather_finish(gp, psE)
                pend.remove(ent)
        st = [phase_a(t, 0), phase_a(t, 1)]
        h = [phase_b(t, gb, *st[gb]) for gb in range(2)]

    # ---- output: transpose h^T [256, 512] -> [512, 256] fp32 ----
    for q in range(4):
        gb, half = q // 2, q % 2
        ho = work.tile([128, D], F32, tag="ho")
        for j in range(2):
            pt = psG.tile([128, 128], BF16, tag="pgs", name="ptout")
            nc.tensor.transpose(pt[:], h[gb][:, j, 128 * half:128 * half + 128],
                                I128[:])
            nc.vector.tensor_copy(out=ho[:, 128 * j:128 * (j + 1)], in_=pt[:])
        nc.sync.dma_start(out=hout[128 * q:128 * (q + 1), :], in_=ho[:])

    ctx.close()


def _prep_core(pr, mask, keys_c, emb):
    """Host-side marshaling for one core's shard."""
    uniq, inv = np.unique(pr, return_inverse=True)
    assert len(uniq) <= TABLE_ROWS
    table = np.zeros((TABLE_ROWS, D), dtype=ml_dtypes.bfloat16)
    table[: len(uniq)] = emb[uniq].astype(ml_dtypes.bfloat16)
    ranks = inv.reshape(BL, S, L).astype(np.int16)

    # token order per group g: i = (ds*16 + b)*32 + w
    idx_groups = []
    for g in range(NG):
        blk = ranks[:, 8 * g:8 * (g + 1), :]          # [b, ds, w]
        lst = blk.transpose(1, 0, 2).reshape(-1)      # [(ds, b, w)] length 4096
        idx_groups.append(np.tile(lst.reshape(TOKG // 16, 16).T, (8, 1)))
    idx16 = np.concatenate(idx_groups, axis=1).astype(np.int16)  # [128, NG*256]

    keysT = np.ascontiguousarray(
        keys_c.reshape(BK, D).T).astype(ml_dtypes.bfloat16)      # [256, 512]


    # mbig[r=(ds,gb,b), g, c=(bglob,k)] gate-logit offsets:
    #   0    where the gate is live (diagonal batch, unmasked step),
    #   -35  diagonal but step-masked  (exp(-logit) huge -> sigmoid ~ 0),
    #   +35  off-diagonal              (exp(-logit) ~ 0, vanishes in the sum).
    m = mask.astype(bool)                                        # [16, 64]
    mb = np.full((128, NG, BK), 35.0, np.float32)
    r = np.arange(128)
    ds_, gb_, b_ = r // 16, (r % 16) // 8, r % 8
    bglob = 8 * gb_ + b_
    for g in range(NG):
        t_ = 8 * g + ds_                                         # [128]
        ok_row = m[bglob, t_]                                    # [128]
        cols = (np.arange(BK) // K)[None, :] == bglob[:, None]   # [128, BK]
        gm_ = mb[:, g, :]
        gm_[cols] = np.where(np.repeat(ok_row, K), 0.0, -35.0)
        mb[:, g, :] = gm_
    return table, idx16, keysT, mb.astype(ml_dtypes.bfloat16)


def kernel(prgrph, prgrph_mask, keys, embedding_matrix, U, V, W):
    prgrph = np.asarray(prgrph)
    prgrph_mask = np.asarray(prgrph_mask)
    keys = np.asarray(keys, dtype=np.float32)
    emb = np.asarray(embedding_matrix, dtype=np.float32)
    U = np.asarray(U, dtype=np.float32)
    V = np.asarray(V, dtype=np.float32)
    W = np.asarray(W, dtype=np.float32)

    if "nc" not in _CACHED:
        _CACHED["nc"] = _build_program()
    nc = _CACHED["nc"]

    Ub, Vb, Wb = (x.astype(ml_dtypes.bfloat16) for x in (U, V, W))

    in_maps = []
    for c in range(NC):
        sl = slice(BL * c, BL * (c + 1))
        table, idx16, keysT, mb = _prep_core(
            prgrph[sl], prgrph_mask[sl, :, 0], keys[sl], emb)
        kv = keys[sl].reshape(BK, D) @ V                       # [bk, de]
        kvtf = np.ascontiguousarray(
            kv.T.reshape(2, 128, BL, K).transpose(1, 0, 2, 3)).astype(ml_dtypes.bfloat16)
        in_maps.append({
            "table": table, "idx16": idx16, "keysT": keysT,
            "Umat": Ub, "Vmat": Vb, "Wmat": Wb,
            "mbig": mb, "kvtf": kvtf,
        })

    res = run_bass_kernel_spmd(nc, in_maps, core_ids=list(range(NC)))
    out = np.concatenate(
        [res.results[c]["hout"].reshape(BL, K, D) for c in range(NC)], axis=0)
    return out.astype(np.float32)


# revision 74
# speedup vs baseline: 1.0742x; 1.0017x over previous
"""Trainium2 Bass kernel for nn_BasicRecurrentEntityEncoder.

Full-input contract: kernel(**inputs) takes the complete (unsharded) numpy
inputs and returns the full [B, K, D] float32 output. Internally the batch
is sharded over 8 NeuronCores (data parallel, no collectives), the embedding
bag-of-words gather runs through dma_gather against a per-core compacted
bf16 table, and the 64-step entity recurrence runs in a transposed
[D, (b,k)] layout with bf16 matmul operands.

Structure (625.8us baseline -> 429.4us):
  - Gather pipeline fully overlapped with the scan: group 0 up front (dma
    split in two so the word-sum starts early, behind a PE p-state
    warm-up); groups 1-7 streamed under the scan with the word-sum spread
    4 slot-matmuls per step (a monolithic group-1 word-sum used to block
    step 0 on the in-order PE for 5us). Dedicated gather PSUM banks (psG)
    keep it off the scan's PSUM tags.
  - e.keys gate logits precomputed per group ([128,512] matmul) with the
    sentence mask and block-diagonal batch mask folded in host-side as
    +-35 logit offsets (gk0m); per step they are matmul-injected into the
    gate bank, so the gate path is inject + 2 E_t^T h matmuls + one
    ScalarE exp. The exp(-logit) rows are broadcast-SUMMED by a ones8
    matmul (masked entries vanish), and the sigmoid reciprocal is fused
    into the custom DVE op MULSIG: u = relu(psh) * 1/(1+sum exp), with
    relu on ScalarE (HW allows only one PSUM operand per DVE op).
  - kvt = keys V is computed on the host; kvt + eW broadcasts are PSUM
    preloads (h-independent), U^T h accumulates on top.
  - sq = (u+h)^2 fused into the 2-stage custom DVE op ADDSQ so the sumsq
    path is one uninterruptible op (the other group's MULSIG used to land
    between upd and sq); upd = u+h (2x tensor_tensor) is deferred to the
    norm phase since only hn consumes it. relu/MULSIG are split by de-half
    so ScalarE and DVE pipeline. Norm rsqrt = exp(-.5 ln)
    on ScalarE with ln written back to PSUM (cheaper access), inv
    broadcast by a ones1 matmul, h_new = upd * inv on DVE.
  - Two batch groups (b 0-7 | b 8-15) with separate PSUM banks pipeline
    the serial dependency chain across engines.
"""

import sys

if "/opt/trn_rl_repo" not in sys.path:
    sys.path.insert(0, "/opt/trn_rl_repo")

import numpy as np
import ml_dtypes

from concourse import bacc, mybir
import concourse.bass as bass
import concourse.tile as tile
from concourse.bass_utils import run_bass_kernel_spmd
from concourse.masks import make_identity

# Force every ScalarE activation onto the one table set that covers all the
# functions this kernel uses (relu/exp/ln/copy/identity). The default
# chooser greedily picks the first set per function, inserting ~550ns table
# reloads on the critical path. Padding the dict keeps act_func_set_id
# indices aligned with act_info.json while making only the all-covering set
# usable.
_ONE_SET = "natural_log_exp_and_others"


import concourse.hw_specs as _hw_specs
_ORIG_TABLES = _hw_specs.get_activation_tables


def _patched_tables(module_arch):
    real = _ORIG_TABLES(module_arch)
    names = list(real.keys())
    assert _ONE_SET in names, names
    out = {}
    for n in names:
        if n == _ONE_SET:
            out[n] = real[n]
            break
        out[n] = set()
    return out


def _install_table_patch():
    import functools
    cached = functools.cache(_patched_tables)
    bacc.get_activation_tables = cached
    _hw_specs.get_activation_tables = cached


_install_table_patch()

# Custom DVE op #1: out ~= 1/(1 + in0) in ONE VectorE instruction (8 ALU
# stages): u = in0+1; seed y0 = bitcast(~bits(u)); t = u*y0 lands in
# [-4.5, -4] for any positive u; quadratic minimax fixup P(t) ~= 1/t gives
# out = y0*P(t) at ~1e-5 relative error.
import concourse.dve_ops as _dve_ops
from concourse.dve_spec import AluOp as _AluOp, Bin as _Bin, Spec as _Spec
from concourse.dve_spec import C0 as _C0, C1 as _C1, C2 as _C2, One as _One
from concourse.dve_spec import Src0 as _Src0, Src1 as _Src1, relu as _relu
from concourse.dve_spec import lower as _dve_lower
from concourse.dve_spec import _has_src1 as _dve_has_src1
from concourse.dve_uop import DveOpSpec as _DveOpSpec

# MULSIG fuses u = r * sigmoid(logit): in0 = relu(psh) (SBUF bf16, relu on
# ScalarE -- HW allows only ONE PSUM input per DVE op so the gate broadcast
# keeps the PSUM slot), in1 = sum_sent exp(-logit) from the ones8 matmul.
# out = in0 * 1/(1+in1) via the bitwise-NOT seed and a LINEAR fixup
# P(t) = c0 + c1*t on t in [-4.5, -4] (max sigmoid abs err ~1.4e-3).
_RS_C0, _RS_C1 = -0.47250233, -0.05572371


def _mulsig_ref(in0, in1, c0, c1, c2):
    u = (np.asarray(in1, np.float32) + np.float32(1.0)).astype(np.float32)
    y0 = (~u.view(np.int32)).view(np.float32)
    t = (u * y0).astype(np.float32)
    sig = (y0 * (np.float32(c0) + np.float32(c1) * t)).astype(np.float32)
    return np.asarray(in0, np.float32) * sig


def _register_op(name, spec):
    row = 1 + len(_dve_ops.OPS)
    assert row < 0x20
    shas = {}
    for ver in ("v3", "v4"):
        s = _DveOpSpec(name=name, opcode=row, uops=_dve_lower(spec, ver=ver),
                       rd1_en=_dve_has_src1(spec))
        shas[ver] = s.sha(ver)
    op = _dve_ops.DveOp(name, spec, subdim=False, uops_sha=shas)
    _dve_ops.OPS.append(op)
    _dve_ops._SUB_OPCODE_FOR_NAME[name] = row
    _dve_ops.CUSTOM_DVE_SPECS[name] = spec
    return op


def _relusig_ref(in0, in1, c0, c1, c2):
    r = np.maximum(np.nan_to_num(np.asarray(in0, np.float32), nan=0.0), 0.0)
    return _mulsig_ref(r, in1, c0, c1, c2)


def _make_mulsig():
    u = _Bin(_AluOp.ADD, _Src1, _One)
    y0 = _Bin(_AluOp.BITWISE_NOT, u, u)
    t = u * y0
    sig = y0 * (_C0 + _C1 * t)
    return _register_op("MULSIG_ANT",
                        _Spec(body=_Src0 * sig, reference=_mulsig_ref))


def _make_relusig():
    u = _Bin(_AluOp.ADD, _Src1, _One)
    y0 = _Bin(_AluOp.BITWISE_NOT, u, u)
    t = u * y0
    sig = y0 * (_C0 + _C1 * t)
    return _register_op("RELUSIG_ANT",
                        _Spec(body=_relu(_Src0) * sig, reference=_relusig_ref))


def _addsq_ref(in0, in1, c0, c1, c2):
    a = (np.asarray(in0, np.float32) + np.asarray(in1, np.float32)).astype(np.float32)
    return a * a


def _make_addsq():
    a = _Bin(_AluOp.ADD, _Src0, _Src1)
    return _register_op("ADDSQ_ANT",
                        _Spec(body=a * a, reference=_addsq_ref))


_MULSIG = _make_mulsig()
_RELUSIG = _make_relusig()
_ADDSQ = _make_addsq()

F32 = mybir.dt.float32
BF16 = mybir.dt.bfloat16
I16 = mybir.dt.int16
AF = mybir.ActivationFunctionType
OP = mybir.AluOpType

B, S, L, K, D = 128, 64, 32, 32, 256
NC = 8
BL = B // NC              # 16 batch rows per core
BK = BL * K               # 512 = free dim of the state
NG = 8                    # gather groups per core (128 sentences each)
TOKG = 128 * L            # 4096 tokens per group
TABLE_ROWS = 32768        # compacted per-core vocab (unique ids <= 32768)
EPS = 1e-12

_CACHED = {}


def _build_program():
    nc = bacc.Bacc("TRN2", target_bir_lowering=False, debug=False, num_devices=NC)

    table = nc.dram_tensor("table", [TABLE_ROWS, D], BF16, kind="ExternalInput").ap()
    idx16 = nc.dram_tensor("idx16", [128, NG * TOKG // 16], I16, kind="ExternalInput").ap()
    keysT = nc.dram_tensor("keysT", [D, BK], BF16, kind="ExternalInput").ap()
    Umat = nc.dram_tensor("Umat", [D, D], BF16, kind="ExternalInput").ap()
    Vmat = nc.dram_tensor("Vmat", [D, D], BF16, kind="ExternalInput").ap()
    Wmat = nc.dram_tensor("Wmat", [D, D], BF16, kind="ExternalInput").ap()
    mbig = nc.dram_tensor("mbig", [128, NG, BK], BF16, kind="ExternalInput").ap()
    kvtf = nc.dram_tensor("kvtf", [128, 2, BL, K], BF16, kind="ExternalInput").ap()
    hout = nc.dram_tensor("hout", [BK, D], F32, kind="ExternalOutput").ap()

    with tile.TileContext(nc) as tc:
        _emit(nc, tc, table, idx16, keysT, Umat, Vmat, Wmat, mbig, kvtf, hout)
    nc.compile()
    return nc


def _emit(nc, tc, table, idx16, keysT, Umat, Vmat, Wmat, mbig, kvtf, hout):
    from contextlib import ExitStack

    ctx = ExitStack()
    const = ctx.enter_context(tc.tile_pool(name="const", bufs=1))
    persist = ctx.enter_context(tc.tile_pool(name="persist", bufs=1))
    gpool = ctx.enter_context(tc.tile_pool(name="g", bufs=2))
    work = ctx.enter_context(tc.tile_pool(name="work", bufs=4))
    hpool = ctx.enter_context(tc.tile_pool(name="h", bufs=3))
    # PSUM budget (8 banks): psH = pshG double-buffered x2 groups (4 banks,
    # with the step's sumsq row overlaid into bank rows after relusig reads
    # it); psM = psg+gate-bcast [128,256] x2 groups (1 bank); psB = inv
    # bcast [128,256] x2 groups (1 bank); psG = gather scratch (2 banks) so
    # the gather pipeline overlaps the scan instead of serializing on scan
    # PSUM tags.
    psH = ctx.enter_context(tc.tile_pool(name="psH", bufs=1, space="PSUM"))
    psM = ctx.enter_context(tc.tile_pool(name="psM", bufs=1, space="PSUM"))
    psB = ctx.enter_context(tc.tile_pool(name="psB", bufs=1, space="PSUM"))
    psG = ctx.enter_context(tc.tile_pool(name="psG", bufs=1, space="PSUM"))

    # ---- constants into SBUF ----
    sb_idx = const.tile([128, NG * TOKG // 16], I16)
    nc.sync.dma_start(out=sb_idx[:, 0:TOKG // 16], in_=idx16[:, 0:TOKG // 16])
    nc.sync.dma_start(out=sb_idx[:, TOKG // 16:], in_=idx16[:, TOKG // 16:])
    kT = [const.tile([128, BK], BF16, tag=f"kT{j}", name=f"kT{j}") for j in range(2)]
    for j in range(2):
        nc.sync.dma_start(out=kT[j][:], in_=keysT[128 * j:128 * (j + 1), :])
    sbU = [const.tile([128, D], BF16, tag=f"sbU{j}", name=f"sbU{j}") for j in range(2)]
    sbV = [const.tile([128, D], BF16, tag=f"sbV{j}", name=f"sbV{j}") for j in range(2)]
    sbW = [const.tile([128, D], BF16, tag=f"sbW{j}", name=f"sbW{j}") for j in range(2)]
    for j in range(2):
        nc.sync.dma_start(out=sbU[j][:], in_=Umat[128 * j:128 * (j + 1), :])
        nc.sync.dma_start(out=sbV[j][:], in_=Vmat[128 * j:128 * (j + 1), :])
        nc.sync.dma_start(out=sbW[j][:], in_=Wmat[128 * j:128 * (j + 1), :])
    sb_mb = const.tile([128, NG, BK], BF16)
    nc.sync.dma_start(out=sb_mb[:], in_=mbig[:])

    I128 = const.tile([128, 128], BF16)
    make_identity(nc, I128[:])
    ones8 = const.tile([8, 128], BF16)
    nc.vector.memset(ones8[:], 1.0)
    ones128 = const.tile([128, 1], BF16)
    nc.vector.memset(ones128[:], 1.0)
    ones1 = const.tile([1, 128], BF16)
    nc.vector.memset(ones1[:], 1.0)
    epsap = const.tile([1, 1], F32)
    nc.vector.memset(epsap[:], EPS)
    one1f = const.tile([1, 1], F32)
    nc.vector.memset(one1f[:], 1.0)
    # word-sum reducers: Ablk[i][p, m] = 1 iff m == 4*i + p//32.
    Ablk = []
    for i in range(16):
        a = const.tile([128, 64], BF16, tag=f"Ablk{i}", name=f"Ablk{i}")
        nc.vector.memset(a[:], 0.0)
        for q in range(4):
            nc.vector.memset(a[32 * q:32 * (q + 1), 4 * i + q:4 * i + q + 1], 1.0)
        Ablk.append(a)

    # ---- persistent intermediates ----
    ET = [persist.tile([128, NG * 128], BF16, tag=f"ET{j}", name=f"ET{j}") for j in range(2)]
    eWc = persist.tile([128, 2, NG * 128], BF16, tag="eWc", name="eWc")
    # kvt = keys V, host-computed; shaped [128, 2(de half), BL, K]
    kvt = persist.tile([128, 2, BL, K], BF16, tag="kvt", name="kvt")
    gk0m = persist.tile([128, NG, BK], BF16, tag="gk0m", name="gk0m")

    def gather_dma(g, split=1, petag="pgs"):
        G = gpool.tile([128, L, D], BF16, tag="G")
        HT = TOKG // split
        for hseg in range(split):
            nc.gpsimd.dma_gather(
                out_ap=G[:, (L // split) * hseg:(L // split) * (hseg + 1), :],
                in_ap=table[:],
                idxs_ap=sb_idx[:, (TOKG // 16) * g + (HT // 16) * hseg:
                               (TOKG // 16) * g + (HT // 16) * (hseg + 1)],
                num_idxs=HT, num_idxs_reg=HT, elem_size=D, single_packet=False,
            )
        psE = psG.tile([128, D], F32, tag=petag, name="psE")
        return G, psE

    def gather_wordsum(G, psE, cs):
        for c in cs:
            j, i = c // 16, c % 16
            nc.tensor.matmul(psE[64 * j:64 * (j + 1), :], lhsT=Ablk[i][:],
                             rhs=G[:, c, :], start=(i == 0), stop=(i == 15))

    def gather_finish(g, psE):
        enc = work.tile([128, D], BF16, tag="enc")
        nc.scalar.copy(out=enc[:], in_=psE[:])
        for j in range(2):
            pt = psG.tile([128, 128], BF16, tag="pgs", name="pt")
            nc.tensor.transpose(pt[:], enc[:, 128 * j:128 * (j + 1)], I128[:])
            nc.vector.tensor_copy(out=ET[j][:, 128 * g:128 * (g + 1)], in_=pt[:])
        for m in range(2):
            pw = psG.tile([128, 128], F32, tag="pgs", name="pw")
            nc.tensor.matmul(pw[:], lhsT=sbW[0][:, 128 * m:128 * (m + 1)],
                             rhs=ET[0][:, 128 * g:128 * (g + 1)], start=True, stop=False)
            nc.tensor.matmul(pw[:], lhsT=sbW[1][:, 128 * m:128 * (m + 1)],
                             rhs=ET[1][:, 128 * g:128 * (g + 1)], start=False, stop=True)
            nc.vector.tensor_copy(out=eWc[:, m, 128 * g:128 * (g + 1)], in_=pw[:])
        pgk = psG.tile([128, BK], F32, tag="pgk", name="pgk")
        nc.tensor.matmul(pgk[:], lhsT=ET[0][:, 128 * g:128 * (g + 1)], rhs=kT[0][:],
                         start=True, stop=False)
        nc.tensor.matmul(pgk[:], lhsT=ET[1][:, 128 * g:128 * (g + 1)], rhs=kT[1][:],
                         start=False, stop=True)
        nc.vector.tensor_tensor(out=gk0m[:, g, :], in0=pgk[:],
                                in1=sb_mb[:, g, :], op=OP.add)

    # ---- scan: two pipelined batch groups (b 0-7 | b 8-15) ----
    HB = BK // 2  # 256
    h = [hpool.tile([128, 2, HB], BF16, tag=f"h{gb}", name=f"h{gb}")
         for gb in range(2)]
    for gb in range(2):
        nc.vector.memset(h[gb][:], 0.0)

    def phase_a(t, gb):
        """Injections, h-dependent matmuls, gate, relu*sigmoid, upd, sq."""
        g, ds = t // 8, t % 8
        cg = 128 * g + 16 * ds + 8 * gb
        bks = slice(HB * gb, HB * (gb + 1))
        hg = h[gb]

        # h-independent PSUM preloads
        pshG = psH.tile([128, 2, HB], F32, tag=f"psh{gb}", name=f"psh{gb}")
        nc.tensor.matmul(pshG[:, :, :], lhsT=I128[:],
                         rhs=kvt[:, :, 8 * gb:8 * gb + 8, :], start=True, stop=False)
        for m in range(2):
            ew_bc = eWc[:, m, cg:cg + 8].unsqueeze(2).broadcast_to([128, 8, 32])
            nc.tensor.matmul(pshG[:, m, :], lhsT=I128[:], rhs=ew_bc,
                             start=False, stop=False)
        psMt = psM.tile([128, BK], F32, tag=f"psm{gb}", name=f"psm{gb}")
        psg = psMt[0:8, 0:HB]
        off = 16 * ds + 8 * gb
        nc.tensor.matmul(psg, lhsT=I128[:, off:off + 8],
                         rhs=gk0m[:, g, bks], start=True, stop=False)

        # h-dependent matmuls
        nc.tensor.matmul(psg, lhsT=ET[0][:, cg:cg + 8], rhs=hg[:, 0, :],
                         start=False, stop=False)
        nc.tensor.matmul(psg, lhsT=ET[1][:, cg:cg + 8], rhs=hg[:, 1, :],
                         start=False, stop=True)
        for m in range(2):
            nc.tensor.matmul(pshG[:, m, :], lhsT=sbU[0][:, 128 * m:128 * (m + 1)],
                             rhs=hg[:, 0, :], start=False, stop=False)
            nc.tensor.matmul(pshG[:, m, :], lhsT=sbU[1][:, 128 * m:128 * (m + 1)],
                             rhs=hg[:, 1, :], start=False, stop=(m == 1))

        # gate: eg = exp(-logit); sigmoid folded into RELUSIG below. The
        # gate broadcast overwrites the psg columns (WAR after eg reads).
        eg = work.tile([8, HB], BF16, tag=f"eg{gb}", name=f"eg{gb}")
        nc.scalar.activation(eg[:], psg, AF.Exp, scale=-1.0)
        nc.tensor.matmul(psMt[:, 0:HB], lhsT=ones8[:], rhs=eg[:],
                         start=True, stop=True)

        # u = relu(psh) * sigmoid, split by de-half so the second relu
        # half overlaps the first MULSIG half.
        r = work.tile([128, 2, HB], BF16, tag=f"r{gb}", name=f"r{gb}")
        u = work.tile([128, 2, HB], BF16, tag=f"u{gb}", name=f"u{gb}")
        for m in range(2):
            nc.scalar.activation(r[:, m, :], pshG[:, m, :], AF.Relu)
            nc.vector._custom_dve(
                _MULSIG, out=u[:, m, :], in0=r[:, m, :],
                in1=psMt[:, 0:HB],
                s0=float(_RS_C0), s1=float(_RS_C1))
        # sq = (u+h)^2 fused (keeps the sumsq path one uninterruptible DVE
        # op); upd itself is only needed by hn much later.
        sq = work.tile([128, 2, HB], BF16, tag=f"sq{gb}", name=f"sq{gb}")
        nc.vector._custom_dve(_ADDSQ, out=sq[:], in0=u[:], in1=hg[:])
        return psMt, (u, hg), sq

    def phase_b(t, gb, psMt, uh, sq, last=False):
        """Norm tail: sumsq, rsqrt via ln/exp, inv broadcast, hn."""
        u, hg = uh
        upd = work.tile([128, 2, HB], BF16, tag=f"upd{gb}", name=f"upd{gb}")
        nc.vector.tensor_tensor(out=upd[:], in0=u[:], in1=hg[:], op=OP.add)
        pss = psMt[0:1, HB:HB + HB]
        nc.tensor.matmul(pss, lhsT=ones128[:], rhs=sq[:, 0, :],
                         start=True, stop=False)
        nc.tensor.matmul(pss, lhsT=ones128[:], rhs=sq[:, 1, :],
                         start=False, stop=True)
        lns = psMt[32:33, HB:HB + HB]
        nc.scalar.activation(lns, pss, AF.Ln, bias=epsap[:])
        inv = work.tile([1, HB], BF16, tag=f"inv{gb}", name=f"inv{gb}")
        nc.scalar.activation(inv[:], lns, AF.Exp, scale=-0.5)
        if last:
            # final step: normalization is folded into the output drain
            # (inv applied per-partition after the transpose), so skip the
            # broadcast matmul and the hn multiply. f32 copy of inv so the
            # transposed column satisfies tensor_scalar's f32-scalar rule.
            invf = work.tile([1, HB], F32, tag=f"invf{gb}", name=f"invf{gb}")
            nc.scalar.activation(invf[:], lns, AF.Exp, scale=-0.5)
            return upd, invf
        psBI = psB.tile([128, HB], F32, tag=f"psb{gb}", name=f"psb{gb}")
        nc.tensor.matmul(psBI[:, :], lhsT=ones1[:], rhs=inv[:],
                         start=True, stop=True)
        hni = hpool.tile([128, 2, HB], BF16, tag=f"h{gb}", name=f"hn{gb}")
        nc.vector.tensor_tensor(
            out=hni[:], in0=upd[:],
            in1=psBI[:, :].unsqueeze(1).broadcast_to([128, 2, HB]),
            op=OP.mult)
        return hni

    # PE p-state warm-up: keep the PE continuously busy while the first
    # gather's DMA is in flight so the word-sum runs at full clock.
    warm = psB.tile([128, HB], F32, tag="psb1", name="warm")
    for w in range(8):
        nc.tensor.matmul(warm[:, :], lhsT=I128[:], rhs=kT[0][:, 0:HB],
                         start=(w == 0), stop=(w == 7))

    def gather_group(g, split=1):
        G, psE = gather_dma(g, split=split)
        gather_wordsum(G, psE, range(L))
        gather_finish(g, psE)

    gather_group(0, split=4)
    nc.sync.dma_start(out=kvt[:], in_=kvtf[:])
    # Groups 1..7 stream under the scan: DMA issued up front / at the block
    # start, the word-sum spread 4 slots per step, finalize at the block
    # end (group g's outputs are first needed at step 8*(g-1)). Group 1's
    # psE parks in the pgk bank so it can coexist with group 2's.
    pend = [(1, 0) + gather_dma(1, petag="pgk")]
    for t in range(S):
        ti = t % 8
        if ti == 0 and t // 8 + 2 < NG:
            pend.append((t // 8 + 2, t) + gather_dma(t // 8 + 2))
        for ent in pend[:]:
            gp, t0, G, psE = ent
            k = t - t0
            gather_wordsum(G, psE, range(4 * k, 4 * k + 4))
            if k == 7:
                gather_finish(gp, psE)
                pend.remove(ent)
        st = [phase_a(t, 0), phase_a(t, 1)]
        h = [phase_b(t, gb, *st[gb], last=(t == S - 1)) for gb in range(2)]

    # ---- output: transpose upd^T and scale rows by inv (rows = bk after
    # the transpose, so the final normalize is a per-partition multiply) ----
    for q in range(4):
        gb, half = q // 2, q % 2
        updf, invf = h[gb]
        pti = psG.tile([128, 1], F32, tag="pgk", name="pti")
        nc.tensor.transpose(pti[:], invf[0:1, 128 * half:128 * half + 128],
                            one1f[:])
        ho = work.tile([128, D], F32, tag="ho")
        for j in range(2):
            pt = psG.tile([128, 128], BF16, tag="pgs", name="ptout")
            nc.tensor.transpose(pt[:], updf[:, j, 128 * half:128 * half + 128],
                                I128[:])
            nc.vector.tensor_scalar(out=ho[:, 128 * j:128 * (j + 1)], in0=pt[:],
                                    scalar1=pti[:, 0:1], scalar2=None,
                                    op0=OP.mult)
        nc.sync.dma_start(out=hout[128 * q:128 * (q + 1), :], in_=ho[:])

    ctx.close()


def _prep_core(pr, mask, keys_c, emb):
    """Host-side marshaling for one core's shard."""
    uniq, inv = np.unique(pr, return_inverse=True)
    assert len(uniq) <= TABLE_ROWS
    table = np.zeros((TABLE_ROWS, D), dtype=ml_dtypes.bfloat16)
    table[: len(uniq)] = emb[uniq].astype(ml_dtypes.bfloat16)
    ranks = inv.reshape(BL, S, L).astype(np.int16)

    # token order per group g: i = (ds*16 + b)*32 + w
    idx_groups = []
    for g in range(NG):
        blk = ranks[:, 8 * g:8 * (g + 1), :]          # [b, ds, w]
        lst = blk.transpose(1, 0, 2).reshape(-1)      # [(ds, b, w)] length 4096
        idx_groups.append(np.tile(lst.reshape(TOKG // 16, 16).T, (8, 1)))
    idx16 = np.concatenate(idx_groups, axis=1).astype(np.int16)  # [128, NG*256]

    keysT = np.ascontiguousarray(
        keys_c.reshape(BK, D).T).astype(ml_dtypes.bfloat16)      # [256, 512]


    # mbig[r=(ds,gb,b), g, c=(bglob,k)] gate-logit offsets:
    #   0    where the gate is live (diagonal batch, unmasked step),
    #   -35  diagonal but step-masked  (exp(-logit) huge -> sigmoid ~ 0),
    #   +35  off-diagonal              (exp(-logit) ~ 0, vanishes in the sum).
    m = mask.astype(bool)                                        # [16, 64]
    mb = np.full((128, NG, BK), 35.0, np.float32)
    r = np.arange(128)
    ds_, gb_, b_ = r // 16, (r % 16) // 8, r % 8
    bglob = 8 * gb_ + b_
    for g in range(NG):
        t_ = 8 * g + ds_                                         # [128]
        ok_row = m[bglob, t_]                                    # [128]
        cols = (np.arange(BK) // K)[None, :] == bglob[:, None]   # [128, BK]
        gm_ = mb[:, g, :]
        gm_[cols] = np.where(np.repeat(ok_row, K), 0.0, -35.0)
        mb[:, g, :] = gm_
    return table, idx16, keysT, mb.astype(ml_dtypes.bfloat16)


def kernel(prgrph, prgrph_mask, keys, embedding_matrix, U, V, W):
    prgrph = np.asarray(prgrph)
    prgrph_mask = np.asarray(prgrph_mask)
    keys = np.asarray(keys, dtype=np.float32)
    emb = np.asarray(embedding_matrix, dtype=np.float32)
    U = np.asarray(U, dtype=np.float32)
    V = np.asarray(V, dtype=np.float32)
    W = np.asarray(W, dtype=np.float32)

    if "nc" not in _CACHED:
        _CACHED["nc"] = _build_program()
    nc = _CACHED["nc"]

    Ub, Vb, Wb = (x.astype(ml_dtypes.bfloat16) for x in (U, V, W))

    in_maps = []
    for c in range(NC):
        sl = slice(BL * c, BL * (c + 1))
        table, idx16, keysT, mb = _prep_core(
            prgrph[sl], prgrph_mask[sl, :, 0], keys[sl], emb)
        kv = keys[sl].reshape(BK, D) @ V                       # [bk, de]
        kvtf = np.ascontiguousarray(
            kv.T.reshape(2, 128, BL, K).transpose(1, 0, 2, 3)).astype(ml_dtypes.bfloat16)
        in_maps.append({
            "table": table, "idx16": idx16, "keysT": keysT,
            "Umat": Ub, "Vmat": Vb, "Wmat": Wb,
            "mbig": mb, "kvtf": kvtf,
        })

    res = run_bass_kernel_spmd(nc, in_maps, core_ids=list(range(NC)))
    out = np.concatenate(
        [res.results[c]["hout"].reshape(BL, K, D) for c in range(NC)], axis=0)
    return out.astype(np.float32)


# revision 76
# speedup vs baseline: 1.0748x; 1.0005x over previous
"""Trainium2 Bass kernel for nn_BasicRecurrentEntityEncoder.

Full-input contract: kernel(**inputs) takes the complete (unsharded) numpy
inputs and returns the full [B, K, D] float32 output. Internally the batch
is sharded over 8 NeuronCores (data parallel, no collectives), the embedding
bag-of-words gather runs through dma_gather against a per-core compacted
bf16 table, and the 64-step entity recurrence runs in a transposed
[D, (b,k)] layout with bf16 matmul operands.

Structure (625.8us baseline -> 425.6us):
  - Gather pipeline fully overlapped with the scan: group 0 up front (dma
    split in two so the word-sum starts early, behind a PE p-state
    warm-up); groups 1-7 streamed under the scan with the word-sum spread
    4 slot-matmuls per step (a monolithic group-1 word-sum used to block
    step 0 on the in-order PE for 5us). Dedicated gather PSUM banks (psG)
    keep it off the scan's PSUM tags.
  - e.keys gate logits precomputed per group ([128,512] matmul) with the
    sentence mask and block-diagonal batch mask folded in host-side as
    +-35 logit offsets (gk0m); per step they are matmul-injected into the
    gate bank, so the gate path is inject + 2 E_t^T h matmuls + one
    ScalarE exp. The exp(-logit) rows are broadcast-SUMMED by a ones8
    matmul (masked entries vanish), and the sigmoid reciprocal is fused
    into the custom DVE op MULSIG: u = relu(psh) * 1/(1+sum exp), with
    relu on ScalarE (HW allows only one PSUM operand per DVE op).
  - kvt = keys V is computed on the host; kvt + eW broadcasts are PSUM
    preloads (h-independent), U^T h accumulates on top.
  - sq = (u+h)^2 fused into the 2-stage custom DVE op ADDSQ so the sumsq
    path is one uninterruptible op (the other group's MULSIG used to land
    between upd and sq); upd = u+h (2x tensor_tensor) is deferred to the
    norm phase since only hn consumes it. relu/MULSIG are split by de-half
    so ScalarE and DVE pipeline. Norm rsqrt = exp(-.5 ln)
    on ScalarE with ln written back to PSUM (cheaper access), inv
    broadcast by a ones1 matmul, h_new = upd * inv on DVE.
  - Two batch groups (b 0-7 | b 8-15) with separate PSUM banks pipeline
    the serial dependency chain across engines.
"""

import sys

if "/opt/trn_rl_repo" not in sys.path:
    sys.path.insert(0, "/opt/trn_rl_repo")

import numpy as np
import ml_dtypes

from concourse import bacc, mybir
import concourse.bass as bass
import concourse.tile as tile
from concourse.bass_utils import run_bass_kernel_spmd
from concourse.masks import make_identity

# Force every ScalarE activation onto the one table set that covers all the
# functions this kernel uses (relu/exp/ln/copy/identity). The default
# chooser greedily picks the first set per function, inserting ~550ns table
# reloads on the critical path. Padding the dict keeps act_func_set_id
# indices aligned with act_info.json while making only the all-covering set
# usable.
_ONE_SET = "natural_log_exp_and_others"


import concourse.hw_specs as _hw_specs
_ORIG_TABLES = _hw_specs.get_activation_tables


def _patched_tables(module_arch):
    real = _ORIG_TABLES(module_arch)
    names = list(real.keys())
    assert _ONE_SET in names, names
    out = {}
    for n in names:
        if n == _ONE_SET:
            out[n] = real[n]
            break
        out[n] = set()
    return out


def _install_table_patch():
    import functools
    cached = functools.cache(_patched_tables)
    bacc.get_activation_tables = cached
    _hw_specs.get_activation_tables = cached


_install_table_patch()

# Custom DVE op #1: out ~= 1/(1 + in0) in ONE VectorE instruction (8 ALU
# stages): u = in0+1; seed y0 = bitcast(~bits(u)); t = u*y0 lands in
# [-4.5, -4] for any positive u; quadratic minimax fixup P(t) ~= 1/t gives
# out = y0*P(t) at ~1e-5 relative error.
import concourse.dve_ops as _dve_ops
from concourse.dve_spec import AluOp as _AluOp, Bin as _Bin, Spec as _Spec
from concourse.dve_spec import C0 as _C0, C1 as _C1, C2 as _C2, One as _One
from concourse.dve_spec import Src0 as _Src0, Src1 as _Src1, relu as _relu
from concourse.dve_spec import lower as _dve_lower
from concourse.dve_spec import _has_src1 as _dve_has_src1
from concourse.dve_uop import DveOpSpec as _DveOpSpec

# MULSIG fuses u = r * sigmoid(logit): in0 = relu(psh) (SBUF bf16, relu on
# ScalarE -- HW allows only ONE PSUM input per DVE op so the gate broadcast
# keeps the PSUM slot), in1 = sum_sent exp(-logit) from the ones8 matmul.
# out = in0 * 1/(1+in1) via the bitwise-NOT seed and a LINEAR fixup
# P(t) = c0 + c1*t on t in [-4.5, -4] (max sigmoid abs err ~1.4e-3).
_RS_C0, _RS_C1 = -0.47250233, -0.05572371


def _mulsig_ref(in0, in1, c0, c1, c2):
    u = (np.asarray(in1, np.float32) + np.float32(1.0)).astype(np.float32)
    y0 = (~u.view(np.int32)).view(np.float32)
    t = (u * y0).astype(np.float32)
    sig = (y0 * (np.float32(c0) + np.float32(c1) * t)).astype(np.float32)
    return np.asarray(in0, np.float32) * sig


def _register_op(name, spec):
    row = 1 + len(_dve_ops.OPS)
    assert row < 0x20
    shas = {}
    for ver in ("v3", "v4"):
        s = _DveOpSpec(name=name, opcode=row, uops=_dve_lower(spec, ver=ver),
                       rd1_en=_dve_has_src1(spec))
        shas[ver] = s.sha(ver)
    op = _dve_ops.DveOp(name, spec, subdim=False, uops_sha=shas)
    _dve_ops.OPS.append(op)
    _dve_ops._SUB_OPCODE_FOR_NAME[name] = row
    _dve_ops.CUSTOM_DVE_SPECS[name] = spec
    return op


def _relusig_ref(in0, in1, c0, c1, c2):
    r = np.maximum(np.nan_to_num(np.asarray(in0, np.float32), nan=0.0), 0.0)
    return _mulsig_ref(r, in1, c0, c1, c2)


def _make_mulsig():
    u = _Bin(_AluOp.ADD, _Src1, _One)
    y0 = _Bin(_AluOp.BITWISE_NOT, u, u)
    t = u * y0
    sig = y0 * (_C0 + _C1 * t)
    return _register_op("MULSIG_ANT",
                        _Spec(body=_Src0 * sig, reference=_mulsig_ref))


def _make_relusig():
    u = _Bin(_AluOp.ADD, _Src1, _One)
    y0 = _Bin(_AluOp.BITWISE_NOT, u, u)
    t = u * y0
    sig = y0 * (_C0 + _C1 * t)
    return _register_op("RELUSIG_ANT",
                        _Spec(body=_relu(_Src0) * sig, reference=_relusig_ref))


def _addsq_ref(in0, in1, c0, c1, c2):
    a = (np.asarray(in0, np.float32) + np.asarray(in1, np.float32)).astype(np.float32)
    return a * a


def _make_addsq():
    a = _Bin(_AluOp.ADD, _Src0, _Src1)
    return _register_op("ADDSQ_ANT",
                        _Spec(body=a * a, reference=_addsq_ref))


_MULSIG = _make_mulsig()
_RELUSIG = _make_relusig()
_ADDSQ = _make_addsq()

F32 = mybir.dt.float32
BF16 = mybir.dt.bfloat16
I16 = mybir.dt.int16
AF = mybir.ActivationFunctionType
OP = mybir.AluOpType

B, S, L, K, D = 128, 64, 32, 32, 256
NC = 8
BL = B // NC              # 16 batch rows per core
BK = BL * K               # 512 = free dim of the state
NG = 8                    # gather groups per core (128 sentences each)
TOKG = 128 * L            # 4096 tokens per group
TABLE_ROWS = 32768        # compacted per-core vocab (unique ids <= 32768)
EPS = 1e-12

_CACHED = {}


def _build_program():
    nc = bacc.Bacc("TRN2", target_bir_lowering=False, debug=False, num_devices=NC)

    table = nc.dram_tensor("table", [TABLE_ROWS, D], BF16, kind="ExternalInput").ap()
    idx16 = nc.dram_tensor("idx16", [128, NG * TOKG // 16], I16, kind="ExternalInput").ap()
    keysT = nc.dram_tensor("keysT", [D, BK], BF16, kind="ExternalInput").ap()
    Umat = nc.dram_tensor("Umat", [D, D], BF16, kind="ExternalInput").ap()
    Vmat = nc.dram_tensor("Vmat", [D, D], BF16, kind="ExternalInput").ap()
    Wmat = nc.dram_tensor("Wmat", [D, D], BF16, kind="ExternalInput").ap()
    mbig = nc.dram_tensor("mbig", [128, NG, BK], BF16, kind="ExternalInput").ap()
    kvtf = nc.dram_tensor("kvtf", [128, 2, BL, K], BF16, kind="ExternalInput").ap()
    hout = nc.dram_tensor("hout", [BK, D], F32, kind="ExternalOutput").ap()

    with tile.TileContext(nc) as tc:
        _emit(nc, tc, table, idx16, keysT, Umat, Vmat, Wmat, mbig, kvtf, hout)
    nc.compile()
    return nc


def _emit(nc, tc, table, idx16, keysT, Umat, Vmat, Wmat, mbig, kvtf, hout):
    from contextlib import ExitStack

    ctx = ExitStack()
    const = ctx.enter_context(tc.tile_pool(name="const", bufs=1))
    persist = ctx.enter_context(tc.tile_pool(name="persist", bufs=1))
    gpool = ctx.enter_context(tc.tile_pool(name="g", bufs=2))
    work = ctx.enter_context(tc.tile_pool(name="work", bufs=4))
    hpool = ctx.enter_context(tc.tile_pool(name="h", bufs=3))
    # PSUM budget (8 banks): psH = pshG double-buffered x2 groups (4 banks,
    # with the step's sumsq row overlaid into bank rows after relusig reads
    # it); psM = psg+gate-bcast [128,256] x2 groups (1 bank); psB = inv
    # bcast [128,256] x2 groups (1 bank); psG = gather scratch (2 banks) so
    # the gather pipeline overlaps the scan instead of serializing on scan
    # PSUM tags.
    psH = ctx.enter_context(tc.tile_pool(name="psH", bufs=1, space="PSUM"))
    psM = ctx.enter_context(tc.tile_pool(name="psM", bufs=1, space="PSUM"))
    psB = ctx.enter_context(tc.tile_pool(name="psB", bufs=1, space="PSUM"))
    psG = ctx.enter_context(tc.tile_pool(name="psG", bufs=1, space="PSUM"))

    # ---- constants into SBUF ----
    sb_idx = const.tile([128, NG * TOKG // 16], I16)
    nc.sync.dma_start(out=sb_idx[:, 0:TOKG // 16], in_=idx16[:, 0:TOKG // 16])
    nc.sync.dma_start(out=sb_idx[:, TOKG // 16:], in_=idx16[:, TOKG // 16:])
    kT = [const.tile([128, BK], BF16, tag=f"kT{j}", name=f"kT{j}") for j in range(2)]
    for j in range(2):
        nc.sync.dma_start(out=kT[j][:], in_=keysT[128 * j:128 * (j + 1), :])
    sbU = [const.tile([128, D], BF16, tag=f"sbU{j}", name=f"sbU{j}") for j in range(2)]
    sbV = [const.tile([128, D], BF16, tag=f"sbV{j}", name=f"sbV{j}") for j in range(2)]
    sbW = [const.tile([128, D], BF16, tag=f"sbW{j}", name=f"sbW{j}") for j in range(2)]
    for j in range(2):
        nc.sync.dma_start(out=sbU[j][:], in_=Umat[128 * j:128 * (j + 1), :])
        nc.sync.dma_start(out=sbV[j][:], in_=Vmat[128 * j:128 * (j + 1), :])
        nc.sync.dma_start(out=sbW[j][:], in_=Wmat[128 * j:128 * (j + 1), :])
    sb_mb = const.tile([128, NG, BK], BF16)
    nc.sync.dma_start(out=sb_mb[:], in_=mbig[:])

    I128 = const.tile([128, 128], BF16)
    make_identity(nc, I128[:])
    ones8 = const.tile([8, 128], BF16)
    nc.vector.memset(ones8[:], 1.0)
    ones128 = const.tile([128, 1], BF16)
    nc.vector.memset(ones128[:], 1.0)
    ones1 = const.tile([1, 128], BF16)
    nc.vector.memset(ones1[:], 1.0)
    epsap = const.tile([1, 1], F32)
    nc.vector.memset(epsap[:], EPS)
    one1f = const.tile([1, 1], F32)
    nc.vector.memset(one1f[:], 1.0)
    # word-sum reducers: Ablk[i][p, m] = 1 iff m == 4*i + p//32.
    Ablk = []
    for i in range(16):
        a = const.tile([128, 64], BF16, tag=f"Ablk{i}", name=f"Ablk{i}")
        nc.vector.memset(a[:], 0.0)
        for q in range(4):
            nc.vector.memset(a[32 * q:32 * (q + 1), 4 * i + q:4 * i + q + 1], 1.0)
        Ablk.append(a)

    # ---- persistent intermediates ----
    ET = [persist.tile([128, NG * 128], BF16, tag=f"ET{j}", name=f"ET{j}") for j in range(2)]
    eWc = persist.tile([128, 2, NG * 128], BF16, tag="eWc", name="eWc")
    # kvt = keys V, host-computed; shaped [128, 2(de half), BL, K]
    kvt = persist.tile([128, 2, BL, K], BF16, tag="kvt", name="kvt")
    gk0m = persist.tile([128, NG, BK], BF16, tag="gk0m", name="gk0m")

    def gather_dma(g, split=1, petag="pgs"):
        G = gpool.tile([128, L, D], BF16, tag="G")
        HT = TOKG // split
        for hseg in range(split):
            nc.gpsimd.dma_gather(
                out_ap=G[:, (L // split) * hseg:(L // split) * (hseg + 1), :],
                in_ap=table[:],
                idxs_ap=sb_idx[:, (TOKG // 16) * g + (HT // 16) * hseg:
                               (TOKG // 16) * g + (HT // 16) * (hseg + 1)],
                num_idxs=HT, num_idxs_reg=HT, elem_size=D, single_packet=False,
            )
        psE = psG.tile([128, D], F32, tag=petag, name="psE")
        return G, psE

    def gather_wordsum(G, psE, cs):
        for c in cs:
            j, i = c // 16, c % 16
            nc.tensor.matmul(psE[64 * j:64 * (j + 1), :], lhsT=Ablk[i][:],
                             rhs=G[:, c, :], start=(i == 0), stop=(i == 15))

    def gather_finish(g, psE):
        enc = work.tile([128, D], BF16, tag="enc")
        nc.scalar.copy(out=enc[:], in_=psE[:])
        for j in range(2):
            pt = psG.tile([128, 128], BF16, tag="pgs", name="pt")
            nc.tensor.transpose(pt[:], enc[:, 128 * j:128 * (j + 1)], I128[:])
            nc.vector.tensor_copy(out=ET[j][:, 128 * g:128 * (g + 1)], in_=pt[:])
        for m in range(2):
            pw = psG.tile([128, 128], F32, tag="pgs", name="pw")
            nc.tensor.matmul(pw[:], lhsT=sbW[0][:, 128 * m:128 * (m + 1)],
                             rhs=ET[0][:, 128 * g:128 * (g + 1)], start=True, stop=False)
            nc.tensor.matmul(pw[:], lhsT=sbW[1][:, 128 * m:128 * (m + 1)],
                             rhs=ET[1][:, 128 * g:128 * (g + 1)], start=False, stop=True)
            nc.vector.tensor_copy(out=eWc[:, m, 128 * g:128 * (g + 1)], in_=pw[:])
        pgk = psG.tile([128, BK], F32, tag="pgk", name="pgk")
        nc.tensor.matmul(pgk[:], lhsT=ET[0][:, 128 * g:128 * (g + 1)], rhs=kT[0][:],
                         start=True, stop=False)
        nc.tensor.matmul(pgk[:], lhsT=ET[1][:, 128 * g:128 * (g + 1)], rhs=kT[1][:],
                         start=False, stop=True)
        nc.vector.tensor_tensor(out=gk0m[:, g, :], in0=pgk[:],
                                in1=sb_mb[:, g, :], op=OP.add)

    # ---- scan: two pipelined batch groups (b 0-7 | b 8-15) ----
    HB = BK // 2  # 256
    h = [hpool.tile([128, 2, HB], BF16, tag=f"h{gb}", name=f"h{gb}")
         for gb in range(2)]
    for gb in range(2):
        nc.vector.memset(h[gb][:], 0.0)

    def phase_a(t, gb, first=False):
        """Injections, h-dependent matmuls, gate, relu*sigmoid, upd, sq."""
        g, ds = t // 8, t % 8
        cg = 128 * g + 16 * ds + 8 * gb
        bks = slice(HB * gb, HB * (gb + 1))
        hg = h[gb]

        # h-independent PSUM preloads
        pshG = psH.tile([128, 2, HB], F32, tag=f"psh{gb}", name=f"psh{gb}")
        nc.tensor.matmul(pshG[:, :, :], lhsT=I128[:],
                         rhs=kvt[:, :, 8 * gb:8 * gb + 8, :], start=True, stop=False)
        for m in range(2):
            ew_bc = eWc[:, m, cg:cg + 8].unsqueeze(2).broadcast_to([128, 8, 32])
            nc.tensor.matmul(pshG[:, m, :], lhsT=I128[:], rhs=ew_bc,
                             start=False, stop=first and m == 1)
        psMt = psM.tile([128, BK], F32, tag=f"psm{gb}", name=f"psm{gb}")
        psg = psMt[0:8, 0:HB]
        off = 16 * ds + 8 * gb
        nc.tensor.matmul(psg, lhsT=I128[:, off:off + 8],
                         rhs=gk0m[:, g, bks], start=True, stop=first)

        # h-dependent matmuls (step 0 has h = 0: E^T h and U^T h vanish)
        if not first:
            nc.tensor.matmul(psg, lhsT=ET[0][:, cg:cg + 8], rhs=hg[:, 0, :],
                             start=False, stop=False)
            nc.tensor.matmul(psg, lhsT=ET[1][:, cg:cg + 8], rhs=hg[:, 1, :],
                             start=False, stop=True)
            for m in range(2):
                nc.tensor.matmul(pshG[:, m, :], lhsT=sbU[0][:, 128 * m:128 * (m + 1)],
                                 rhs=hg[:, 0, :], start=False, stop=False)
                nc.tensor.matmul(pshG[:, m, :], lhsT=sbU[1][:, 128 * m:128 * (m + 1)],
                                 rhs=hg[:, 1, :], start=False, stop=(m == 1))

        # gate: eg = exp(-logit); sigmoid folded into RELUSIG below. The
        # gate broadcast overwrites the psg columns (WAR after eg reads).
        eg = work.tile([8, HB], BF16, tag=f"eg{gb}", name=f"eg{gb}")
        nc.scalar.activation(eg[:], psg, AF.Exp, scale=-1.0)
        nc.tensor.matmul(psMt[:, 0:HB], lhsT=ones8[:], rhs=eg[:],
                         start=True, stop=True)

        # u = relu(psh) * sigmoid, split by de-half so the second relu
        # half overlaps the first MULSIG half.
        r = work.tile([128, 2, HB], BF16, tag=f"r{gb}", name=f"r{gb}")
        u = work.tile([128, 2, HB], BF16, tag=f"u{gb}", name=f"u{gb}")
        for m in range(2):
            nc.scalar.activation(r[:, m, :], pshG[:, m, :], AF.Relu)
            nc.vector._custom_dve(
                _MULSIG, out=u[:, m, :], in0=r[:, m, :],
                in1=psMt[:, 0:HB],
                s0=float(_RS_C0), s1=float(_RS_C1))
        # sq = (u+h)^2 fused (keeps the sumsq path one uninterruptible DVE
        # op); upd itself is only needed by hn much later.
        sq = work.tile([128, 2, HB], BF16, tag=f"sq{gb}", name=f"sq{gb}")
        nc.vector._custom_dve(_ADDSQ, out=sq[:], in0=u[:], in1=hg[:])
        return psMt, (u, hg), sq

    def phase_b(t, gb, psMt, uh, sq, last=False, first=False):
        """Norm tail: sumsq, rsqrt via ln/exp, inv broadcast, hn."""
        u, hg = uh
        if first:
            upd = u
        else:
            upd = work.tile([128, 2, HB], BF16, tag=f"upd{gb}", name=f"upd{gb}")
            nc.vector.tensor_tensor(out=upd[:], in0=u[:], in1=hg[:], op=OP.add)
        pss = psMt[0:1, HB:HB + HB]
        nc.tensor.matmul(pss, lhsT=ones128[:], rhs=sq[:, 0, :],
                         start=True, stop=False)
        nc.tensor.matmul(pss, lhsT=ones128[:], rhs=sq[:, 1, :],
                         start=False, stop=True)
        lns = psMt[32:33, HB:HB + HB]
        nc.scalar.activation(lns, pss, AF.Ln, bias=epsap[:])
        inv = work.tile([1, HB], BF16, tag=f"inv{gb}", name=f"inv{gb}")
        nc.scalar.activation(inv[:], lns, AF.Exp, scale=-0.5)
        if last:
            # final step: normalization is folded into the output drain
            # (inv applied per-partition after the transpose), so skip the
            # broadcast matmul and the hn multiply. f32 copy of inv so the
            # transposed column satisfies tensor_scalar's f32-scalar rule.
            invf = work.tile([1, HB], F32, tag=f"invf{gb}", name=f"invf{gb}")
            nc.scalar.activation(invf[:], lns, AF.Exp, scale=-0.5)
            return upd, invf
        psBI = psB.tile([128, HB], F32, tag=f"psb{gb}", name=f"psb{gb}")
        nc.tensor.matmul(psBI[:, :], lhsT=ones1[:], rhs=inv[:],
                         start=True, stop=True)
        hni = hpool.tile([128, 2, HB], BF16, tag=f"h{gb}", name=f"hn{gb}")
        nc.vector.tensor_tensor(
            out=hni[:], in0=upd[:],
            in1=psBI[:, :].unsqueeze(1).broadcast_to([128, 2, HB]),
            op=OP.mult)
        return hni

    # PE p-state warm-up: keep the PE continuously busy while the first
    # gather's DMA is in flight so the word-sum runs at full clock.
    warm = psB.tile([128, HB], F32, tag="psb1", name="warm")
    for w in range(8):
        nc.tensor.matmul(warm[:, :], lhsT=I128[:], rhs=kT[0][:, 0:HB],
                         start=(w == 0), stop=(w == 7))

    def gather_group(g, split=1):
        G, psE = gather_dma(g, split=split)
        gather_wordsum(G, psE, range(L))
        gather_finish(g, psE)

    gather_group(0, split=4)
    nc.sync.dma_start(out=kvt[:], in_=kvtf[:])
    # Groups 1..7 stream under the scan: DMA issued up front / at the block
    # start, the word-sum spread 4 slots per step, finalize at the block
    # end (group g's outputs are first needed at step 8*(g-1)). Group 1's
    # psE parks in the pgk bank so it can coexist with group 2's.
    pend = [(1, 0) + gather_dma(1, petag="pgk")]
    for t in range(S):
        ti = t % 8
        if ti == 0 and t // 8 + 2 < NG:
            pend.append((t // 8 + 2, t) + gather_dma(t // 8 + 2))
        for ent in pend[:]:
            gp, t0, G, psE = ent
            k = t - t0
            gather_wordsum(G, psE, range(4 * k, 4 * k + 4))
            if k == 7:
                gather_finish(gp, psE)
                pend.remove(ent)
        st = [phase_a(t, 0, first=(t == 0)), phase_a(t, 1, first=(t == 0))]
        h = [phase_b(t, gb, *st[gb], last=(t == S - 1), first=(t == 0))
             for gb in range(2)]

    # ---- output: transpose upd^T and scale rows by inv (rows = bk after
    # the transpose, so the final normalize is a per-partition multiply) ----
    for q in range(4):
        gb, half = q // 2, q % 2
        updf, invf = h[gb]
        pti = psG.tile([128, 1], F32, tag="pgk", name="pti")
        nc.tensor.transpose(pti[:], invf[0:1, 128 * half:128 * half + 128],
                            one1f[:])
        ho = work.tile([128, D], F32, tag="ho")
        for j in range(2):
            pt = psG.tile([128, 128], BF16, tag="pgs", name="ptout")
            nc.tensor.transpose(pt[:], updf[:, j, 128 * half:128 * half + 128],
                                I128[:])
            nc.vector.tensor_scalar(out=ho[:, 128 * j:128 * (j + 1)], in0=pt[:],
                                    scalar1=pti[:, 0:1], scalar2=None,
                                    op0=OP.mult)
        nc.sync.dma_start(out=hout[128 * q:128 * (q + 1), :], in_=ho[:])

    ctx.close()


def _prep_core(pr, mask, keys_c, emb):
    """Host-side marshaling for one core's shard."""
    uniq, inv = np.unique(pr, return_inverse=True)
    assert len(uniq) <= TABLE_ROWS
    table = np.zeros((TABLE_ROWS, D), dtype=ml_dtypes.bfloat16)
    table[: len(uniq)] = emb[uniq].astype(ml_dtypes.bfloat16)
    ranks = inv.reshape(BL, S, L).astype(np.int16)

    # token order per group g: i = (ds*16 + b)*32 + w
    idx_groups = []
    for g in range(NG):
        blk = ranks[:, 8 * g:8 * (g + 1), :]          # [b, ds, w]
        lst = blk.transpose(1, 0, 2).reshape(-1)      # [(ds, b, w)] length 4096
        idx_groups.append(np.tile(lst.reshape(TOKG // 16, 16).T, (8, 1)))
    idx16 = np.concatenate(idx_groups, axis=1).astype(np.int16)  # [128, NG*256]

    keysT = np.ascontiguousarray(
        keys_c.reshape(BK, D).T).astype(ml_dtypes.bfloat16)      # [256, 512]


    # mbig[r=(ds,gb,b), g, c=(bglob,k)] gate-logit offsets:
    #   0    where the gate is live (diagonal batch, unmasked step),
    #   -35  diagonal but step-masked  (exp(-logit) huge -> sigmoid ~ 0),
    #   +35  off-diagonal              (exp(-logit) ~ 0, vanishes in the sum).
    m = mask.astype(bool)                                        # [16, 64]
    mb = np.full((128, NG, BK), 35.0, np.float32)
    r = np.arange(128)
    ds_, gb_, b_ = r // 16, (r % 16) // 8, r % 8
    bglob = 8 * gb_ + b_
    for g in range(NG):
        t_ = 8 * g + ds_                                         # [128]
        ok_row = m[bglob, t_]                                    # [128]
        cols = (np.arange(BK) // K)[None, :] == bglob[:, None]   # [128, BK]
        gm_ = mb[:, g, :]
        gm_[cols] = np.where(np.repeat(ok_row, K), 0.0, -35.0)
        mb[:, g, :] = gm_
    return table, idx16, keysT, mb.astype(ml_dtypes.bfloat16)


def kernel(prgrph, prgrph_mask, keys, embedding_matrix, U, V, W):
    prgrph = np.asarray(prgrph)
    prgrph_mask = np.asarray(prgrph_mask)
    keys = np.asarray(keys, dtype=np.float32)
    emb = np.asarray(embedding_matrix, dtype=np.float32)
    U = np.asarray(U, dtype=np.float32)
    V = np.asarray(V, dtype=np.float32)
    W = np.asarray(W, dtype=np.float32)

    if "nc" not in _CACHED:
        _CACHED["nc"] = _build_program()
    nc = _CACHED["nc"]

    Ub, Vb, Wb = (x.astype(ml_dtypes.bfloat16) for x in (U, V, W))

    in_maps = []
    for c in range(NC):
        sl = slice(BL * c, BL * (c + 1))
        table, idx16, keysT, mb = _prep_core(
            prgrph[sl], prgrph_mask[sl, :, 0], keys[sl], emb)
        kv = keys[sl].reshape(BK, D) @ V                       # [bk, de]
        kvtf = np.ascontiguousarray(
            kv.T.reshape(2, 128, BL, K).transpose(1, 0, 2, 3)).astype(ml_dtypes.bfloat16)
        in_maps.append({
            "table": table, "idx16": idx16, "keysT": keysT,
            "Umat": Ub, "Vmat": Vb, "Wmat": Wb,
            "mbig": mb, "kvtf": kvtf,
        })

    res = run_bass_kernel_spmd(nc, in_maps, core_ids=list(range(NC)))
    out = np.concatenate(
        [res.results[c]["hout"].reshape(BL, K, D) for c in range(NC)], axis=0)
    return out.astype(np.float32)


# revision 78
# speedup vs baseline: 1.0766x; 1.0017x over previous
"""Trainium2 Bass kernel for nn_BasicRecurrentEntityEncoder.

Full-input contract: kernel(**inputs) takes the complete (unsharded) numpy
inputs and returns the full [B, K, D] float32 output. Internally the batch
is sharded over 8 NeuronCores (data parallel, no collectives), the embedding
bag-of-words gather runs through dma_gather against a per-core compacted
bf16 table, and the 64-step entity recurrence runs in a transposed
[D, (b,k)] layout with bf16 matmul operands.

Structure (625.8us baseline -> 425.4us):
  - Gather pipeline fully overlapped with the scan: group 0 up front (dma
    split in two so the word-sum starts early, behind a PE p-state
    warm-up); groups 1-7 streamed under the scan with the word-sum spread
    4 slot-matmuls per step (a monolithic group-1 word-sum used to block
    step 0 on the in-order PE for 5us). Dedicated gather PSUM banks (psG)
    keep it off the scan's PSUM tags.
  - e.keys gate logits precomputed per group ([128,512] matmul) with the
    sentence mask and block-diagonal batch mask folded in host-side as
    +-35 logit offsets (gk0m); per step they are matmul-injected into the
    gate bank, so the gate path is inject + 2 E_t^T h matmuls + one
    ScalarE exp. The exp(-logit) rows are broadcast-SUMMED by a ones8
    matmul (masked entries vanish), and the sigmoid reciprocal is fused
    into the custom DVE op MULSIG: u = relu(psh) * 1/(1+sum exp), with
    relu on ScalarE (HW allows only one PSUM operand per DVE op).
  - kvt = keys V is computed on the host; kvt + eW broadcasts are PSUM
    preloads (h-independent), U^T h accumulates on top.
  - sq = (u+h)^2 fused into the 2-stage custom DVE op ADDSQ so the sumsq
    path is one uninterruptible op (the other group's MULSIG used to land
    between upd and sq); upd = u+h (2x tensor_tensor) is deferred to the
    norm phase since only hn consumes it. relu/MULSIG are split by de-half
    so ScalarE and DVE pipeline. Norm rsqrt = exp(-.5 ln)
    on ScalarE with ln written back to PSUM (cheaper access), inv
    broadcast by a ones1 matmul, h_new = upd * inv on DVE.
  - Two batch groups (b 0-7 | b 8-15) with separate PSUM banks pipeline
    the serial dependency chain across engines.
"""

import sys

if "/opt/trn_rl_repo" not in sys.path:
    sys.path.insert(0, "/opt/trn_rl_repo")

import numpy as np
import ml_dtypes

from concourse import bacc, mybir
import concourse.bass as bass
import concourse.tile as tile
from concourse.bass_utils import run_bass_kernel_spmd
from concourse.masks import make_identity

# Force every ScalarE activation onto the one table set that covers all the
# functions this kernel uses (relu/exp/ln/copy/identity). The default
# chooser greedily picks the first set per function, inserting ~550ns table
# reloads on the critical path. Padding the dict keeps act_func_set_id
# indices aligned with act_info.json while making only the all-covering set
# usable.
_ONE_SET = "natural_log_exp_and_others"


import concourse.hw_specs as _hw_specs
_ORIG_TABLES = _hw_specs.get_activation_tables


def _patched_tables(module_arch):
    real = _ORIG_TABLES(module_arch)
    names = list(real.keys())
    assert _ONE_SET in names, names
    out = {}
    for n in names:
        if n == _ONE_SET:
            out[n] = real[n]
            break
        out[n] = set()
    return out


def _install_table_patch():
    import functools
    cached = functools.cache(_patched_tables)
    bacc.get_activation_tables = cached
    _hw_specs.get_activation_tables = cached


_install_table_patch()

# Custom DVE op #1: out ~= 1/(1 + in0) in ONE VectorE instruction (8 ALU
# stages): u = in0+1; seed y0 = bitcast(~bits(u)); t = u*y0 lands in
# [-4.5, -4] for any positive u; quadratic minimax fixup P(t) ~= 1/t gives
# out = y0*P(t) at ~1e-5 relative error.
import concourse.dve_ops as _dve_ops
from concourse.dve_spec import AluOp as _AluOp, Bin as _Bin, Spec as _Spec
from concourse.dve_spec import C0 as _C0, C1 as _C1, C2 as _C2, One as _One
from concourse.dve_spec import Src0 as _Src0, Src1 as _Src1, relu as _relu
from concourse.dve_spec import lower as _dve_lower
from concourse.dve_spec import _has_src1 as _dve_has_src1
from concourse.dve_uop import DveOpSpec as _DveOpSpec

# MULSIG fuses u = r * sigmoid(logit): in0 = relu(psh) (SBUF bf16, relu on
# ScalarE -- HW allows only ONE PSUM input per DVE op so the gate broadcast
# keeps the PSUM slot), in1 = sum_sent exp(-logit) from the ones8 matmul.
# out = in0 * 1/(1+in1) via the bitwise-NOT seed and a LINEAR fixup
# P(t) = c0 + c1*t on t in [-4.5, -4] (max sigmoid abs err ~1.4e-3).
_RS_C0, _RS_C1 = -0.47250233, -0.05572371


def _mulsig_ref(in0, in1, c0, c1, c2):
    u = (np.asarray(in1, np.float32) + np.float32(1.0)).astype(np.float32)
    y0 = (~u.view(np.int32)).view(np.float32)
    t = (u * y0).astype(np.float32)
    sig = (y0 * (np.float32(c0) + np.float32(c1) * t)).astype(np.float32)
    return np.asarray(in0, np.float32) * sig


def _register_op(name, spec):
    row = 1 + len(_dve_ops.OPS)
    assert row < 0x20
    shas = {}
    for ver in ("v3", "v4"):
        s = _DveOpSpec(name=name, opcode=row, uops=_dve_lower(spec, ver=ver),
                       rd1_en=_dve_has_src1(spec))
        shas[ver] = s.sha(ver)
    op = _dve_ops.DveOp(name, spec, subdim=False, uops_sha=shas)
    _dve_ops.OPS.append(op)
    _dve_ops._SUB_OPCODE_FOR_NAME[name] = row
    _dve_ops.CUSTOM_DVE_SPECS[name] = spec
    return op


def _relusig_ref(in0, in1, c0, c1, c2):
    r = np.maximum(np.nan_to_num(np.asarray(in0, np.float32), nan=0.0), 0.0)
    return _mulsig_ref(r, in1, c0, c1, c2)


def _make_mulsig():
    u = _Bin(_AluOp.ADD, _Src1, _One)
    y0 = _Bin(_AluOp.BITWISE_NOT, u, u)
    t = u * y0
    sig = y0 * (_C0 + _C1 * t)
    return _register_op("MULSIG_ANT",
                        _Spec(body=_Src0 * sig, reference=_mulsig_ref))


def _make_relusig():
    u = _Bin(_AluOp.ADD, _Src1, _One)
    y0 = _Bin(_AluOp.BITWISE_NOT, u, u)
    t = u * y0
    sig = y0 * (_C0 + _C1 * t)
    return _register_op("RELUSIG_ANT",
                        _Spec(body=_relu(_Src0) * sig, reference=_relusig_ref))


def _addsq_ref(in0, in1, c0, c1, c2):
    a = (np.asarray(in0, np.float32) + np.asarray(in1, np.float32)).astype(np.float32)
    return a * a


def _make_addsq():
    a = _Bin(_AluOp.ADD, _Src0, _Src1)
    return _register_op("ADDSQ_ANT",
                        _Spec(body=a * a, reference=_addsq_ref))


_MULSIG = _make_mulsig()
_RELUSIG = _make_relusig()
_ADDSQ = _make_addsq()

F32 = mybir.dt.float32
BF16 = mybir.dt.bfloat16
I16 = mybir.dt.int16
AF = mybir.ActivationFunctionType
OP = mybir.AluOpType

B, S, L, K, D = 128, 64, 32, 32, 256
NC = 8
BL = B // NC              # 16 batch rows per core
BK = BL * K               # 512 = free dim of the state
NG = 8                    # gather groups per core (128 sentences each)
TOKG = 128 * L            # 4096 tokens per group
TABLE_ROWS = 32768        # compacted per-core vocab (unique ids <= 32768)
EPS = 1e-12

_CACHED = {}


def _build_program():
    nc = bacc.Bacc("TRN2", target_bir_lowering=False, debug=False, num_devices=NC)

    table = nc.dram_tensor("table", [TABLE_ROWS, D], BF16, kind="ExternalInput").ap()
    idx16 = nc.dram_tensor("idx16", [128, NG * TOKG // 16], I16, kind="ExternalInput").ap()
    keysT = nc.dram_tensor("keysT", [D, BK], BF16, kind="ExternalInput").ap()
    Umat = nc.dram_tensor("Umat", [D, D], BF16, kind="ExternalInput").ap()
    Vmat = nc.dram_tensor("Vmat", [D, D], BF16, kind="ExternalInput").ap()
    Wmat = nc.dram_tensor("Wmat", [D, D], BF16, kind="ExternalInput").ap()
    mbig = nc.dram_tensor("mbig", [128, NG, BK], BF16, kind="ExternalInput").ap()
    kvtf = nc.dram_tensor("kvtf", [128, 2, BL, K], BF16, kind="ExternalInput").ap()
    hout = nc.dram_tensor("hout", [BK, D], F32, kind="ExternalOutput").ap()

    with tile.TileContext(nc) as tc:
        _emit(nc, tc, table, idx16, keysT, Umat, Vmat, Wmat, mbig, kvtf, hout)
    nc.compile()
    return nc


def _emit(nc, tc, table, idx16, keysT, Umat, Vmat, Wmat, mbig, kvtf, hout):
    from contextlib import ExitStack

    ctx = ExitStack()
    const = ctx.enter_context(tc.tile_pool(name="const", bufs=1))
    persist = ctx.enter_context(tc.tile_pool(name="persist", bufs=1))
    gpool = ctx.enter_context(tc.tile_pool(name="g", bufs=2))
    work = ctx.enter_context(tc.tile_pool(name="work", bufs=4))
    hpool = ctx.enter_context(tc.tile_pool(name="h", bufs=3))
    # PSUM budget (8 banks): psH = pshG double-buffered x2 groups (4 banks,
    # with the step's sumsq row overlaid into bank rows after relusig reads
    # it); psM = psg+gate-bcast [128,256] x2 groups (1 bank); psB = inv
    # bcast [128,256] x2 groups (1 bank); psG = gather scratch (2 banks) so
    # the gather pipeline overlaps the scan instead of serializing on scan
    # PSUM tags.
    psH = ctx.enter_context(tc.tile_pool(name="psH", bufs=1, space="PSUM"))
    psM = ctx.enter_context(tc.tile_pool(name="psM", bufs=1, space="PSUM"))
    psB = ctx.enter_context(tc.tile_pool(name="psB", bufs=1, space="PSUM"))
    psG = ctx.enter_context(tc.tile_pool(name="psG", bufs=1, space="PSUM"))

    # ---- constants into SBUF ----
    sb_idx = const.tile([128, NG * TOKG // 16], I16)
    nc.sync.dma_start(out=sb_idx[:, 0:TOKG // 16], in_=idx16[:, 0:TOKG // 16])
    nc.sync.dma_start(out=sb_idx[:, TOKG // 16:], in_=idx16[:, TOKG // 16:])
    kT = [const.tile([128, BK], BF16, tag=f"kT{j}", name=f"kT{j}") for j in range(2)]
    for j in range(2):
        nc.sync.dma_start(out=kT[j][:], in_=keysT[128 * j:128 * (j + 1), :])
    sbU = [const.tile([128, D], BF16, tag=f"sbU{j}", name=f"sbU{j}") for j in range(2)]
    sbV = [const.tile([128, D], BF16, tag=f"sbV{j}", name=f"sbV{j}") for j in range(2)]
    sbW = [const.tile([128, D], BF16, tag=f"sbW{j}", name=f"sbW{j}") for j in range(2)]
    for j in range(2):
        nc.sync.dma_start(out=sbU[j][:], in_=Umat[128 * j:128 * (j + 1), :])
        nc.sync.dma_start(out=sbV[j][:], in_=Vmat[128 * j:128 * (j + 1), :])
        nc.sync.dma_start(out=sbW[j][:], in_=Wmat[128 * j:128 * (j + 1), :])
    sb_mb = const.tile([128, NG, BK], BF16)
    nc.sync.dma_start(out=sb_mb[:], in_=mbig[:])

    I128 = const.tile([128, 128], BF16)
    make_identity(nc, I128[:])
    ones8 = const.tile([8, 128], BF16)
    nc.vector.memset(ones8[:], 1.0)
    ones128 = const.tile([128, 1], BF16)
    nc.vector.memset(ones128[:], 1.0)
    ones1 = const.tile([1, 128], BF16)
    nc.vector.memset(ones1[:], 1.0)
    epsap = const.tile([1, 1], F32)
    nc.vector.memset(epsap[:], EPS)
    one1f = const.tile([1, 1], F32)
    nc.vector.memset(one1f[:], 1.0)
    # word-sum reducers: Ablk[i][p, m] = 1 iff m == 4*i + p//32.
    Ablk = []
    for i in range(16):
        a = const.tile([128, 64], BF16, tag=f"Ablk{i}", name=f"Ablk{i}")
        nc.vector.memset(a[:], 0.0)
        for q in range(4):
            nc.vector.memset(a[32 * q:32 * (q + 1), 4 * i + q:4 * i + q + 1], 1.0)
        Ablk.append(a)

    # ---- persistent intermediates ----
    ET = [persist.tile([128, NG * 128], BF16, tag=f"ET{j}", name=f"ET{j}") for j in range(2)]
    eWc = persist.tile([128, 2, NG * 128], BF16, tag="eWc", name="eWc")
    # kvt = keys V, host-computed; shaped [128, 2(de half), BL, K]
    kvt = persist.tile([128, 2, BL, K], BF16, tag="kvt", name="kvt")
    gk0m = persist.tile([128, NG, BK], BF16, tag="gk0m", name="gk0m")

    def gather_dma(g, split=1, petag="pgs"):
        G = gpool.tile([128, L, D], BF16, tag="G")
        HT = TOKG // split
        for hseg in range(split):
            nc.gpsimd.dma_gather(
                out_ap=G[:, (L // split) * hseg:(L // split) * (hseg + 1), :],
                in_ap=table[:],
                idxs_ap=sb_idx[:, (TOKG // 16) * g + (HT // 16) * hseg:
                               (TOKG // 16) * g + (HT // 16) * (hseg + 1)],
                num_idxs=HT, num_idxs_reg=HT, elem_size=D, single_packet=False,
            )
        psE = psG.tile([128, D], F32, tag=petag, name="psE")
        return G, psE

    def gather_wordsum(G, psE, cs):
        for c in cs:
            j, i = c // 16, c % 16
            nc.tensor.matmul(psE[64 * j:64 * (j + 1), :], lhsT=Ablk[i][:],
                             rhs=G[:, c, :], start=(i == 0), stop=(i == 15))

    def gather_finish(g, psE):
        enc = work.tile([128, D], BF16, tag="enc")
        nc.scalar.copy(out=enc[:], in_=psE[:])
        for j in range(2):
            pt = psG.tile([128, 128], BF16, tag="pgs", name="pt")
            nc.tensor.transpose(pt[:], enc[:, 128 * j:128 * (j + 1)], I128[:])
            nc.vector.tensor_copy(out=ET[j][:, 128 * g:128 * (g + 1)], in_=pt[:])
        for m in range(2):
            pw = psG.tile([128, 128], F32, tag="pgs", name="pw")
            nc.tensor.matmul(pw[:], lhsT=sbW[0][:, 128 * m:128 * (m + 1)],
                             rhs=ET[0][:, 128 * g:128 * (g + 1)], start=True, stop=False)
            nc.tensor.matmul(pw[:], lhsT=sbW[1][:, 128 * m:128 * (m + 1)],
                             rhs=ET[1][:, 128 * g:128 * (g + 1)], start=False, stop=True)
            nc.vector.tensor_copy(out=eWc[:, m, 128 * g:128 * (g + 1)], in_=pw[:])
        pgk = psG.tile([128, BK], F32, tag="pgk", name="pgk")
        nc.tensor.matmul(pgk[:], lhsT=ET[0][:, 128 * g:128 * (g + 1)], rhs=kT[0][:],
                         start=True, stop=False)
        nc.tensor.matmul(pgk[:], lhsT=ET[1][:, 128 * g:128 * (g + 1)], rhs=kT[1][:],
                         start=False, stop=True)
        nc.vector.tensor_tensor(out=gk0m[:, g, :], in0=pgk[:],
                                in1=sb_mb[:, g, :], op=OP.add)

    # ---- scan: two pipelined batch groups (b 0-7 | b 8-15) ----
    HB = BK // 2  # 256
    h = [hpool.tile([128, 2, HB], BF16, tag=f"h{gb}", name=f"h{gb}")
         for gb in range(2)]
    for gb in range(2):
        nc.vector.memset(h[gb][:], 0.0)

    def phase_a(t, gb, first=False):
        """Injections, h-dependent matmuls, gate, relu*sigmoid, upd, sq."""
        g, ds = t // 8, t % 8
        cg = 128 * g + 16 * ds + 8 * gb
        bks = slice(HB * gb, HB * (gb + 1))
        hg = h[gb]

        # h-independent PSUM preloads
        pshG = psH.tile([128, 2, HB], F32, tag=f"psh{gb}", name=f"psh{gb}")
        nc.tensor.matmul(pshG[:, :, :], lhsT=I128[:],
                         rhs=kvt[:, :, 8 * gb:8 * gb + 8, :], start=True, stop=False)
        for m in range(2):
            ew_bc = eWc[:, m, cg:cg + 8].unsqueeze(2).broadcast_to([128, 8, 32])
            nc.tensor.matmul(pshG[:, m, :], lhsT=I128[:], rhs=ew_bc,
                             start=False, stop=first and m == 1)
        psMt = psM.tile([128, BK], F32, tag=f"psm{gb}", name=f"psm{gb}")
        psg = psMt[0:8, 0:HB]
        off = 16 * ds + 8 * gb
        nc.tensor.matmul(psg, lhsT=I128[:, off:off + 8],
                         rhs=gk0m[:, g, bks], start=True, stop=first)

        # h-dependent matmuls (step 0 has h = 0: E^T h and U^T h vanish)
        if not first:
            nc.tensor.matmul(psg, lhsT=ET[0][:, cg:cg + 8], rhs=hg[:, 0, :],
                             start=False, stop=False)
            nc.tensor.matmul(psg, lhsT=ET[1][:, cg:cg + 8], rhs=hg[:, 1, :],
                             start=False, stop=True)
            for m in range(2):
                nc.tensor.matmul(pshG[:, m, :], lhsT=sbU[0][:, 128 * m:128 * (m + 1)],
                                 rhs=hg[:, 0, :], start=False, stop=False)
                nc.tensor.matmul(pshG[:, m, :], lhsT=sbU[1][:, 128 * m:128 * (m + 1)],
                                 rhs=hg[:, 1, :], start=False, stop=(m == 1))

        # gate: eg = exp(-logit); sigmoid folded into RELUSIG below. The
        # gate broadcast overwrites the psg columns (WAR after eg reads).
        eg = work.tile([8, HB], BF16, tag=f"eg{gb}", name=f"eg{gb}")
        nc.scalar.activation(eg[:], psg, AF.Exp, scale=-1.0)
        nc.tensor.matmul(psMt[:, 0:HB], lhsT=ones8[:], rhs=eg[:],
                         start=True, stop=True)

        # u = relu(psh) * sigmoid, split by de-half so the second relu
        # half overlaps the first MULSIG half.
        r = work.tile([128, 2, HB], BF16, tag=f"r{gb}", name=f"r{gb}")
        u = work.tile([128, 2, HB], BF16, tag=f"u{gb}", name=f"u{gb}")
        for m in range(2):
            nc.scalar.activation(r[:, m, :], pshG[:, m, :], AF.Relu)
            nc.vector._custom_dve(
                _MULSIG, out=u[:, m, :], in0=r[:, m, :],
                in1=psMt[:, 0:HB],
                s0=float(_RS_C0), s1=float(_RS_C1))
        # sq = (u+h)^2 fused (keeps the sumsq path one uninterruptible DVE
        # op); upd itself is only needed by hn much later.
        sq = work.tile([128, 2, HB], BF16, tag=f"sq{gb}", name=f"sq{gb}")
        nc.vector._custom_dve(_ADDSQ, out=sq[:], in0=u[:], in1=hg[:])
        return psMt, (u, hg), sq

    def phase_b(t, gb, psMt, uh, sq, last=False, first=False):
        """Norm tail: sumsq, rsqrt via ln/exp, inv broadcast, hn."""
        u, hg = uh
        if first:
            upd = u
        else:
            upd = work.tile([128, 2, HB], BF16, tag=f"upd{gb}", name=f"upd{gb}")
            nc.vector.tensor_tensor(out=upd[:], in0=u[:], in1=hg[:], op=OP.add)
        pss = psMt[0:1, HB:HB + HB]
        nc.tensor.matmul(pss, lhsT=ones128[:], rhs=sq[:, 0, :],
                         start=True, stop=False)
        nc.tensor.matmul(pss, lhsT=ones128[:], rhs=sq[:, 1, :],
                         start=False, stop=True)
        lns = psMt[32:33, HB:HB + HB]
        nc.scalar.activation(lns, pss, AF.Ln, bias=epsap[:])
        inv = work.tile([1, HB], BF16, tag=f"inv{gb}", name=f"inv{gb}")
        nc.scalar.activation(inv[:], lns, AF.Exp, scale=-0.5)
        if last:
            # final step: normalization is folded into the output drain
            # (inv applied per-partition after the transpose), so skip the
            # broadcast matmul and the hn multiply. f32 copy of inv so the
            # transposed column satisfies tensor_scalar's f32-scalar rule.
            invf = work.tile([1, HB], F32, tag=f"invf{gb}", name=f"invf{gb}")
            nc.scalar.activation(invf[:], lns, AF.Exp, scale=-0.5)
            return upd, invf
        psBI = psB.tile([128, HB], F32, tag=f"psb{gb}", name=f"psb{gb}")
        nc.tensor.matmul(psBI[:, :], lhsT=ones1[:], rhs=inv[:],
                         start=True, stop=True)
        hni = hpool.tile([128, 2, HB], BF16, tag=f"h{gb}", name=f"hn{gb}")
        nc.vector.tensor_tensor(
            out=hni[:], in0=upd[:],
            in1=psBI[:, :].unsqueeze(1).broadcast_to([128, 2, HB]),
            op=OP.mult)
        return hni

    # PE p-state warm-up: keep the PE continuously busy while the first
    # gather's DMA is in flight so the word-sum runs at full clock.
    warm = psB.tile([128, HB], F32, tag="psb1", name="warm")
    for w in range(8):
        nc.tensor.matmul(warm[:, :], lhsT=I128[:], rhs=kT[0][:, 0:HB],
                         start=(w == 0), stop=(w == 7))

    def gather_group(g, split=1):
        G, psE = gather_dma(g, split=split)
        gather_wordsum(G, psE, range(L))
        gather_finish(g, psE)

    gather_group(0, split=4)
    nc.sync.dma_start(out=kvt[:], in_=kvtf[:])
    # Groups 1..7 stream under the scan: DMA issued up front / at the block
    # start, the word-sum spread 4 slots per step, finalize at the block
    # end (group g's outputs are first needed at step 8*(g-1)). Group 1's
    # psE parks in the pgk bank so it can coexist with group 2's.
    pend = [(1, 0) + gather_dma(1, petag="pgk")]
    for t in range(S):
        ti = t % 8
        if ti == 0 and t // 8 + 2 < NG:
            pend.append((t // 8 + 2, t) + gather_dma(t // 8 + 2))
        for ent in pend[:]:
            gp, t0, G, psE = ent
            k = t - t0
            gather_wordsum(G, psE, range(4 * k, 4 * k + 4))
            if k == 7:
                gather_finish(gp, psE)
                pend.remove(ent)
        st = [phase_a(t, 0, first=(t == 0)), phase_a(t, 1, first=(t == 0))]
        h = [phase_b(t, gb, *st[gb], last=(t == S - 1), first=(t == 0))
             for gb in range(2)]

    # ---- output: transpose upd^T and scale rows by inv (rows = bk after
    # the transpose, so the final normalize is a per-partition multiply) ----
    for q in range(4):
        gb, half = q // 2, q % 2
        updf, invf = h[gb]
        pti = psG.tile([128, 1], F32, tag="pgk", name="pti")
        nc.tensor.transpose(pti[:], invf[0:1, 128 * half:128 * half + 128],
                            one1f[:])
        ho = work.tile([128, D], F32, tag="ho")
        for j in range(2):
            # alternate PSUM tags (psb banks are free after the last step)
            # so the two transposes don't serialize on one buffer.
            if j == 0:
                pt = psG.tile([128, 128], BF16, tag="pgs", name="ptout")
            else:
                pt = psB.tile([128, 128], BF16, tag=f"psb{gb}", name="ptout2")
            nc.tensor.transpose(pt[:], updf[:, j, 128 * half:128 * half + 128],
                                I128[:])
            nc.vector.tensor_scalar(out=ho[:, 128 * j:128 * (j + 1)], in0=pt[:],
                                    scalar1=pti[:, 0:1], scalar2=None,
                                    op0=OP.mult)
        nc.sync.dma_start(out=hout[128 * q:128 * (q + 1), :], in_=ho[:])

    ctx.close()


def _prep_core(pr, mask, keys_c, emb):
    """Host-side marshaling for one core's shard."""
    uniq, inv = np.unique(pr, return_inverse=True)
    assert len(uniq) <= TABLE_ROWS
    table = np.zeros((TABLE_ROWS, D), dtype=ml_dtypes.bfloat16)
    table[: len(uniq)] = emb[uniq].astype(ml_dtypes.bfloat16)
    ranks = inv.reshape(BL, S, L).astype(np.int16)

    # token order per group g: i = (ds*16 + b)*32 + w
    idx_groups = []
    for g in range(NG):
        blk = ranks[:, 8 * g:8 * (g + 1), :]          # [b, ds, w]
        lst = blk.transpose(1, 0, 2).reshape(-1)      # [(ds, b, w)] length 4096
        idx_groups.append(np.tile(lst.reshape(TOKG // 16, 16).T, (8, 1)))
    idx16 = np.concatenate(idx_groups, axis=1).astype(np.int16)  # [128, NG*256]

    keysT = np.ascontiguousarray(
        keys_c.reshape(BK, D).T).astype(ml_dtypes.bfloat16)      # [256, 512]


    # mbig[r=(ds,gb,b), g, c=(bglob,k)] gate-logit offsets:
    #   0    where the gate is live (diagonal batch, unmasked step),
    #   -35  diagonal but step-masked  (exp(-logit) huge -> sigmoid ~ 0),
    #   +35  off-diagonal              (exp(-logit) ~ 0, vanishes in the sum).
    m = mask.astype(bool)                                        # [16, 64]
    mb = np.full((128, NG, BK), 35.0, np.float32)
    r = np.arange(128)
    ds_, gb_, b_ = r // 16, (r % 16) // 8, r % 8
    bglob = 8 * gb_ + b_
    for g in range(NG):
        t_ = 8 * g + ds_                                         # [128]
        ok_row = m[bglob, t_]                                    # [128]
        cols = (np.arange(BK) // K)[None, :] == bglob[:, None]   # [128, BK]
        gm_ = mb[:, g, :]
        gm_[cols] = np.where(np.repeat(ok_row, K), 0.0, -35.0)
        mb[:, g, :] = gm_
    return table, idx16, keysT, mb.astype(ml_dtypes.bfloat16)


def kernel(prgrph, prgrph_mask, keys, embedding_matrix, U, V, W):
    prgrph = np.asarray(prgrph)
    prgrph_mask = np.asarray(prgrph_mask)
    keys = np.asarray(keys, dtype=np.float32)
    emb = np.asarray(embedding_matrix, dtype=np.float32)
    U = np.asarray(U, dtype=np.float32)
    V = np.asarray(V, dtype=np.float32)
    W = np.asarray(W, dtype=np.float32)

    if "nc" not in _CACHED:
        _CACHED["nc"] = _build_program()
    nc = _CACHED["nc"]

    Ub, Vb, Wb = (x.astype(ml_dtypes.bfloat16) for x in (U, V, W))

    in_maps = []
    for c in range(NC):
        sl = slice(BL * c, BL * (c + 1))
        table, idx16, keysT, mb = _prep_core(
            prgrph[sl], prgrph_mask[sl, :, 0], keys[sl], emb)
        kv = keys[sl].reshape(BK, D) @ V                       # [bk, de]
        kvtf = np.ascontiguousarray(
            kv.T.reshape(2, 128, BL, K).transpose(1, 0, 2, 3)).astype(ml_dtypes.bfloat16)
        in_maps.append({
            "table": table, "idx16": idx16, "keysT": keysT,
            "Umat": Ub, "Vmat": Vb, "Wmat": Wb,
            "mbig": mb, "kvtf": kvtf,
        })

    res = run_bass_kernel_spmd(nc, in_maps, core_ids=list(range(NC)))
    out = np.concatenate(
        [res.results[c]["hout"].reshape(BL, K, D) for c in range(NC)], axis=0)
    return out.astype(np.float32)


# revision 87
# speedup vs baseline: 1.0775x; 1.0008x over previous
"""Trainium2 Bass kernel for nn_BasicRecurrentEntityEncoder.

Full-input contract: kernel(**inputs) takes the complete (unsharded) numpy
inputs and returns the full [B, K, D] float32 output. Internally the batch
is sharded over 8 NeuronCores (data parallel, no collectives), the embedding
bag-of-words gather runs through dma_gather against a per-core compacted
bf16 table, and the 64-step entity recurrence runs in a transposed
[D, (b,k)] layout with bf16 matmul operands.

Structure (625.8us baseline -> 424.7us):
  - Gather pipeline fully overlapped with the scan: group 0 up front (dma
    split in two so the word-sum starts early, behind a PE p-state
    warm-up); groups 1-7 streamed under the scan with the word-sum spread
    4 slot-matmuls per step (a monolithic group-1 word-sum used to block
    step 0 on the in-order PE for 5us). Dedicated gather PSUM banks (psG)
    keep it off the scan's PSUM tags.
  - e.keys gate logits precomputed per group ([128,512] matmul) with the
    sentence mask and block-diagonal batch mask folded in host-side as
    +-35 logit offsets (gk0m); per step they are matmul-injected into the
    gate bank, so the gate path is inject + 2 E_t^T h matmuls + one
    ScalarE exp. The exp(-logit) rows are broadcast-SUMMED by a ones8
    matmul (masked entries vanish), and the sigmoid reciprocal is fused
    into the custom DVE op MULSIG: u = relu(psh) * 1/(1+sum exp), with
    relu on ScalarE (HW allows only one PSUM operand per DVE op).
  - kvt = keys V is computed on the host; kvt + eW broadcasts are PSUM
    preloads (h-independent), U^T h accumulates on top.
  - sq = (u+h)^2 fused into the 2-stage custom DVE op ADDSQ so the sumsq
    path is one uninterruptible op (the other group's MULSIG used to land
    between upd and sq); upd = u+h (2x tensor_tensor) is deferred to the
    norm phase since only hn consumes it. relu/MULSIG are split by de-half
    so ScalarE and DVE pipeline. Norm rsqrt = exp(-.5 ln)
    on ScalarE with ln written back to PSUM (cheaper access), inv
    broadcast by a ones1 matmul, h_new = upd * inv on DVE.
  - Two batch groups (b 0-7 | b 8-15) with separate PSUM banks pipeline
    the serial dependency chain across engines.
"""

import sys

if "/opt/trn_rl_repo" not in sys.path:
    sys.path.insert(0, "/opt/trn_rl_repo")

import numpy as np
import ml_dtypes

from concourse import bacc, mybir
import concourse.bass as bass
import concourse.tile as tile
from concourse.bass_utils import run_bass_kernel_spmd
from concourse.masks import make_identity

# Force every ScalarE activation onto the one table set that covers all the
# functions this kernel uses (relu/exp/ln/copy/identity). The default
# chooser greedily picks the first set per function, inserting ~550ns table
# reloads on the critical path. Padding the dict keeps act_func_set_id
# indices aligned with act_info.json while making only the all-covering set
# usable.
_ONE_SET = "natural_log_exp_and_others"


import concourse.hw_specs as _hw_specs
_ORIG_TABLES = _hw_specs.get_activation_tables


def _patched_tables(module_arch):
    real = _ORIG_TABLES(module_arch)
    names = list(real.keys())
    assert _ONE_SET in names, names
    out = {}
    for n in names:
        if n == _ONE_SET:
            out[n] = real[n]
            break
        out[n] = set()
    return out


def _install_table_patch():
    import functools
    cached = functools.cache(_patched_tables)
    bacc.get_activation_tables = cached
    _hw_specs.get_activation_tables = cached


_install_table_patch()

# Custom DVE op #1: out ~= 1/(1 + in0) in ONE VectorE instruction (8 ALU
# stages): u = in0+1; seed y0 = bitcast(~bits(u)); t = u*y0 lands in
# [-4.5, -4] for any positive u; quadratic minimax fixup P(t) ~= 1/t gives
# out = y0*P(t) at ~1e-5 relative error.
import concourse.dve_ops as _dve_ops
from concourse.dve_spec import AluOp as _AluOp, Bin as _Bin, Spec as _Spec
from concourse.dve_spec import C0 as _C0, C1 as _C1, C2 as _C2, One as _One
from concourse.dve_spec import Src0 as _Src0, Src1 as _Src1, relu as _relu
from concourse.dve_spec import lower as _dve_lower
from concourse.dve_spec import _has_src1 as _dve_has_src1
from concourse.dve_uop import DveOpSpec as _DveOpSpec

# MULSIG fuses u = r * sigmoid(logit): in0 = relu(psh) (SBUF bf16, relu on
# ScalarE -- HW allows only ONE PSUM input per DVE op so the gate broadcast
# keeps the PSUM slot), in1 = sum_sent exp(-logit) from the ones8 matmul.
# out = in0 * 1/(1+in1) via the bitwise-NOT seed and a LINEAR fixup
# P(t) = c0 + c1*t on t in [-4.5, -4] (max sigmoid abs err ~1.4e-3).
_RS_C0, _RS_C1 = -0.47250233, -0.05572371


def _mulsig_ref(in0, in1, c0, c1, c2):
    u = (np.asarray(in1, np.float32) + np.float32(1.0)).astype(np.float32)
    y0 = (~u.view(np.int32)).view(np.float32)
    t = (u * y0).astype(np.float32)
    sig = (y0 * (np.float32(c0) + np.float32(c1) * t)).astype(np.float32)
    return np.asarray(in0, np.float32) * sig


def _register_op(name, spec):
    row = 1 + len(_dve_ops.OPS)
    assert row < 0x20
    shas = {}
    for ver in ("v3", "v4"):
        s = _DveOpSpec(name=name, opcode=row, uops=_dve_lower(spec, ver=ver),
                       rd1_en=_dve_has_src1(spec))
        shas[ver] = s.sha(ver)
    op = _dve_ops.DveOp(name, spec, subdim=False, uops_sha=shas)
    _dve_ops.OPS.append(op)
    _dve_ops._SUB_OPCODE_FOR_NAME[name] = row
    _dve_ops.CUSTOM_DVE_SPECS[name] = spec
    return op


def _relusig_ref(in0, in1, c0, c1, c2):
    r = np.maximum(np.nan_to_num(np.asarray(in0, np.float32), nan=0.0), 0.0)
    return _mulsig_ref(r, in1, c0, c1, c2)


def _make_mulsig():
    u = _Bin(_AluOp.ADD, _Src1, _One)
    y0 = _Bin(_AluOp.BITWISE_NOT, u, u)
    t = u * y0
    sig = y0 * (_C0 + _C1 * t)
    return _register_op("MULSIG_ANT",
                        _Spec(body=_Src0 * sig, reference=_mulsig_ref))


def _make_relusig():
    u = _Bin(_AluOp.ADD, _Src1, _One)
    y0 = _Bin(_AluOp.BITWISE_NOT, u, u)
    t = u * y0
    sig = y0 * (_C0 + _C1 * t)
    return _register_op("RELUSIG_ANT",
                        _Spec(body=_relu(_Src0) * sig, reference=_relusig_ref))


def _addsq_ref(in0, in1, c0, c1, c2):
    a = (np.asarray(in0, np.float32) + np.asarray(in1, np.float32)).astype(np.float32)
    return a * a


def _make_addsq():
    a = _Bin(_AluOp.ADD, _Src0, _Src1)
    return _register_op("ADDSQ_ANT",
                        _Spec(body=a * a, reference=_addsq_ref))


_MULSIG = _make_mulsig()
_RELUSIG = _make_relusig()
_ADDSQ = _make_addsq()

F32 = mybir.dt.float32
BF16 = mybir.dt.bfloat16
I16 = mybir.dt.int16
AF = mybir.ActivationFunctionType
OP = mybir.AluOpType

B, S, L, K, D = 128, 64, 32, 32, 256
NC = 8
BL = B // NC              # 16 batch rows per core
BK = BL * K               # 512 = free dim of the state
NG = 8                    # gather groups per core (128 sentences each)
TOKG = 128 * L            # 4096 tokens per group
TABLE_ROWS = 32768        # compacted per-core vocab (unique ids <= 32768)
EPS = 1e-12

_CACHED = {}


def _build_program():
    nc = bacc.Bacc("TRN2", target_bir_lowering=False, debug=False, num_devices=NC)

    table = nc.dram_tensor("table", [TABLE_ROWS, D], BF16, kind="ExternalInput").ap()
    idx16 = nc.dram_tensor("idx16", [128, NG * TOKG // 16], I16, kind="ExternalInput").ap()
    keysT = nc.dram_tensor("keysT", [D, BK], BF16, kind="ExternalInput").ap()
    Umat = nc.dram_tensor("Umat", [D, D], BF16, kind="ExternalInput").ap()
    Vmat = nc.dram_tensor("Vmat", [D, D], BF16, kind="ExternalInput").ap()
    Wmat = nc.dram_tensor("Wmat", [D, D], BF16, kind="ExternalInput").ap()
    mbig = nc.dram_tensor("mbig", [128, NG, BK], BF16, kind="ExternalInput").ap()
    kvtf = nc.dram_tensor("kvtf", [128, 2, BL, K], BF16, kind="ExternalInput").ap()
    hout = nc.dram_tensor("hout", [BK, D], F32, kind="ExternalOutput").ap()

    with tile.TileContext(nc) as tc:
        _emit(nc, tc, table, idx16, keysT, Umat, Vmat, Wmat, mbig, kvtf, hout)
    nc.compile()
    return nc


def _emit(nc, tc, table, idx16, keysT, Umat, Vmat, Wmat, mbig, kvtf, hout):
    from contextlib import ExitStack

    ctx = ExitStack()
    const = ctx.enter_context(tc.tile_pool(name="const", bufs=1))
    persist = ctx.enter_context(tc.tile_pool(name="persist", bufs=1))
    gpool = ctx.enter_context(tc.tile_pool(name="g", bufs=2))
    work = ctx.enter_context(tc.tile_pool(name="work", bufs=4))
    hpool = ctx.enter_context(tc.tile_pool(name="h", bufs=3))
    # PSUM budget (8 banks): psH = pshG double-buffered x2 groups (4 banks,
    # with the step's sumsq row overlaid into bank rows after relusig reads
    # it); psM = psg+gate-bcast [128,256] x2 groups (1 bank); psB = inv
    # bcast [128,256] x2 groups (1 bank); psG = gather scratch (2 banks) so
    # the gather pipeline overlaps the scan instead of serializing on scan
    # PSUM tags.
    psH = ctx.enter_context(tc.tile_pool(name="psH", bufs=1, space="PSUM"))
    psM = ctx.enter_context(tc.tile_pool(name="psM", bufs=1, space="PSUM"))
    psB = ctx.enter_context(tc.tile_pool(name="psB", bufs=1, space="PSUM"))
    psG = ctx.enter_context(tc.tile_pool(name="psG", bufs=1, space="PSUM"))

    # ---- constants into SBUF ----
    sb_idx = const.tile([128, NG * TOKG // 16], I16)
    nc.sync.dma_start(out=sb_idx[:, 0:TOKG // 16], in_=idx16[:, 0:TOKG // 16])
    nc.sync.dma_start(out=sb_idx[:, TOKG // 16:], in_=idx16[:, TOKG // 16:])
    kT = [const.tile([128, BK], BF16, tag=f"kT{j}", name=f"kT{j}") for j in range(2)]
    for j in range(2):
        nc.sync.dma_start(out=kT[j][:], in_=keysT[128 * j:128 * (j + 1), :])
    sbU = [const.tile([128, D], BF16, tag=f"sbU{j}", name=f"sbU{j}") for j in range(2)]
    sbV = [const.tile([128, D], BF16, tag=f"sbV{j}", name=f"sbV{j}") for j in range(2)]
    sbW = [const.tile([128, D], BF16, tag=f"sbW{j}", name=f"sbW{j}") for j in range(2)]
    for j in range(2):
        nc.sync.dma_start(out=sbU[j][:], in_=Umat[128 * j:128 * (j + 1), :])
        nc.sync.dma_start(out=sbV[j][:], in_=Vmat[128 * j:128 * (j + 1), :])
        nc.sync.dma_start(out=sbW[j][:], in_=Wmat[128 * j:128 * (j + 1), :])
    sb_mb = const.tile([128, NG, BK], BF16)
    nc.sync.dma_start(out=sb_mb[:], in_=mbig[:])

    I128 = const.tile([128, 128], BF16)
    make_identity(nc, I128[:])
    ones8 = const.tile([8, 128], BF16)
    nc.vector.memset(ones8[:], 1.0)
    ones128 = const.tile([128, 1], BF16)
    nc.vector.memset(ones128[:], 1.0)
    ones1 = const.tile([1, 128], BF16)
    nc.vector.memset(ones1[:], 1.0)
    epsap = const.tile([1, 1], F32)
    nc.vector.memset(epsap[:], EPS)
    one1f = const.tile([1, 1], F32)
    nc.vector.memset(one1f[:], 1.0)
    # word-sum reducers: Ablk[i][p, m] = 1 iff m == 4*i + p//32.
    Ablk = []
    for i in range(16):
        a = const.tile([128, 64], BF16, tag=f"Ablk{i}", name=f"Ablk{i}")
        nc.vector.memset(a[:], 0.0)
        for q in range(4):
            nc.vector.memset(a[32 * q:32 * (q + 1), 4 * i + q:4 * i + q + 1], 1.0)
        Ablk.append(a)

    # ---- persistent intermediates ----
    ET = [persist.tile([128, NG * 128], BF16, tag=f"ET{j}", name=f"ET{j}") for j in range(2)]
    eWc = persist.tile([128, 2, NG * 128], BF16, tag="eWc", name="eWc")
    # kvt = keys V, host-computed; shaped [128, 2(de half), BL, K]
    kvt = persist.tile([128, 2, BL, K], BF16, tag="kvt", name="kvt")
    gk0m = persist.tile([128, NG, BK], BF16, tag="gk0m", name="gk0m")

    def gather_dma(g, split=1, petag="pgs"):
        G = gpool.tile([128, L, D], BF16, tag="G")
        HT = TOKG // split
        for hseg in range(split):
            nc.gpsimd.dma_gather(
                out_ap=G[:, (L // split) * hseg:(L // split) * (hseg + 1), :],
                in_ap=table[:],
                idxs_ap=sb_idx[:, (TOKG // 16) * g + (HT // 16) * hseg:
                               (TOKG // 16) * g + (HT // 16) * (hseg + 1)],
                num_idxs=HT, num_idxs_reg=HT, elem_size=D, single_packet=False,
            )
        psE = psG.tile([128, D], F32, tag=petag, name="psE")
        return G, psE

    def gather_wordsum(G, psE, cs):
        for c in cs:
            j, i = c // 16, c % 16
            nc.tensor.matmul(psE[64 * j:64 * (j + 1), :], lhsT=Ablk[i][:],
                             rhs=G[:, c, :], start=(i == 0), stop=(i == 15))

    def gather_finish(g, psE):
        enc = work.tile([128, D], BF16, tag="enc")
        nc.scalar.copy(out=enc[:], in_=psE[:])
        for j in range(2):
            pt = psG.tile([128, 128], BF16, tag="pgs", name="pt")
            nc.tensor.transpose(pt[:], enc[:, 128 * j:128 * (j + 1)], I128[:])
            nc.vector.tensor_copy(out=ET[j][:, 128 * g:128 * (g + 1)], in_=pt[:])
        for m in range(2):
            pw = psG.tile([128, 128], F32, tag="pgs", name="pw")
            nc.tensor.matmul(pw[:], lhsT=sbW[0][:, 128 * m:128 * (m + 1)],
                             rhs=ET[0][:, 128 * g:128 * (g + 1)], start=True, stop=False)
            nc.tensor.matmul(pw[:], lhsT=sbW[1][:, 128 * m:128 * (m + 1)],
                             rhs=ET[1][:, 128 * g:128 * (g + 1)], start=False, stop=True)
            nc.vector.tensor_copy(out=eWc[:, m, 128 * g:128 * (g + 1)], in_=pw[:])
        pgk = psG.tile([128, BK], F32, tag="pgk", name="pgk")
        nc.tensor.matmul(pgk[:], lhsT=ET[0][:, 128 * g:128 * (g + 1)], rhs=kT[0][:],
                         start=True, stop=False)
        nc.tensor.matmul(pgk[:], lhsT=ET[1][:, 128 * g:128 * (g + 1)], rhs=kT[1][:],
                         start=False, stop=True)
        nc.vector.tensor_tensor(out=gk0m[:, g, :], in0=pgk[:],
                                in1=sb_mb[:, g, :], op=OP.add)

    # ---- scan: two pipelined batch groups (b 0-7 | b 8-15) ----
    HB = BK // 2  # 256
    h = [hpool.tile([128, 2, HB], BF16, tag=f"h{gb}", name=f"h{gb}")
         for gb in range(2)]
    for gb in range(2):
        nc.vector.memset(h[gb][:], 0.0)

    def phase_a(t, gb, first=False):
        """Injections, h-dependent matmuls, gate, relu*sigmoid, upd, sq."""
        g, ds = t // 8, t % 8
        cg = 128 * g + 16 * ds + 8 * gb
        bks = slice(HB * gb, HB * (gb + 1))
        hg = h[gb]

        # h-independent PSUM preloads
        pshG = psH.tile([128, 2, HB], F32, tag=f"psh{gb}", name=f"psh{gb}")
        nc.tensor.matmul(pshG[:, :, :], lhsT=I128[:],
                         rhs=kvt[:, :, 8 * gb:8 * gb + 8, :], start=True, stop=False)
        for m in range(2):
            ew_bc = eWc[:, m, cg:cg + 8].unsqueeze(2).broadcast_to([128, 8, 32])
            nc.tensor.matmul(pshG[:, m, :], lhsT=I128[:], rhs=ew_bc,
                             start=False, stop=first and m == 1)
        psMt = psM.tile([128, BK], F32, tag=f"psm{gb}", name=f"psm{gb}")
        psg = psMt[0:8, 0:HB]
        off = 16 * ds + 8 * gb
        nc.tensor.matmul(psg, lhsT=I128[:, off:off + 8],
                         rhs=gk0m[:, g, bks], start=True, stop=first)

        # h-dependent matmuls (step 0 has h = 0: E^T h and U^T h vanish)
        if not first:
            nc.tensor.matmul(psg, lhsT=ET[0][:, cg:cg + 8], rhs=hg[:, 0, :],
                             start=False, stop=False)
            nc.tensor.matmul(psg, lhsT=ET[1][:, cg:cg + 8], rhs=hg[:, 1, :],
                             start=False, stop=True)
            for m in range(2):
                nc.tensor.matmul(pshG[:, m, :], lhsT=sbU[0][:, 128 * m:128 * (m + 1)],
                                 rhs=hg[:, 0, :], start=False, stop=False)
                nc.tensor.matmul(pshG[:, m, :], lhsT=sbU[1][:, 128 * m:128 * (m + 1)],
                                 rhs=hg[:, 1, :], start=False, stop=(m == 1))

        # gate: eg = exp(-logit); sigmoid folded into RELUSIG below. The
        # gate broadcast overwrites the psg columns (WAR after eg reads).
        eg = work.tile([8, HB], BF16, tag=f"eg{gb}", name=f"eg{gb}")
        nc.scalar.activation(eg[:], psg, AF.Exp, scale=-1.0)
        nc.tensor.matmul(psMt[:, 0:HB], lhsT=ones8[:], rhs=eg[:],
                         start=True, stop=True)

        # u = relu(psh) * sigmoid, split by de-half so the second relu
        # half overlaps the first MULSIG half.
        r = work.tile([128, 2, HB], BF16, tag=f"r{gb}", name=f"r{gb}")
        u = work.tile([128, 2, HB], BF16, tag=f"u{gb}", name=f"u{gb}")
        for m in range(2):
            nc.scalar.activation(r[:, m, :], pshG[:, m, :], AF.Relu)
            nc.vector._custom_dve(
                _MULSIG, out=u[:, m, :], in0=r[:, m, :],
                in1=psMt[:, 0:HB],
                s0=float(_RS_C0), s1=float(_RS_C1))
        # sq = (u+h)^2 fused (keeps the sumsq path one uninterruptible DVE
        # op); upd itself is only needed by hn much later.
        sq = work.tile([128, 2, HB], BF16, tag=f"sq{gb}", name=f"sq{gb}")
        nc.vector._custom_dve(_ADDSQ, out=sq[:], in0=u[:], in1=hg[:])
        return psMt, (u, hg), sq

    def phase_b(t, gb, psMt, uh, sq, last=False, first=False):
        """Norm tail: sumsq, rsqrt via ln/exp, inv broadcast, hn."""
        u, hg = uh
        if first:
            upd = u
        else:
            upd = work.tile([128, 2, HB], BF16, tag=f"upd{gb}", name=f"upd{gb}")
            nc.vector.tensor_tensor(out=upd[:], in0=u[:], in1=hg[:], op=OP.add)
        pss = psMt[0:1, HB:HB + HB]
        nc.tensor.matmul(pss, lhsT=ones128[:], rhs=sq[:, 0, :],
                         start=True, stop=False)
        nc.tensor.matmul(pss, lhsT=ones128[:], rhs=sq[:, 1, :],
                         start=False, stop=True)
        lns = psMt[32:33, HB:HB + HB]
        nc.scalar.activation(lns, pss, AF.Ln, bias=epsap[:])
        inv = work.tile([1, HB], BF16, tag=f"inv{gb}", name=f"inv{gb}")
        nc.scalar.activation(inv[:], lns, AF.Exp, scale=-0.5)
        if last:
            # final step: normalization is folded into the output drain
            # (inv applied per-partition after the transpose), so skip the
            # broadcast matmul and the hn multiply. f32 copy of inv so the
            # transposed column satisfies tensor_scalar's f32-scalar rule.
            invf = work.tile([1, HB], F32, tag=f"invf{gb}", name=f"invf{gb}")
            nc.scalar.activation(invf[:], lns, AF.Exp, scale=-0.5)
            return upd, invf
        psBI = psB.tile([128, HB], F32, tag=f"psb{gb}", name=f"psb{gb}")
        nc.tensor.matmul(psBI[:, :], lhsT=ones1[:], rhs=inv[:],
                         start=True, stop=True)
        hni = hpool.tile([128, 2, HB], BF16, tag=f"h{gb}", name=f"hn{gb}")
        nc.vector.tensor_tensor(
            out=hni[:], in0=upd[:],
            in1=psBI[:, :].unsqueeze(1).broadcast_to([128, 2, HB]),
            op=OP.mult)
        return hni

    # PE p-state warm-up: keep the PE continuously busy while the first
    # gather's DMA is in flight so the word-sum runs at full clock.
    warm = psB.tile([128, HB], F32, tag="psb1", name="warm")
    for w in range(8):
        nc.tensor.matmul(warm[:, :], lhsT=I128[:], rhs=kT[0][:, 0:HB],
                         start=(w == 0), stop=(w == 7))

    def gather_group(g, split=1):
        G, psE = gather_dma(g, split=split)
        gather_wordsum(G, psE, range(L))
        gather_finish(g, psE)

    gather_group(0, split=8)
    nc.sync.dma_start(out=kvt[:], in_=kvtf[:])
    # Groups 1..7 stream under the scan: DMA issued up front / at the block
    # start, the word-sum spread 4 slots per step, finalize at the block
    # end (group g's outputs are first needed at step 8*(g-1)). Group 1's
    # psE parks in the pgk bank so it can coexist with group 2's.
    pend = [(1, 0) + gather_dma(1, petag="pgk")]
    for t in range(S):
        ti = t % 8
        if ti == 0 and t // 8 + 2 < NG:
            pend.append((t // 8 + 2, t) + gather_dma(t // 8 + 2))
        for ent in pend[:]:
            gp, t0, G, psE = ent
            k = t - t0
            gather_wordsum(G, psE, range(4 * k, 4 * k + 4))
            if k == 7:
                gather_finish(gp, psE)
                pend.remove(ent)
        st = [phase_a(t, 0, first=(t == 0)), phase_a(t, 1, first=(t == 0))]
        h = [phase_b(t, gb, *st[gb], last=(t == S - 1), first=(t == 0))
             for gb in range(2)]

    # ---- output: transpose upd^T and scale rows by inv (rows = bk after
    # the transpose, so the final normalize is a per-partition multiply) ----
    for q in range(4):
        gb, half = q // 2, q % 2
        updf, invf = h[gb]
        pti = psG.tile([128, 1], F32, tag="pgk", name="pti")
        nc.tensor.transpose(pti[:], invf[0:1, 128 * half:128 * half + 128],
                            one1f[:])
        ho = work.tile([128, D], F32, tag="ho")
        for j in range(2):
            # alternate PSUM tags (psb banks are free after the last step)
            # so the two transposes don't serialize on one buffer.
            if j == 0:
                pt = psG.tile([128, 128], BF16, tag="pgs", name="ptout")
            else:
                pt = psB.tile([128, 128], BF16, tag=f"psb{gb}", name="ptout2")
            nc.tensor.transpose(pt[:], updf[:, j, 128 * half:128 * half + 128],
                                I128[:])
            nc.vector.tensor_scalar(out=ho[:, 128 * j:128 * (j + 1)], in0=pt[:],
                                    scalar1=pti[:, 0:1], scalar2=None,
                                    op0=OP.mult)
        nc.sync.dma_start(out=hout[128 * q:128 * (q + 1), :], in_=ho[:])

    ctx.close()


def _prep_core(pr, mask, keys_c, emb):
    """Host-side marshaling for one core's shard."""
    uniq, inv = np.unique(pr, return_inverse=True)
    assert len(uniq) <= TABLE_ROWS
    table = np.zeros((TABLE_ROWS, D), dtype=ml_dtypes.bfloat16)
    table[: len(uniq)] = emb[uniq].astype(ml_dtypes.bfloat16)
    ranks = inv.reshape(BL, S, L).astype(np.int16)

    # token order per group g: i = (ds*16 + b)*32 + w
    idx_groups = []
    for g in range(NG):
        blk = ranks[:, 8 * g:8 * (g + 1), :]          # [b, ds, w]
        lst = blk.transpose(1, 0, 2).reshape(-1)      # [(ds, b, w)] length 4096
        idx_groups.append(np.tile(lst.reshape(TOKG // 16, 16).T, (8, 1)))
    idx16 = np.concatenate(idx_groups, axis=1).astype(np.int16)  # [128, NG*256]

    keysT = np.ascontiguousarray(
        keys_c.reshape(BK, D).T).astype(ml_dtypes.bfloat16)      # [256, 512]


    # mbig[r=(ds,gb,b), g, c=(bglob,k)] gate-logit offsets:
    #   0    where the gate is live (diagonal batch, unmasked step),
    #   -35  diagonal but step-masked  (exp(-logit) huge -> sigmoid ~ 0),
    #   +35  off-diagonal              (exp(-logit) ~ 0, vanishes in the sum).
    m = mask.astype(bool)                                        # [16, 64]
    mb = np.full((128, NG, BK), 35.0, np.float32)
    r = np.arange(128)
    ds_, gb_, b_ = r // 16, (r % 16) // 8, r % 8
    bglob = 8 * gb_ + b_
    for g in range(NG):
        t_ = 8 * g + ds_                                         # [128]
        ok_row = m[bglob, t_]                                    # [128]
        cols = (np.arange(BK) // K)[None, :] == bglob[:, None]   # [128, BK]
        gm_ = mb[:, g, :]
        gm_[cols] = np.where(np.repeat(ok_row, K), 0.0, -35.0)
        mb[:, g, :] = gm_
    return table, idx16, keysT, mb.astype(ml_dtypes.bfloat16)


def kernel(prgrph, prgrph_mask, keys, embedding_matrix, U, V, W):
    prgrph = np.asarray(prgrph)
    prgrph_mask = np.asarray(prgrph_mask)
    keys = np.asarray(keys, dtype=np.float32)
    emb = np.asarray(embedding_matrix, dtype=np.float32)
    U = np.asarray(U, dtype=np.float32)
    V = np.asarray(V, dtype=np.float32)
    W = np.asarray(W, dtype=np.float32)

    if "nc" not in _CACHED:
        _CACHED["nc"] = _build_program()
    nc = _CACHED["nc"]

    Ub, Vb, Wb = (x.astype(ml_dtypes.bfloat16) for x in (U, V, W))

    in_maps = []
    for c in range(NC):
        sl = slice(BL * c, BL * (c + 1))
        table, idx16, keysT, mb = _prep_core(
            prgrph[sl], prgrph_mask[sl, :, 0], keys[sl], emb)
        kv = keys[sl].reshape(BK, D) @ V                       # [bk, de]
        kvtf = np.ascontiguousarray(
            kv.T.reshape(2, 128, BL, K).transpose(1, 0, 2, 3)).astype(ml_dtypes.bfloat16)
        in_maps.append({
            "table": table, "idx16": idx16, "keysT": keysT,
            "Umat": Ub, "Vmat": Vb, "Wmat": Wb,
            "mbig": mb, "kvtf": kvtf,
        })

    res = run_bass_kernel_spmd(nc, in_maps, core_ids=list(range(NC)))
    out = np.concatenate(
        [res.results[c]["hout"].reshape(BL, K, D) for c in range(NC)], axis=0)
    return out.astype(np.float32)
